# revision 14
# baseline (speedup 1.0000x reference)
"""BiMamba (bidirectional Mamba block) Trainium2 kernel.

Contract: kernel(**inputs) takes the full (unsharded) numpy inputs of the
reference and returns the full (2, 4096, 1024) float32 output.

Sharding: 8 cores = 2 batches x 4 channel-groups of 512 d_inner channels.
Each core runs both scan directions for its channel slice; the x_dbl
reduction over d_inner is an on-chip AllReduce within each batch's 4-core
group; the host sums the four partial out-projections per batch.

Key algebraic facts used:
  * xz for the reverse direction is the L-flip of the forward xz, so the
    input projection is computed once.
  * (y_f + flip(y_r)) @ W_out.T == out_f + flip(out_r), so one output
    projection suffices.

Performance structure (engine balance per scan chunk):
  * Pool (gpsimd) runs the 64 tensor_tensor_scan ops (the serial core).
  * DVE runs the bf16 TensorTensor mults (2x_1p packed mode).
  * Act runs the exp/softplus and most PSUM->SBUF broadcast copies.
  * PE accumulates y over the 16 states via identity matmuls into PSUM,
    plus the projections.
  * All DMA uses contiguous descriptors (reversals happen in SBUF reads).
"""

import os
import sys

import numpy as np

sys.path.insert(0, "/opt/trn_rl_repo")

B, L, DM, DI, DS, DR, DC = 2, 4096, 1024, 2048, 16, 64, 4
CH = 512          # d_inner channels per core
NCH = CH // 128   # channel tiles per core
T1 = 512          # pass-1 (projection/conv) token chunk
NC1 = L // T1
T2 = 512          # pass-2 (scan) token chunk
NC2 = L // T2

# engine assignment tuning: V=DVE, P=Pool(gpsimd), A=Act
# (scans must run on DVE: walrus cannot lower tensor_tensor_scan on Pool)
CFG = dict(
    bcopy=os.environ.get("CFG_BCOPY", "A" * 16),     # per s: B broadcast copy
    ccopy=os.environ.get("CFG_CCOPY", "A" * 16),     # per s: C broadcast copy
    carry=os.environ.get("CFG_CARRY", "V"),          # batched carry copies
    bt=os.environ.get("CFG_BT", ""),                 # per (s*NCH+m): bt engine
    cm=os.environ.get("CFG_CM", ""),                 # per (s*NCH+m): cmul engine
)
if not CFG["bt"]:
    CFG["bt"] = "".join("V" if (s + m) % 2 else "P"
                        for s in range(DS) for m in range(NCH))
if not CFG["cm"]:
    CFG["cm"] = "".join("V" if (s + m) % 2 == 0 else "P"
                        for s in range(DS) for m in range(NCH))

_COMPILED = [None]


def _split_sync_waits(nc, mybir, max_waits=1):
    """walrus in this environment rejects >1 sync wait per instruction;
    hoist excess waits onto dedicated same-engine NOPs."""
    uid = [0]
    for f in nc.m.functions:
        for bb in f.blocks:
            new = []
            dirty = False
            for inst in bb.instructions:
                si = inst.sync_info
                if si is not None and len(si.on_wait) > max_waits:
                    waits = list(si.on_wait)
                    keep = waits[len(waits) - max_waits:]
                    hoist = waits[: len(waits) - max_waits]
                    for i in range(0, len(hoist), max_waits):
                        uid[0] += 1
                        nop = mybir.InstNoOp(
                            name=f"splitwait-{id(nc)}-{uid[0]}", engine=inst.engine
                        )
                        nop.sync_info = mybir.SyncInfo(
                            on_wait=hoist[i : i + max_waits], on_update=[]
                        )
                        nc.register_instruction(nop, overwrite=True)
                        new.append(nop)
                    inst.sync_info = mybir.SyncInfo(
                        on_wait=keep, on_update=list(si.on_update)
                    )
                    dirty = True
                new.append(inst)
            if dirty:
                bb.instructions = new


def _build_program(debug=False, collective=True):
    import concourse.bass as bass
    import concourse.tile as tile
    from concourse import mybir

    f32 = mybir.dt.float32
    f32r = mybir.dt.float32r
    bf16 = mybir.dt.bfloat16
    AF = mybir.ActivationFunctionType
    OP = mybir.AluOpType

    nc = bass.Bass("TRN2", target_bir_lowering=False, debug=False, num_devices=8)

    # ---- external inputs (per-core shards prepared on host) ----
    hT = nc.dram_tensor("hT", [DM, L], f32r, kind="ExternalInput")
    winxT = nc.dram_tensor("winxT", [DM, CH], f32r, kind="ExternalInput")
    winzT = nc.dram_tensor("winzT", [DM, CH], f32r, kind="ExternalInput")
    woutT_d = nc.dram_tensor("woutT", [CH, DM], bf16, kind="ExternalInput")
    sel_d = nc.dram_tensor("sel", [48, DS * 128], bf16, kind="ExternalInput")
    ident_d = nc.dram_tensor("ident", [128, 128], bf16, kind="ExternalInput")
    wx_d = {}
    wdt_d = {}
    A_d = {}
    cw_d = {}
    cb_d = {}
    db_d = {}
    D_d = {}
    for d in ("f", "r"):
        wx_d[d] = nc.dram_tensor(f"wx_{d}", [CH, 128], bf16, kind="ExternalInput")
        wdt_d[d] = nc.dram_tensor(f"wdt_{d}", [DR, CH], bf16, kind="ExternalInput")
        A_d[d] = nc.dram_tensor(f"A_{d}", [128, NCH * DS], f32, kind="ExternalInput")
        cw_d[d] = nc.dram_tensor(f"cwdiag_{d}", [NCH * DC, 128, 128], bf16,
                                 kind="ExternalInput")
        cb_d[d] = nc.dram_tensor(f"cb_{d}", [128, NCH], f32, kind="ExternalInput")
        db_d[d] = nc.dram_tensor(f"db_{d}", [128, NCH], f32, kind="ExternalInput")
        D_d[d] = nc.dram_tensor(f"D_{d}", [128, NCH], f32, kind="ExternalInput")

    pout = nc.dram_tensor("pout", [L, DM], bf16, kind="ExternalOutput")
    dbg = {}
    if debug:
        dbg["xc_f"] = nc.dram_tensor("dbg_xc_f", [NCH, 128, L], bf16, kind="ExternalOutput")
        dbg["xc_r"] = nc.dram_tensor("dbg_xc_r", [NCH, 128, L], bf16, kind="ExternalOutput")
        dbg["xdbl_f"] = nc.dram_tensor("dbg_xdbl_f", [96, L], f32, kind="ExternalOutput")
        dbg["xdbl_r"] = nc.dram_tensor("dbg_xdbl_r", [96, L], f32, kind="ExternalOutput")
        dbg["dt_f"] = nc.dram_tensor("dbg_dt_f", [NCH, 128, L], f32, kind="ExternalOutput")
        dbg["y_f"] = nc.dram_tensor("dbg_y_f", [NCH, 128, L], bf16, kind="ExternalOutput")
        dbg["siluz"] = nc.dram_tensor("dbg_siluz", [NCH, 128, L], bf16, kind="ExternalOutput")

    with tile.TileContext(nc, num_cores=8) as tc:
        _build_tile_program(
            nc, tc, tile, mybir, f32, f32r, bf16, AF, OP,
            hT, winxT, winzT, woutT_d, sel_d, ident_d, wx_d, wdt_d, A_d, cw_d,
            cb_d, db_d, D_d, pout, dbg, collective,
        )

    _split_sync_waits(nc, mybir)
    return nc


def _build_tile_program(
    nc, tc, tile, mybir, f32, f32r, bf16, AF, OP,
    hT, winxT, winzT, woutT_d, sel_d, ident_d, wx_d, wdt_d, A_d, cw_d, cb_d,
    db_d, D_d, pout, dbg, collective=True,
):
    from contextlib import ExitStack

    MM = nc.tensor.matmul
    ACT = nc.scalar.activation
    TT = nc.vector.tensor_tensor
    STT = nc.vector.scalar_tensor_tensor
    TSMUL = nc.vector.tensor_scalar_mul

    def veng(code):
        return nc.vector if code == "V" else nc.gpsimd

    def bcopy(code, out, in_):
        """PSUM f32 -> SBUF copy on the chosen engine."""
        if code == "A":
            ACT(out, in_, AF.Copy)
        else:
            veng(code).tensor_copy(out, in_)

    ctx = ExitStack()
    with ctx:
        # -------- persistent pools --------
        pers = ctx.enter_context(tc.tile_pool(name="pers", bufs=1))
        dram = ctx.enter_context(tc.tile_pool(name="dram", bufs=1, space="DRAM"))

        wout_sb = pers.tile([128, NCH, DM], bf16)
        nc.sync.dma_start(wout_sb[:], woutT_d.ap().rearrange("(k p) n -> p k n", p=128))
        sel_sb = pers.tile([48, DS * 128], bf16)
        nc.sync.dma_start(sel_sb[:], sel_d[:])
        ident_sb = pers.tile([128, 128], bf16)
        nc.sync.dma_start(ident_sb[:], ident_d[:])
        xdbl = {}      # bf16 [128, L]: rows [0:16]=B, [32:48]=C, [64:128]=dt-rank
        carry = {}
        wdt_sb = {}
        A_sb = {}
        db_sb = {}
        D_sb = {}
        for d in ("f", "r"):
            xdbl[d] = pers.tile([128, L], bf16, name=f"xdbl_{d}")
            carry[d] = pers.tile([128, NCH, DS], bf16, name=f"carry_{d}")
            nc.vector.memset(carry[d][:], 0.0)
            wdt_sb[d] = pers.tile([128, CH], bf16, name=f"wdt_sb_{d}")
            nc.sync.dma_start(wdt_sb[d][DR:128, :], wdt_d[d][:])
            A_sb[d] = pers.tile([128, NCH, DS], f32, name=f"A_sb_{d}")
            nc.sync.dma_start(A_sb[d][:], A_d[d].ap().rearrange("p (m s) -> p m s", m=NCH))
            db_sb[d] = pers.tile([128, NCH], f32, name=f"db_sb_{d}")
            nc.sync.dma_start(db_sb[d][:], db_d[d][:])
            D_sb[d] = pers.tile([128, NCH], f32, name=f"D_sb_{d}")
            nc.sync.dma_start(D_sb[d][:], D_d[d][:])
        ones = pers.tile([128, 1], f32)
        nc.vector.memset(ones[:], 1.0)

        # DRAM spill buffers (per-core local HBM); all in ORIGINAL time order
        # for the forward direction; xr/sz are original-time too (pass 2r
        # flips with reversed SBUF reads).  ygr is in flipped time.
        xf_dram = dram.tile([NCH, 128, L], bf16)
        xr_dram = dram.tile([NCH, 128, L], bf16)
        sz_dram = dram.tile([NCH, 128, L], bf16)
        ygr_dram = dram.tile([NCH, 128, L], bf16)
        # AllReduce staging: [dir, 128 rows, L] f32; rows as xdbl layout.
        # dir 0 = forward (original time), dir 1 = reverse (flipped time).
        ar_in = dram.tile([2, 128, L], f32)
        ar_out = dram.tile([2, 128, L], f32)

        # ================= PASS 1: in_proj + conv + silu + partial x_dbl ====
        with tc.tile_pool(name="p1", bufs=1) as p1, \
             tc.tile_pool(name="p1psum", bufs=1, space="PSUM") as p1psum:
            winx_sb = p1.tile([128, DM // 128, CH], f32r)
            nc.sync.dma_start(winx_sb[:], winxT.ap().rearrange("(k p) n -> p k n", p=128))
            winz_sb = p1.tile([128, DM // 128, CH], f32r)
            nc.sync.dma_start(winz_sb[:], winzT.ap().rearrange("(k p) n -> p k n", p=128))
            wx_sb = {}
            cw_sb = {}
            cb_sb = {}
            for d in ("f", "r"):
                wx_sb[d] = p1.tile([128, NCH, 128], bf16, name=f"wx_sb_{d}")
                nc.sync.dma_start(wx_sb[d][:], wx_d[d].ap().rearrange("(m p) n -> p m n", p=128))
                # diag(conv_w[:, j]) per channel tile: stationaries for the
                # depthwise conv as 4 accumulated PE matmuls
                cw_sb[d] = p1.tile([128, NCH * DC, 128], bf16, name=f"cw_sb_{d}")
                nc.sync.dma_start(cw_sb[d][:], cw_d[d].ap().rearrange("k p n -> p k n"))
                cb_sb[d] = p1.tile([128, NCH], f32, name=f"cb_sb_{d}")
                nc.sync.dma_start(cb_sb[d][:], cb_d[d][:])

            hT_r = hT.ap().rearrange("(k p) l -> p k l", p=128)
            prev_xe = [None] * NCH

            def conv_dir(cc, d, xe_list):
                """Causal (d=f) / anti-causal (d=r) depthwise conv + silu on
                original-time chunk cc, using extended tiles [3|T1|3].
                Conv taps are accumulated diag-matmuls on PE.  Returns bf16
                silu'd tiles."""
                out = []
                for m in range(NCH):
                    cps = p1psum.tile([128, T1], f32, tag="cps", bufs=2,
                                      name=f"cps{m}_{d}_{cc}")
                    xe = xe_list[m]
                    for j in range(DC):
                        off = j if d == "f" else (6 - j)
                        MM(cps[:], cw_sb[d][:, m * DC + j, :],
                           xe[:, off : off + T1],
                           start=(j == 0), stop=(j == DC - 1))
                    xcb = p1.tile([128, T1], bf16, tag=f"xcb{m}_{d}", bufs=2,
                                  name=f"xcb{m}_{d}_{cc}")
                    ACT(xcb[:], cps[:], AF.Silu, bias=cb_sb[d][:, m : m + 1])
                    out.append(xcb)
                return out

            def xdbl_chunk(cc, d, xc_tiles):
                # psum rows laid out as [B 0:16 | C 32:48 | dt 64:128]
                # (W_x rows reordered+padded on host); full 128 rows go to AR.
                ps = p1psum.tile([128, T1], f32, tag="psx", bufs=2,
                                 name=f"psx_{d}_{cc}")
                for m in range(NCH):
                    MM(ps[:], wx_sb[d][:, m, :], xc_tiles[m][:],
                       start=(m == 0), stop=(m == NCH - 1))
                stage = p1.tile([128, T1], f32, tag="arstage", bufs=2,
                                name=f"arstage_{d}_{cc}")
                if d == "f":
                    ACT(stage[:], ps[:], AF.Copy)
                    nc.sync.dma_start(
                        ar_in[0, :, cc * T1 : (cc + 1) * T1], stage[:]
                    )
                else:
                    nc.vector.tensor_copy(stage[:], ps[:, ::-1])
                    nc.sync.dma_start(
                        ar_in[1, :, L - (cc + 1) * T1 : L - cc * T1], stage[:]
                    )

            def spill_chunk(cc, d, xc_tiles):
                x_dram = xf_dram if d == "f" else xr_dram
                for m in range(NCH):
                    nc.sync.dma_start(
                        x_dram[m, :, cc * T1 : (cc + 1) * T1], xc_tiles[m][:]
                    )
                    if dbg:
                        key = "xc_f" if d == "f" else "xc_r"
                        nc.sync.dma_start(
                            dbg[key][m, :, cc * T1 : (cc + 1) * T1], xc_tiles[m][:]
                        )

            def finish_reverse(cc, xe_list):
                xcr = conv_dir(cc, "r", xe_list)
                xdbl_chunk(cc, "r", xcr)
                spill_chunk(cc, "r", xcr)

            for c in range(NC1):
                hTt = p1.tile([128, DM // 128, T1], f32r, tag="hTt", bufs=1,
                              name=f"hTt_{c}")
                nc.sync.dma_start(hTt[:], hT_r[:, :, c * T1 : (c + 1) * T1])

                # x part (extended with halos) and z part (-> silu -> spill)
                cur_xe = []
                for m in range(NCH):
                    ps = p1psum.tile([128, T1], f32, tag="ps_ip", bufs=2,
                                     name=f"psx_{c}_{m}")
                    for ko in range(DM // 128):
                        MM(ps[:], winx_sb[:, ko, m * 128 : (m + 1) * 128],
                           hTt[:, ko, :], start=(ko == 0), stop=(ko == DM // 128 - 1))
                    xe = p1.tile([128, T1 + 6], bf16, tag=f"xe{m}", bufs=3,
                                 name=f"xe{m}_{c}")
                    ACT(xe[:, 3 : 3 + T1], ps[:], AF.Copy)
                    if c == 0:
                        nc.vector.memset(xe[:, 0:3], 0.0)
                    else:
                        nc.vector.tensor_copy(xe[:, 0:3], prev_xe[m][:, T1 : T1 + 3])
                    cur_xe.append(xe)
                for m in range(NCH):
                    ps = p1psum.tile([128, T1], f32, tag="ps_ip", bufs=2,
                                     name=f"psz_{c}_{m}")
                    for ko in range(DM // 128):
                        MM(ps[:], winz_sb[:, ko, m * 128 : (m + 1) * 128],
                           hTt[:, ko, :], start=(ko == 0), stop=(ko == DM // 128 - 1))
                    zs = p1.tile([128, T1], bf16, tag=f"zs{m}", bufs=2,
                                 name=f"zs{m}_{c}")
                    ACT(zs[:], ps[:], AF.Silu)
                    nc.sync.dma_start(sz_dram[m, :, c * T1 : (c + 1) * T1], zs[:])
                    if dbg:
                        nc.sync.dma_start(
                            dbg["siluz"][m, :, c * T1 : (c + 1) * T1], zs[:]
                        )

                if c > 0:
                    # fill previous chunk's right halo, then do its reverse conv
                    for m in range(NCH):
                        nc.vector.tensor_copy(
                            prev_xe[m][:, T1 + 3 : T1 + 6], cur_xe[m][:, 3:6]
                        )
                    finish_reverse(c - 1, prev_xe)

                # forward conv on current chunk
                xcf = conv_dir(c, "f", cur_xe)
                xdbl_chunk(c, "f", xcf)
                spill_chunk(c, "f", xcf)

                prev_xe = cur_xe

            for m in range(NCH):
                nc.vector.memset(prev_xe[m][:, T1 + 3 : T1 + 6], 0.0)
            finish_reverse(NC1 - 1, prev_xe)

            # -------- AllReduce of x_dbl over the 4 cores of this batch ----
            if collective:
                nc.gpsimd.collective_compute(
                    "AllReduce", OP.add,
                    replica_groups=[[0, 1, 2, 3], [4, 5, 6, 7]],
                    ins=[ar_in[:].opt()], outs=[ar_out[:].opt()],
                )
            else:
                nc.gpsimd.dma_start(ar_out[:], ar_in[:])
            # cast-readback f32 -> bf16 into SBUF (gpsimd DMAs may cast)
            for di, d in enumerate(("f", "r")):
                nc.gpsimd.dma_start(xdbl[d][:], ar_out[di, :, :])
            if dbg:
                for di, d in enumerate(("f", "r")):
                    nc.sync.dma_start(dbg[f"xdbl_{d}"][0:64, :], ar_out[di, 64:128, :])
                    nc.sync.dma_start(dbg[f"xdbl_{d}"][64:80, :], ar_out[di, 0:16, :])
                    nc.sync.dma_start(dbg[f"xdbl_{d}"][80:96, :], ar_out[di, 32:48, :])

        # ================= PASS 2: dt + selective scan (+gating, out_proj) ==
        def scan_pass(d, p2, p2psum, ytot_cb, mmt_bufs=3):
            """d: 'f' or 'r'.  'r' reads x/sz spills (original time) with
            reversed SBUF access; everything else runs in flipped time.
            ytot_cb(c2, yg_tiles): consumes gated y tiles for chunk c2."""
            x_dram = xf_dram if d == "f" else xr_dram
            rev = (lambda ap: ap) if d == "f" else (lambda ap: ap[:, ::-1])
            for c2 in range(NC2):
                sl = slice(c2 * T2, (c2 + 1) * T2)
                osl = sl if d == "f" else slice(L - (c2 + 1) * T2, L - c2 * T2)
                # ---- dt projection + softplus (f32 path) ----
                dt_sb = []
                for m in range(NCH):
                    psd = p2psum.tile([128, T2], f32, tag="mmt", bufs=mmt_bufs,
                                      name=f"psd_{d}_{c2}_{m}")
                    MM(psd[:], wdt_sb[d][DR:128, m * 128 : (m + 1) * 128],
                       xdbl[d][DR:128, sl], start=True, stop=True)
                    et = p2.tile([128, T2], f32, tag="et", bufs=2,
                                 name=f"et_{d}_{c2}_{m}")
                    ACT(et[:], psd[:], AF.Exp, bias=db_sb[d][:, m : m + 1])
                    dt = p2.tile([128, T2], f32, tag=f"dt{m}", bufs=2,
                                 name=f"dt{m}_{d}_{c2}")
                    ACT(dt[:], et[:], AF.Ln, bias=ones[:])
                    dt_sb.append(dt)
                    if dbg and d == "f":
                        nc.sync.dma_start(dbg["dt_f"][m, :, sl], dt[:])
                # ---- x load (bf16) + wd = dt*x + silu(z) load ----
                xd = []
                wd = []
                szt = []
                for m in range(NCH):
                    xt = p2.tile([128, T2], bf16, tag=f"xd{m}", bufs=2,
                                 name=f"xd{m}_{d}_{c2}")
                    nc.sync.dma_start(xt[:], x_dram[m, :, osl])
                    xd.append(xt)
                    wt = p2.tile([128, T2], bf16, tag=f"wd{m}", bufs=2,
                                 name=f"wd{m}_{d}_{c2}")
                    TT(wt[:], dt_sb[m][:], rev(xt[:]), OP.mult)
                    wd.append(wt)
                    sz = p2.tile([128, T2], bf16, tag=f"sz{m}", bufs=2,
                                 name=f"sz{m}_{d}_{c2}")
                    nc.sync.dma_start(sz[:], sz_dram[m, :, osl])
                    szt.append(sz)
                # ---- selective scan over 16 states ----
                yps = [p2psum.tile([128, T2], f32, tag=f"yp{m}", bufs=1,
                                   name=f"yp{m}_{d}_{c2}") for m in range(NCH)]
                for s in range(DS):
                    Bbp = p2psum.tile([128, T2], f32, tag="mmt", bufs=mmt_bufs,
                                      name=f"Bbp_{d}_{c2}_{s}")
                    MM(Bbp[:], sel_sb[0:DS, s * 128 : (s + 1) * 128],
                       xdbl[d][0:DS, sl], start=True, stop=True)
                    Bb = p2.tile([128, T2], bf16, tag="Bbs", bufs=2,
                                 name=f"Bb_{d}_{c2}_{s}")
                    bcopy(CFG["bcopy"][s], Bb[:], Bbp[:])
                    Cbp = p2psum.tile([128, T2], f32, tag="mmt", bufs=mmt_bufs,
                                      name=f"Cbp_{d}_{c2}_{s}")
                    MM(Cbp[:], sel_sb[32 : 32 + DS, s * 128 : (s + 1) * 128],
                       xdbl[d][32 : 32 + DS, sl], start=True, stop=True)
                    Cb = p2.tile([128, T2], bf16, tag="Cbs", bufs=2,
                                 name=f"Cb_{d}_{c2}_{s}")
                    bcopy(CFG["ccopy"][s], Cb[:], Cbp[:])
                    bt = []
                    for m in range(NCH):
                        b = p2.tile([128, T2], bf16, tag=f"bt{m}", bufs=2,
                                    name=f"bt_{d}_{c2}_{s}_{m}")
                        veng(CFG["bt"][s * NCH + m]).tensor_tensor(
                            b[:], wd[m][:], Bb[:], OP.mult)
                        bt.append(b)
                    dAs = []
                    for m in range(NCH):
                        dA = p2.tile([128, T2], f32, tag=f"dA{m}", bufs=2,
                                     name=f"dA_{d}_{c2}_{s}_{m}")
                        ACT(dA[:], dt_sb[m][:], AF.Exp,
                            scale=A_sb[d][:, m, s : s + 1])
                        dAs.append(dA)
                    # per-state hs tile holding all 4 channel groups, so the
                    # chunk-boundary carry is ONE strided copy per state
                    hs = p2.tile([128, NCH, T2], bf16, tag="hs", bufs=2,
                                 name=f"hs_{d}_{c2}_{s}")
                    for m in range(NCH):
                        nc.vector.tensor_tensor_scan(
                            hs[:, m, :], dAs[m][:], bt[m][:],
                            carry[d][:, m, s : s + 1], OP.mult, OP.add)
                    veng(CFG["carry"]).tensor_copy(
                        carry[d][:, :, s : s + 1], hs[:, :, T2 - 1 : T2])
                    for m in range(NCH):
                        cm = p2.tile([128, T2], bf16, tag=f"cm{m}", bufs=2,
                                     name=f"cm_{d}_{c2}_{s}_{m}")
                        veng(CFG["cm"][s * NCH + m]).tensor_tensor(
                            cm[:], hs[:, m, :], Cb[:], OP.mult)
                        MM(yps[m][:], ident_sb[:], cm[:],
                           start=(s == 0), stop=(s == DS - 1))
                # ---- gating: y = (ypsum + x*D) * silu(z) ----
                yg = []
                for m in range(NCH):
                    y1 = p2.tile([128, T2], bf16, tag=f"y1{m}", bufs=2,
                                 name=f"y1_{d}_{c2}_{m}")
                    STT(y1[:], rev(xd[m][:]), D_sb[d][:, m : m + 1], yps[m][:],
                        OP.mult, OP.add)
                    yt = p2.tile([128, T2], bf16, tag=f"yg{m}", bufs=2,
                                 name=f"yg_{d}_{c2}_{m}")
                    TT(yt[:], y1[:], rev(szt[m][:]), OP.mult)
                    yg.append(yt)
                ytot_cb(c2, yg)

        # ---- pass 2r: reverse direction, spill gated y (flipped time) ----
        with tc.tile_pool(name="p2r", bufs=1) as p2r, \
             tc.tile_pool(name="p2rpsum", bufs=1, space="PSUM") as p2rpsum:

            def spill_ygr(c2, yg):
                for m in range(NCH):
                    nc.sync.dma_start(
                        ygr_dram[m, :, c2 * T2 : (c2 + 1) * T2], yg[m][:]
                    )

            scan_pass("r", p2r, p2rpsum, spill_ygr)

        # ---- pass 2f: forward + combine + out_proj ----
        with tc.tile_pool(name="p2f", bufs=1) as p2f, \
             tc.tile_pool(name="p2fpsum", bufs=1, space="PSUM") as p2fpsum:

            def combine_out(c2, yg):
                ytot = []
                for m in range(NCH):
                    ygr_t = p2f.tile([128, T2], bf16, tag=f"ygr{m}", bufs=2,
                                     name=f"ygr{m}_{c2}")
                    nc.sync.dma_start(
                        ygr_t[:], ygr_dram[m, :, L - (c2 + 1) * T2 : L - c2 * T2]
                    )
                    yt2 = p2f.tile([128, T2], bf16, tag=f"ytot{m}", bufs=2,
                                   name=f"ytot{m}_{c2}")
                    TT(yt2[:], yg[m][:], ygr_t[:, ::-1], OP.add)
                    ytot.append(yt2)
                    if dbg:
                        nc.sync.dma_start(
                            dbg["y_f"][m, :, c2 * T2 : (c2 + 1) * T2], yg[m][:]
                        )
                for mt in range(T2 // 128):
                    ob = p2f.tile([128, DM], bf16, tag="ob", bufs=2,
                                  name=f"ob_{c2}_{mt}")
                    for nh in range(DM // 512):
                        po = p2fpsum.tile([128, 512], f32, tag="po", bufs=2,
                                          name=f"po_{c2}_{mt}_{nh}")
                        for k in range(NCH):
                            MM(po[:], ytot[k][:, mt * 128 : (mt + 1) * 128],
                               wout_sb[:, k, nh * 512 : (nh + 1) * 512],
                               start=(k == 0), stop=(k == NCH - 1))
                        ACT(ob[:, nh * 512 : (nh + 1) * 512], po[:], AF.Copy)
                    nc.sync.dma_start(
                        pout[c2 * T2 + mt * 128 : c2 * T2 + (mt + 1) * 128, :],
                        ob[:],
                    )

            scan_pass("f", p2f, p2fpsum, combine_out, mmt_bufs=2)


def _host_prep(inputs):
    """Slice/transpose the full inputs into the 8 per-core input maps."""
    import ml_dtypes
    bf = ml_dtypes.bfloat16

    h = np.asarray(inputs["hidden_states"], np.float32)
    W_in = np.asarray(inputs["W_in"], np.float32)
    W_out = np.asarray(inputs["W_out"], np.float32)

    sel = np.zeros((48, DS * 128), np.float32)
    for s in range(DS):
        sel[s, s * 128 : (s + 1) * 128] = 1.0
        sel[32 + s, s * 128 : (s + 1) * 128] = 1.0

    maps = []
    for core in range(8):
        b, g = divmod(core, 4)
        c0 = g * CH
        m = {
            "hT": np.ascontiguousarray(h[b].T),
            "winxT": np.ascontiguousarray(W_in[c0 : c0 + CH, :].T),
            "winzT": np.ascontiguousarray(W_in[DI + c0 : DI + c0 + CH, :].T),
            "woutT": np.ascontiguousarray(W_out[:, c0 : c0 + CH].T).astype(bf),
            "sel": sel.astype(bf),
            "ident": np.eye(128, dtype=np.float32).astype(bf),
        }
        for d in ("f", "r"):
            sfx = f"_{d}"
            W_x = np.asarray(inputs[f"W_x{sfx}"], np.float32)
            W_dt = np.asarray(inputs[f"W_dt{sfx}"], np.float32)
            A = -np.exp(np.asarray(inputs[f"A_log{sfx}"], np.float64)).astype(np.float32)
            cw = np.asarray(inputs[f"conv_w{sfx}"], np.float32)
            cb = np.asarray(inputs[f"conv_b{sfx}"], np.float32)
            db = np.asarray(inputs[f"b_dt{sfx}"], np.float32)
            Dp = np.asarray(inputs[f"D{sfx}"], np.float32)
            wx_re = np.zeros((CH, 128), np.float32)
            wx_re[:, 0:DS] = W_x[DR : DR + DS, c0 : c0 + CH].T        # B rows
            wx_re[:, 32 : 32 + DS] = W_x[DR + DS : 96, c0 : c0 + CH].T  # C rows
            wx_re[:, DR:128] = W_x[0:DR, c0 : c0 + CH].T              # dt-rank rows
            m[f"wx{sfx}"] = wx_re.astype(bf)
            m[f"wdt{sfx}"] = np.ascontiguousarray(W_dt[c0 : c0 + CH, :].T).astype(bf)
            # (CH, DS) -> (128, NCH, DS) -> (128, NCH*DS)
            m[f"A{sfx}"] = np.ascontiguousarray(
                A[c0 : c0 + CH].reshape(NCH, 128, DS).transpose(1, 0, 2).reshape(128, NCH * DS)
            )
            cwd = np.zeros((NCH * DC, 128, 128), np.float32)
            cwc = cw[c0 : c0 + CH].reshape(NCH, 128, DC)
            for mm_ in range(NCH):
                for j in range(DC):
                    np.fill_diagonal(cwd[mm_ * DC + j], cwc[mm_, :, j])
            m[f"cwdiag{sfx}"] = cwd.astype(bf)
            m[f"cb{sfx}"] = np.ascontiguousarray(
                cb[c0 : c0 + CH].reshape(NCH, 128).T
            )
            m[f"db{sfx}"] = np.ascontiguousarray(
                db[c0 : c0 + CH].reshape(NCH, 128).T
            )
            m[f"D{sfx}"] = np.ascontiguousarray(
                Dp[c0 : c0 + CH].reshape(NCH, 128).T
            )
        maps.append(m)
    return maps


def run(inputs, debug=False, trace=False):
    from concourse.bass_utils import run_bass_kernel_spmd

    if _COMPILED[0] is None or _COMPILED[0][1] != debug:
        _COMPILED[0] = (_build_program(debug=debug), debug)
    nc = _COMPILED[0][0]
    maps = _host_prep(inputs)
    res = run_bass_kernel_spmd(nc, maps, core_ids=list(range(8)), trace=trace)
    outs = [np.asarray(r["pout"], np.float32) for r in res.results]
    full = np.zeros((B, L, DM), np.float32)
    for core in range(8):
        b = core // 4
        full[b] += outs[core]
    return full, res


def kernel(**inputs):
    out, _ = run(inputs, debug=False, trace=False)
    return out


# revision 15
# speedup vs baseline: 1.0364x; 1.0364x over previous
"""BiMamba (bidirectional Mamba block) Trainium2 kernel.

Contract: kernel(**inputs) takes the full (unsharded) numpy inputs of the
reference and returns the full (2, 4096, 1024) float32 output.

Sharding: 8 cores = 2 batches x 4 channel-groups of 512 d_inner channels.
Each core runs both scan directions for its channel slice; the x_dbl
reduction over d_inner is an on-chip AllReduce within each batch's 4-core
group; the host sums the four partial out-projections per batch.

Key algebraic facts used:
  * xz for the reverse direction is the L-flip of the forward xz, so the
    input projection is computed once.
  * (y_f + flip(y_r)) @ W_out.T == out_f + flip(out_r), so one output
    projection suffices.

Performance structure (engine balance per scan chunk):
  * Pool (gpsimd) runs the 64 tensor_tensor_scan ops (the serial core).
  * DVE runs the bf16 TensorTensor mults (2x_1p packed mode).
  * Act runs the exp/softplus and most PSUM->SBUF broadcast copies.
  * PE accumulates y over the 16 states via identity matmuls into PSUM,
    plus the projections.
  * All DMA uses contiguous descriptors (reversals happen in SBUF reads).
"""

import os
import sys

import numpy as np

sys.path.insert(0, "/opt/trn_rl_repo")

B, L, DM, DI, DS, DR, DC = 2, 4096, 1024, 2048, 16, 64, 4
CH = 512          # d_inner channels per core
NCH = CH // 128   # channel tiles per core
T1 = 512          # pass-1 (projection/conv) token chunk
NC1 = L // T1
T2 = 512          # pass-2 (scan) token chunk
NC2 = L // T2

# engine assignment tuning: V=DVE, P=Pool(gpsimd), A=Act
# (scans must run on DVE: walrus cannot lower tensor_tensor_scan on Pool)
CFG = dict(
    bcopy=os.environ.get("CFG_BCOPY", "A" * 16),     # per s: B broadcast copy
    ccopy=os.environ.get("CFG_CCOPY", "A" * 16),     # per s: C broadcast copy
    carry=os.environ.get("CFG_CARRY", "V"),          # batched carry copies
    bt=os.environ.get("CFG_BT", ""),                 # per (s*NCH+m): bt engine
    cm=os.environ.get("CFG_CM", ""),                 # per (s*NCH+m): cmul engine
    hotbufs=int(os.environ.get("CFG_HOTBUFS", "2")),  # bufs for s-loop tags
)
if not CFG["bt"]:
    CFG["bt"] = "".join("V" if (s + m) % 2 else "P"
                        for s in range(DS) for m in range(NCH))
if not CFG["cm"]:
    CFG["cm"] = "".join("V" if (s + m) % 2 == 0 else "P"
                        for s in range(DS) for m in range(NCH))

_COMPILED = [None]


def _split_sync_waits(nc, mybir, max_waits=1):
    """walrus in this environment rejects >1 sync wait per instruction;
    hoist excess waits onto dedicated same-engine NOPs."""
    uid = [0]
    for f in nc.m.functions:
        for bb in f.blocks:
            new = []
            dirty = False
            for inst in bb.instructions:
                si = inst.sync_info
                if si is not None and len(si.on_wait) > max_waits:
                    waits = list(si.on_wait)
                    keep = waits[len(waits) - max_waits:]
                    hoist = waits[: len(waits) - max_waits]
                    for i in range(0, len(hoist), max_waits):
                        uid[0] += 1
                        nop = mybir.InstNoOp(
                            name=f"splitwait-{id(nc)}-{uid[0]}", engine=inst.engine
                        )
                        nop.sync_info = mybir.SyncInfo(
                            on_wait=hoist[i : i + max_waits], on_update=[]
                        )
                        nc.register_instruction(nop, overwrite=True)
                        new.append(nop)
                    inst.sync_info = mybir.SyncInfo(
                        on_wait=keep, on_update=list(si.on_update)
                    )
                    dirty = True
                new.append(inst)
            if dirty:
                bb.instructions = new


def _build_program(debug=False, collective=True):
    import concourse.bass as bass
    import concourse.tile as tile
    from concourse import mybir

    f32 = mybir.dt.float32
    f32r = mybir.dt.float32r
    bf16 = mybir.dt.bfloat16
    AF = mybir.ActivationFunctionType
    OP = mybir.AluOpType

    nc = bass.Bass("TRN2", target_bir_lowering=False, debug=False, num_devices=8)

    # ---- external inputs (per-core shards prepared on host) ----
    hT = nc.dram_tensor("hT", [DM, L], bf16, kind="ExternalInput")
    winxT = nc.dram_tensor("winxT", [DM, CH], bf16, kind="ExternalInput")
    winzT = nc.dram_tensor("winzT", [DM, CH], bf16, kind="ExternalInput")
    woutT_d = nc.dram_tensor("woutT", [CH, DM], bf16, kind="ExternalInput")
    sel_d = nc.dram_tensor("sel", [48, DS * 128], bf16, kind="ExternalInput")
    ident_d = nc.dram_tensor("ident", [128, 128], bf16, kind="ExternalInput")
    wx_d = {}
    wdt_d = {}
    A_d = {}
    cw_d = {}
    cb_d = {}
    db_d = {}
    D_d = {}
    for d in ("f", "r"):
        wx_d[d] = nc.dram_tensor(f"wx_{d}", [CH, 128], bf16, kind="ExternalInput")
        wdt_d[d] = nc.dram_tensor(f"wdt_{d}", [DR, CH], bf16, kind="ExternalInput")
        A_d[d] = nc.dram_tensor(f"A_{d}", [128, NCH * DS], f32, kind="ExternalInput")
        cw_d[d] = nc.dram_tensor(f"cw_{d}", [128, NCH * DC], f32,
                                 kind="ExternalInput")
        cb_d[d] = nc.dram_tensor(f"cb_{d}", [128, NCH], f32, kind="ExternalInput")
        db_d[d] = nc.dram_tensor(f"db_{d}", [128, NCH], f32, kind="ExternalInput")
        D_d[d] = nc.dram_tensor(f"D_{d}", [128, NCH], f32, kind="ExternalInput")

    pout = nc.dram_tensor("pout", [L, DM], bf16, kind="ExternalOutput")
    dbg = {}
    if debug:
        dbg["xc_f"] = nc.dram_tensor("dbg_xc_f", [NCH, 128, L], bf16, kind="ExternalOutput")
        dbg["xc_r"] = nc.dram_tensor("dbg_xc_r", [NCH, 128, L], bf16, kind="ExternalOutput")
        dbg["xdbl_f"] = nc.dram_tensor("dbg_xdbl_f", [96, L], f32, kind="ExternalOutput")
        dbg["xdbl_r"] = nc.dram_tensor("dbg_xdbl_r", [96, L], f32, kind="ExternalOutput")
        dbg["dt_f"] = nc.dram_tensor("dbg_dt_f", [NCH, 128, L], f32, kind="ExternalOutput")
        dbg["y_f"] = nc.dram_tensor("dbg_y_f", [NCH, 128, L], bf16, kind="ExternalOutput")
        dbg["siluz"] = nc.dram_tensor("dbg_siluz", [NCH, 128, L], bf16, kind="ExternalOutput")

    with tile.TileContext(nc, num_cores=8) as tc:
        _build_tile_program(
            nc, tc, tile, mybir, f32, f32r, bf16, AF, OP,
            hT, winxT, winzT, woutT_d, sel_d, ident_d, wx_d, wdt_d, A_d, cw_d,
            cb_d, db_d, D_d, pout, dbg, collective,
        )

    _split_sync_waits(nc, mybir)
    return nc


def _build_tile_program(
    nc, tc, tile, mybir, f32, f32r, bf16, AF, OP,
    hT, winxT, winzT, woutT_d, sel_d, ident_d, wx_d, wdt_d, A_d, cw_d, cb_d,
    db_d, D_d, pout, dbg, collective=True,
):
    from contextlib import ExitStack

    MM = nc.tensor.matmul
    ACT = nc.scalar.activation
    TT = nc.vector.tensor_tensor
    STT = nc.vector.scalar_tensor_tensor
    TSMUL = nc.vector.tensor_scalar_mul

    def veng(code):
        return nc.vector if code == "V" else nc.gpsimd

    def bcopy(code, out, in_):
        """PSUM f32 -> SBUF copy on the chosen engine."""
        if code == "A":
            ACT(out, in_, AF.Copy)
        else:
            veng(code).tensor_copy(out, in_)

    ctx = ExitStack()
    with ctx:
        # -------- persistent pools --------
        pers = ctx.enter_context(tc.tile_pool(name="pers", bufs=1))
        dram = ctx.enter_context(tc.tile_pool(name="dram", bufs=1, space="DRAM"))

        wout_sb = pers.tile([128, NCH, DM], bf16)
        nc.sync.dma_start(wout_sb[:], woutT_d.ap().rearrange("(k p) n -> p k n", p=128))
        sel_sb = pers.tile([48, DS * 128], bf16)
        nc.sync.dma_start(sel_sb[:], sel_d[:])
        ident_sb = pers.tile([128, 128], bf16)
        nc.sync.dma_start(ident_sb[:], ident_d[:])
        xdbl = {}      # bf16 [128, L]: rows [0:16]=B, [32:48]=C, [64:128]=dt-rank
        carry = {}
        wdt_sb = {}
        A_sb = {}
        db_sb = {}
        D_sb = {}
        for d in ("f", "r"):
            xdbl[d] = pers.tile([128, L], bf16, name=f"xdbl_{d}")
            carry[d] = pers.tile([128, NCH, DS], bf16, name=f"carry_{d}")
            nc.vector.memset(carry[d][:], 0.0)
            wdt_sb[d] = pers.tile([128, CH], bf16, name=f"wdt_sb_{d}")
            nc.sync.dma_start(wdt_sb[d][DR:128, :], wdt_d[d][:])
            A_sb[d] = pers.tile([128, NCH, DS], f32, name=f"A_sb_{d}")
            nc.sync.dma_start(A_sb[d][:], A_d[d].ap().rearrange("p (m s) -> p m s", m=NCH))
            db_sb[d] = pers.tile([128, NCH], f32, name=f"db_sb_{d}")
            nc.sync.dma_start(db_sb[d][:], db_d[d][:])
            D_sb[d] = pers.tile([128, NCH], f32, name=f"D_sb_{d}")
            nc.sync.dma_start(D_sb[d][:], D_d[d][:])
        ones = pers.tile([128, 1], f32)
        nc.vector.memset(ones[:], 1.0)

        # DRAM spill buffers (per-core local HBM); all in ORIGINAL time order
        # for the forward direction; xr/sz are original-time too (pass 2r
        # flips with reversed SBUF reads).  ygr is in flipped time.
        xf_dram = dram.tile([NCH, 128, L], bf16)
        xr_dram = dram.tile([NCH, 128, L], bf16)
        sz_dram = dram.tile([NCH, 128, L], bf16)
        ygr_dram = dram.tile([NCH, 128, L], bf16)
        # AllReduce staging: [dir, 128 rows, L] f32; rows as xdbl layout.
        # dir 0 = forward (original time), dir 1 = reverse (flipped time).
        ar_in = dram.tile([2, 128, L], f32)
        ar_out = dram.tile([2, 128, L], f32)

        # ================= PASS 1: in_proj + conv + silu + partial x_dbl ====
        with tc.tile_pool(name="p1", bufs=1) as p1, \
             tc.tile_pool(name="p1psum", bufs=1, space="PSUM") as p1psum:
            winx_sb = p1.tile([128, DM // 128, CH], bf16)
            nc.sync.dma_start(winx_sb[:], winxT.ap().rearrange("(k p) n -> p k n", p=128))
            winz_sb = p1.tile([128, DM // 128, CH], bf16)
            nc.sync.dma_start(winz_sb[:], winzT.ap().rearrange("(k p) n -> p k n", p=128))
            wx_sb = {}
            cw_sb = {}
            cb_sb = {}
            for d in ("f", "r"):
                wx_sb[d] = p1.tile([128, NCH, 128], bf16, name=f"wx_sb_{d}")
                nc.sync.dma_start(wx_sb[d][:], wx_d[d].ap().rearrange("(m p) n -> p m n", p=128))
                cw_sb[d] = p1.tile([128, NCH, DC], f32, name=f"cw_sb_{d}")
                nc.sync.dma_start(cw_sb[d][:], cw_d[d].ap().rearrange("p (m j) -> p m j", m=NCH))
                cb_sb[d] = p1.tile([128, NCH], f32, name=f"cb_sb_{d}")
                nc.sync.dma_start(cb_sb[d][:], cb_d[d][:])

            hT_r = hT.ap().rearrange("(k p) l -> p k l", p=128)
            prev_xe = [None] * NCH

            def conv_dir(cc, d, xe_list):
                """Causal (d=f) / anti-causal (d=r) depthwise conv + silu on
                original-time chunk cc, using extended tiles [3|T1|3].
                Conv runs on DVE (tap0 as 4x tensor_scalar, taps 1-3 as
                STT accumulate).  Returns bf16 silu'd tiles."""
                out = []
                for m in range(NCH):
                    acc = p1.tile([128, T1], f32, tag=f"cacc{m}", bufs=2,
                                  name=f"cacc{m}_{d}_{cc}")
                    xe = xe_list[m]
                    for j in range(DC):
                        off = j if d == "f" else (6 - j)
                        src = xe[:, off : off + T1]
                        wj = cw_sb[d][:, m, j : j + 1]
                        if j == 0:
                            TSMUL(acc[:], src, wj)
                        else:
                            STT(acc[:], src, wj, acc[:], OP.mult, OP.add)
                    xcb = p1.tile([128, T1], bf16, tag=f"xcb{m}_{d}", bufs=2,
                                  name=f"xcb{m}_{d}_{cc}")
                    ACT(xcb[:], acc[:], AF.Silu, bias=cb_sb[d][:, m : m + 1])
                    out.append(xcb)
                return out

            def xdbl_chunk(cc, d, xc_tiles):
                # psum rows laid out as [B 0:16 | C 32:48 | dt 64:128]
                # (W_x rows reordered+padded on host); full 128 rows go to AR.
                ps = p1psum.tile([128, T1], f32, tag="psx", bufs=2,
                                 name=f"psx_{d}_{cc}")
                for m in range(NCH):
                    MM(ps[:], wx_sb[d][:, m, :], xc_tiles[m][:],
                       start=(m == 0), stop=(m == NCH - 1))
                stage = p1.tile([128, T1], f32, tag="arstage", bufs=2,
                                name=f"arstage_{d}_{cc}")
                if d == "f":
                    ACT(stage[:], ps[:], AF.Copy)
                    nc.sync.dma_start(
                        ar_in[0, :, cc * T1 : (cc + 1) * T1], stage[:]
                    )
                else:
                    nc.vector.tensor_copy(stage[:], ps[:, ::-1])
                    nc.sync.dma_start(
                        ar_in[1, :, L - (cc + 1) * T1 : L - cc * T1], stage[:]
                    )

            def spill_chunk(cc, d, xc_tiles):
                x_dram = xf_dram if d == "f" else xr_dram
                for m in range(NCH):
                    nc.sync.dma_start(
                        x_dram[m, :, cc * T1 : (cc + 1) * T1], xc_tiles[m][:]
                    )
                    if dbg:
                        key = "xc_f" if d == "f" else "xc_r"
                        nc.sync.dma_start(
                            dbg[key][m, :, cc * T1 : (cc + 1) * T1], xc_tiles[m][:]
                        )

            def finish_reverse(cc, xe_list):
                xcr = conv_dir(cc, "r", xe_list)
                xdbl_chunk(cc, "r", xcr)
                spill_chunk(cc, "r", xcr)

            for c in range(NC1):
                hTt = p1.tile([128, DM // 128, T1], bf16, tag="hTt", bufs=2,
                              name=f"hTt_{c}")
                nc.sync.dma_start(hTt[:], hT_r[:, :, c * T1 : (c + 1) * T1])

                # x part (extended with halos) and z part (-> silu -> spill)
                cur_xe = []
                for m in range(NCH):
                    ps = p1psum.tile([128, T1], f32, tag="ps_ip", bufs=2,
                                     name=f"psx_{c}_{m}")
                    for ko in range(DM // 128):
                        MM(ps[:], winx_sb[:, ko, m * 128 : (m + 1) * 128],
                           hTt[:, ko, :], start=(ko == 0), stop=(ko == DM // 128 - 1))
                    xe = p1.tile([128, T1 + 6], bf16, tag=f"xe{m}", bufs=3,
                                 name=f"xe{m}_{c}")
                    ACT(xe[:, 3 : 3 + T1], ps[:], AF.Copy)
                    if c == 0:
                        nc.vector.memset(xe[:, 0:3], 0.0)
                    else:
                        nc.vector.tensor_copy(xe[:, 0:3], prev_xe[m][:, T1 : T1 + 3])
                    cur_xe.append(xe)
                for m in range(NCH):
                    ps = p1psum.tile([128, T1], f32, tag="ps_ip", bufs=2,
                                     name=f"psz_{c}_{m}")
                    for ko in range(DM // 128):
                        MM(ps[:], winz_sb[:, ko, m * 128 : (m + 1) * 128],
                           hTt[:, ko, :], start=(ko == 0), stop=(ko == DM // 128 - 1))
                    zs = p1.tile([128, T1], bf16, tag=f"zs{m}", bufs=2,
                                 name=f"zs{m}_{c}")
                    ACT(zs[:], ps[:], AF.Silu)
                    nc.sync.dma_start(sz_dram[m, :, c * T1 : (c + 1) * T1], zs[:])
                    if dbg:
                        nc.sync.dma_start(
                            dbg["siluz"][m, :, c * T1 : (c + 1) * T1], zs[:]
                        )

                if c > 0:
                    # fill previous chunk's right halo, then do its reverse conv
                    for m in range(NCH):
                        nc.vector.tensor_copy(
                            prev_xe[m][:, T1 + 3 : T1 + 6], cur_xe[m][:, 3:6]
                        )
                    finish_reverse(c - 1, prev_xe)

                # forward conv on current chunk
                xcf = conv_dir(c, "f", cur_xe)
                xdbl_chunk(c, "f", xcf)
                spill_chunk(c, "f", xcf)

                prev_xe = cur_xe

            for m in range(NCH):
                nc.vector.memset(prev_xe[m][:, T1 + 3 : T1 + 6], 0.0)
            finish_reverse(NC1 - 1, prev_xe)

            # -------- AllReduce of x_dbl over the 4 cores of this batch ----
            if collective:
                nc.gpsimd.collective_compute(
                    "AllReduce", OP.add,
                    replica_groups=[[0, 1, 2, 3], [4, 5, 6, 7]],
                    ins=[ar_in[:].opt()], outs=[ar_out[:].opt()],
                )
            else:
                nc.gpsimd.dma_start(ar_out[:], ar_in[:])
            # cast-readback f32 -> bf16 into SBUF (gpsimd DMAs may cast)
            for di, d in enumerate(("f", "r")):
                nc.gpsimd.dma_start(xdbl[d][:], ar_out[di, :, :])
            if dbg:
                for di, d in enumerate(("f", "r")):
                    nc.sync.dma_start(dbg[f"xdbl_{d}"][0:64, :], ar_out[di, 64:128, :])
                    nc.sync.dma_start(dbg[f"xdbl_{d}"][64:80, :], ar_out[di, 0:16, :])
                    nc.sync.dma_start(dbg[f"xdbl_{d}"][80:96, :], ar_out[di, 32:48, :])

        # ================= PASS 2: dt + selective scan (+gating, out_proj) ==
        def scan_pass(d, p2, p2psum, ytot_cb, mmt_bufs=3):
            """d: 'f' or 'r'.  'r' reads x/sz spills (original time) with
            reversed SBUF access; everything else runs in flipped time.
            ytot_cb(c2, yg_tiles): consumes gated y tiles for chunk c2."""
            x_dram = xf_dram if d == "f" else xr_dram
            rev = (lambda ap: ap) if d == "f" else (lambda ap: ap[:, ::-1])
            for c2 in range(NC2):
                sl = slice(c2 * T2, (c2 + 1) * T2)
                osl = sl if d == "f" else slice(L - (c2 + 1) * T2, L - c2 * T2)
                # ---- dt projection + softplus (f32 path) ----
                dt_sb = []
                for m in range(NCH):
                    psd = p2psum.tile([128, T2], f32, tag="mmt", bufs=mmt_bufs,
                                      name=f"psd_{d}_{c2}_{m}")
                    MM(psd[:], wdt_sb[d][DR:128, m * 128 : (m + 1) * 128],
                       xdbl[d][DR:128, sl], start=True, stop=True)
                    et = p2.tile([128, T2], f32, tag="et", bufs=2,
                                 name=f"et_{d}_{c2}_{m}")
                    ACT(et[:], psd[:], AF.Exp, bias=db_sb[d][:, m : m + 1])
                    dt = p2.tile([128, T2], f32, tag=f"dt{m}", bufs=2,
                                 name=f"dt{m}_{d}_{c2}")
                    ACT(dt[:], et[:], AF.Ln, bias=ones[:])
                    dt_sb.append(dt)
                    if dbg and d == "f":
                        nc.sync.dma_start(dbg["dt_f"][m, :, sl], dt[:])
                # ---- x load (bf16) + wd = dt*x + silu(z) load ----
                xd = []
                wd = []
                szt = []
                for m in range(NCH):
                    xt = p2.tile([128, T2], bf16, tag=f"xd{m}", bufs=2,
                                 name=f"xd{m}_{d}_{c2}")
                    nc.sync.dma_start(xt[:], x_dram[m, :, osl])
                    xd.append(xt)
                    wt = p2.tile([128, T2], bf16, tag=f"wd{m}", bufs=2,
                                 name=f"wd{m}_{d}_{c2}")
                    TT(wt[:], dt_sb[m][:], rev(xt[:]), OP.mult)
                    wd.append(wt)
                    sz = p2.tile([128, T2], bf16, tag=f"sz{m}", bufs=2,
                                 name=f"sz{m}_{d}_{c2}")
                    nc.sync.dma_start(sz[:], sz_dram[m, :, osl])
                    szt.append(sz)
                # ---- selective scan over 16 states ----
                yps = [p2psum.tile([128, T2], f32, tag=f"yp{m}", bufs=1,
                                   name=f"yp{m}_{d}_{c2}") for m in range(NCH)]
                for s in range(DS):
                    Bbp = p2psum.tile([128, T2], f32, tag="mmt", bufs=mmt_bufs,
                                      name=f"Bbp_{d}_{c2}_{s}")
                    MM(Bbp[:], sel_sb[0:DS, s * 128 : (s + 1) * 128],
                       xdbl[d][0:DS, sl], start=True, stop=True)
                    Bb = p2.tile([128, T2], bf16, tag="Bbs", bufs=CFG["hotbufs"],
                                 name=f"Bb_{d}_{c2}_{s}")
                    bcopy(CFG["bcopy"][s], Bb[:], Bbp[:])
                    Cbp = p2psum.tile([128, T2], f32, tag="mmt", bufs=mmt_bufs,
                                      name=f"Cbp_{d}_{c2}_{s}")
                    MM(Cbp[:], sel_sb[32 : 32 + DS, s * 128 : (s + 1) * 128],
                       xdbl[d][32 : 32 + DS, sl], start=True, stop=True)
                    Cb = p2.tile([128, T2], bf16, tag="Cbs", bufs=CFG["hotbufs"],
                                 name=f"Cb_{d}_{c2}_{s}")
                    bcopy(CFG["ccopy"][s], Cb[:], Cbp[:])
                    bt = []
                    for m in range(NCH):
                        b = p2.tile([128, T2], bf16, tag=f"bt{m}", bufs=CFG["hotbufs"],
                                    name=f"bt_{d}_{c2}_{s}_{m}")
                        veng(CFG["bt"][s * NCH + m]).tensor_tensor(
                            b[:], wd[m][:], Bb[:], OP.mult)
                        bt.append(b)
                    dAs = []
                    for m in range(NCH):
                        dA = p2.tile([128, T2], f32, tag=f"dA{m}", bufs=CFG["hotbufs"],
                                     name=f"dA_{d}_{c2}_{s}_{m}")
                        ACT(dA[:], dt_sb[m][:], AF.Exp,
                            scale=A_sb[d][:, m, s : s + 1])
                        dAs.append(dA)
                    # per-state hs tile holding all 4 channel groups, so the
                    # chunk-boundary carry is ONE strided copy per state
                    hs = p2.tile([128, NCH, T2], bf16, tag="hs", bufs=2,
                                 name=f"hs_{d}_{c2}_{s}")
                    for m in range(NCH):
                        nc.vector.tensor_tensor_scan(
                            hs[:, m, :], dAs[m][:], bt[m][:],
                            carry[d][:, m, s : s + 1], OP.mult, OP.add)
                    veng(CFG["carry"]).tensor_copy(
                        carry[d][:, :, s : s + 1], hs[:, :, T2 - 1 : T2])
                    for m in range(NCH):
                        cm = p2.tile([128, T2], bf16, tag=f"cm{m}", bufs=CFG["hotbufs"],
                                     name=f"cm_{d}_{c2}_{s}_{m}")
                        veng(CFG["cm"][s * NCH + m]).tensor_tensor(
                            cm[:], hs[:, m, :], Cb[:], OP.mult)
                        MM(yps[m][:], ident_sb[:], cm[:],
                           start=(s == 0), stop=(s == DS - 1))
                # ---- gating: y = (ypsum + x*D) * silu(z) ----
                yg = []
                for m in range(NCH):
                    y1 = p2.tile([128, T2], bf16, tag=f"y1{m}", bufs=2,
                                 name=f"y1_{d}_{c2}_{m}")
                    STT(y1[:], rev(xd[m][:]), D_sb[d][:, m : m + 1], yps[m][:],
                        OP.mult, OP.add)
                    yt = p2.tile([128, T2], bf16, tag=f"yg{m}", bufs=2,
                                 name=f"yg_{d}_{c2}_{m}")
                    TT(yt[:], y1[:], rev(szt[m][:]), OP.mult)
                    yg.append(yt)
                ytot_cb(c2, yg)

        # ---- pass 2r: reverse direction, spill gated y (flipped time) ----
        with tc.tile_pool(name="p2r", bufs=1) as p2r, \
             tc.tile_pool(name="p2rpsum", bufs=1, space="PSUM") as p2rpsum:

            def spill_ygr(c2, yg):
                for m in range(NCH):
                    nc.sync.dma_start(
                        ygr_dram[m, :, c2 * T2 : (c2 + 1) * T2], yg[m][:]
                    )

            scan_pass("r", p2r, p2rpsum, spill_ygr)

        # ---- pass 2f: forward + combine + out_proj ----
        with tc.tile_pool(name="p2f", bufs=1) as p2f, \
             tc.tile_pool(name="p2fpsum", bufs=1, space="PSUM") as p2fpsum:

            def combine_out(c2, yg):
                ytot = []
                for m in range(NCH):
                    ygr_t = p2f.tile([128, T2], bf16, tag=f"ygr{m}", bufs=2,
                                     name=f"ygr{m}_{c2}")
                    nc.sync.dma_start(
                        ygr_t[:], ygr_dram[m, :, L - (c2 + 1) * T2 : L - c2 * T2]
                    )
                    yt2 = p2f.tile([128, T2], bf16, tag=f"ytot{m}", bufs=2,
                                   name=f"ytot{m}_{c2}")
                    TT(yt2[:], yg[m][:], ygr_t[:, ::-1], OP.add)
                    ytot.append(yt2)
                    if dbg:
                        nc.sync.dma_start(
                            dbg["y_f"][m, :, c2 * T2 : (c2 + 1) * T2], yg[m][:]
                        )
                for mt in range(T2 // 128):
                    ob = p2f.tile([128, DM], bf16, tag="ob", bufs=2,
                                  name=f"ob_{c2}_{mt}")
                    for nh in range(DM // 512):
                        po = p2fpsum.tile([128, 512], f32, tag="po", bufs=2,
                                          name=f"po_{c2}_{mt}_{nh}")
                        for k in range(NCH):
                            MM(po[:], ytot[k][:, mt * 128 : (mt + 1) * 128],
                               wout_sb[:, k, nh * 512 : (nh + 1) * 512],
                               start=(k == 0), stop=(k == NCH - 1))
                        ACT(ob[:, nh * 512 : (nh + 1) * 512], po[:], AF.Copy)
                    nc.sync.dma_start(
                        pout[c2 * T2 + mt * 128 : c2 * T2 + (mt + 1) * 128, :],
                        ob[:],
                    )

            scan_pass("f", p2f, p2fpsum, combine_out, mmt_bufs=2)


def _host_prep(inputs):
    """Slice/transpose the full inputs into the 8 per-core input maps."""
    import ml_dtypes
    bf = ml_dtypes.bfloat16

    h = np.asarray(inputs["hidden_states"], np.float32)
    W_in = np.asarray(inputs["W_in"], np.float32)
    W_out = np.asarray(inputs["W_out"], np.float32)

    sel = np.zeros((48, DS * 128), np.float32)
    for s in range(DS):
        sel[s, s * 128 : (s + 1) * 128] = 1.0
        sel[32 + s, s * 128 : (s + 1) * 128] = 1.0

    maps = []
    for core in range(8):
        b, g = divmod(core, 4)
        c0 = g * CH
        m = {
            "hT": np.ascontiguousarray(h[b].T).astype(bf),
            "winxT": np.ascontiguousarray(W_in[c0 : c0 + CH, :].T).astype(bf),
            "winzT": np.ascontiguousarray(W_in[DI + c0 : DI + c0 + CH, :].T).astype(bf),
            "woutT": np.ascontiguousarray(W_out[:, c0 : c0 + CH].T).astype(bf),
            "sel": sel.astype(bf),
            "ident": np.eye(128, dtype=np.float32).astype(bf),
        }
        for d in ("f", "r"):
            sfx = f"_{d}"
            W_x = np.asarray(inputs[f"W_x{sfx}"], np.float32)
            W_dt = np.asarray(inputs[f"W_dt{sfx}"], np.float32)
            A = -np.exp(np.asarray(inputs[f"A_log{sfx}"], np.float64)).astype(np.float32)
            cw = np.asarray(inputs[f"conv_w{sfx}"], np.float32)
            cb = np.asarray(inputs[f"conv_b{sfx}"], np.float32)
            db = np.asarray(inputs[f"b_dt{sfx}"], np.float32)
            Dp = np.asarray(inputs[f"D{sfx}"], np.float32)
            wx_re = np.zeros((CH, 128), np.float32)
            wx_re[:, 0:DS] = W_x[DR : DR + DS, c0 : c0 + CH].T        # B rows
            wx_re[:, 32 : 32 + DS] = W_x[DR + DS : 96, c0 : c0 + CH].T  # C rows
            wx_re[:, DR:128] = W_x[0:DR, c0 : c0 + CH].T              # dt-rank rows
            m[f"wx{sfx}"] = wx_re.astype(bf)
            m[f"wdt{sfx}"] = np.ascontiguousarray(W_dt[c0 : c0 + CH, :].T).astype(bf)
            # (CH, DS) -> (128, NCH, DS) -> (128, NCH*DS)
            m[f"A{sfx}"] = np.ascontiguousarray(
                A[c0 : c0 + CH].reshape(NCH, 128, DS).transpose(1, 0, 2).reshape(128, NCH * DS)
            )
            m[f"cw{sfx}"] = np.ascontiguousarray(
                cw[c0 : c0 + CH].reshape(NCH, 128, DC).transpose(1, 0, 2).reshape(128, NCH * DC)
            )
            m[f"cb{sfx}"] = np.ascontiguousarray(
                cb[c0 : c0 + CH].reshape(NCH, 128).T
            )
            m[f"db{sfx}"] = np.ascontiguousarray(
                db[c0 : c0 + CH].reshape(NCH, 128).T
            )
            m[f"D{sfx}"] = np.ascontiguousarray(
                Dp[c0 : c0 + CH].reshape(NCH, 128).T
            )
        maps.append(m)
    return maps


def run(inputs, debug=False, trace=False):
    from concourse.bass_utils import run_bass_kernel_spmd

    if _COMPILED[0] is None or _COMPILED[0][1] != debug:
        _COMPILED[0] = (_build_program(debug=debug), debug)
    nc = _COMPILED[0][0]
    maps = _host_prep(inputs)
    res = run_bass_kernel_spmd(nc, maps, core_ids=list(range(8)), trace=trace)
    outs = [np.asarray(r["pout"], np.float32) for r in res.results]
    full = np.zeros((B, L, DM), np.float32)
    for core in range(8):
        b = core // 4
        full[b] += outs[core]
    return full, res


def kernel(**inputs):
    out, _ = run(inputs, debug=False, trace=False)
    return out


# revision 16
# speedup vs baseline: 1.1295x; 1.0899x over previous
"""BiMamba (bidirectional Mamba block) Trainium2 kernel.

Contract: kernel(**inputs) takes the full (unsharded) numpy inputs of the
reference and returns the full (2, 4096, 1024) float32 output.

Sharding: 8 cores = 2 batches x 4 channel-groups of 512 d_inner channels.
Each core runs both scan directions for its channel slice; the x_dbl
reduction over d_inner is an on-chip AllReduce within each batch's 4-core
group; the host sums the four partial out-projections per batch.

Key algebraic facts used:
  * xz for the reverse direction is the L-flip of the forward xz, so the
    input projection is computed once.
  * (y_f + flip(y_r)) @ W_out.T == out_f + flip(out_r), so one output
    projection suffices.

Performance structure (engine balance per scan chunk):
  * Pool (gpsimd) runs the 64 tensor_tensor_scan ops (the serial core).
  * DVE runs the bf16 TensorTensor mults (2x_1p packed mode).
  * Act runs the exp/softplus and most PSUM->SBUF broadcast copies.
  * PE accumulates y over the 16 states via identity matmuls into PSUM,
    plus the projections.
  * All DMA uses contiguous descriptors (reversals happen in SBUF reads).
"""

import os
import sys

import numpy as np

sys.path.insert(0, "/opt/trn_rl_repo")

B, L, DM, DI, DS, DR, DC = 2, 4096, 1024, 2048, 16, 64, 4
CH = 512          # d_inner channels per core
NCH = CH // 128   # channel tiles per core
T1 = 512          # pass-1 (projection/conv) token chunk
NC1 = L // T1
T2 = 512          # pass-2 (scan) token chunk
NC2 = L // T2

# engine assignment tuning: V=DVE, P=Pool(gpsimd), A=Act
# (scans must run on DVE: walrus cannot lower tensor_tensor_scan on Pool)
CFG = dict(
    bcopy=os.environ.get("CFG_BCOPY", "A" * 16),     # per s: B broadcast copy
    ccopy=os.environ.get("CFG_CCOPY", "A" * 16),     # per s: C broadcast copy
    carry=os.environ.get("CFG_CARRY", "P"),          # batched carry copies
    bt=os.environ.get("CFG_BT", ""),                 # per (s*NCH+m): bt engine
    cm=os.environ.get("CFG_CM", ""),                 # per (s*NCH+m): cmul engine
    hotbufs=int(os.environ.get("CFG_HOTBUFS", "2")),  # bufs for s-loop tags
)
def _bres(k, n=64):
    out = []
    acc = 0
    for _ in range(n):
        acc += k
        if acc >= n:
            acc -= n
            out.append("V")
        else:
            out.append("P")
    return "".join(out)


if not CFG["bt"]:
    CFG["bt"] = _bres(39)
if not CFG["cm"]:
    CFG["cm"] = _bres(39)

_COMPILED = [None]


def _split_sync_waits(nc, mybir, max_waits=1):
    """walrus in this environment rejects >1 sync wait per instruction;
    hoist excess waits onto dedicated same-engine NOPs."""
    uid = [0]
    for f in nc.m.functions:
        for bb in f.blocks:
            new = []
            dirty = False
            for inst in bb.instructions:
                si = inst.sync_info
                if si is not None and len(si.on_wait) > max_waits:
                    waits = list(si.on_wait)
                    keep = waits[len(waits) - max_waits:]
                    hoist = waits[: len(waits) - max_waits]
                    for i in range(0, len(hoist), max_waits):
                        uid[0] += 1
                        nop = mybir.InstNoOp(
                            name=f"splitwait-{id(nc)}-{uid[0]}", engine=inst.engine
                        )
                        nop.sync_info = mybir.SyncInfo(
                            on_wait=hoist[i : i + max_waits], on_update=[]
                        )
                        nc.register_instruction(nop, overwrite=True)
                        new.append(nop)
                    inst.sync_info = mybir.SyncInfo(
                        on_wait=keep, on_update=list(si.on_update)
                    )
                    dirty = True
                new.append(inst)
            if dirty:
                bb.instructions = new


def _build_program(debug=False, collective=True):
    import concourse.bass as bass
    import concourse.tile as tile
    from concourse import mybir

    f32 = mybir.dt.float32
    f32r = mybir.dt.float32r
    bf16 = mybir.dt.bfloat16
    AF = mybir.ActivationFunctionType
    OP = mybir.AluOpType

    nc = bass.Bass("TRN2", target_bir_lowering=False, debug=False, num_devices=8)

    # ---- external inputs (per-core shards prepared on host) ----
    hT = nc.dram_tensor("hT", [DM, L], bf16, kind="ExternalInput")
    winxT = nc.dram_tensor("winxT", [DM, CH], bf16, kind="ExternalInput")
    winzT = nc.dram_tensor("winzT", [DM, CH], bf16, kind="ExternalInput")
    woutT_d = nc.dram_tensor("woutT", [CH, DM], bf16, kind="ExternalInput")
    sel_d = nc.dram_tensor("sel", [48, DS * 128], bf16, kind="ExternalInput")
    ident_d = nc.dram_tensor("ident", [128, 128], bf16, kind="ExternalInput")
    wx_d = {}
    wdt_d = {}
    A_d = {}
    cw_d = {}
    cb_d = {}
    db_d = {}
    D_d = {}
    for d in ("f", "r"):
        wx_d[d] = nc.dram_tensor(f"wx_{d}", [CH, 128], bf16, kind="ExternalInput")
        wdt_d[d] = nc.dram_tensor(f"wdt_{d}", [DR, CH], bf16, kind="ExternalInput")
        A_d[d] = nc.dram_tensor(f"A_{d}", [128, NCH * DS], f32, kind="ExternalInput")
        cw_d[d] = nc.dram_tensor(f"cw_{d}", [128, NCH * DC], f32,
                                 kind="ExternalInput")
        cb_d[d] = nc.dram_tensor(f"cb_{d}", [128, NCH], f32, kind="ExternalInput")
        db_d[d] = nc.dram_tensor(f"db_{d}", [128, NCH], f32, kind="ExternalInput")
        D_d[d] = nc.dram_tensor(f"D_{d}", [128, NCH], f32, kind="ExternalInput")

    pout = nc.dram_tensor("pout", [L, DM], bf16, kind="ExternalOutput")
    dbg = {}
    if debug:
        dbg["xc_f"] = nc.dram_tensor("dbg_xc_f", [NCH, 128, L], bf16, kind="ExternalOutput")
        dbg["xc_r"] = nc.dram_tensor("dbg_xc_r", [NCH, 128, L], bf16, kind="ExternalOutput")
        dbg["xdbl_f"] = nc.dram_tensor("dbg_xdbl_f", [96, L], f32, kind="ExternalOutput")
        dbg["xdbl_r"] = nc.dram_tensor("dbg_xdbl_r", [96, L], f32, kind="ExternalOutput")
        dbg["dt_f"] = nc.dram_tensor("dbg_dt_f", [NCH, 128, L], f32, kind="ExternalOutput")
        dbg["y_f"] = nc.dram_tensor("dbg_y_f", [NCH, 128, L], bf16, kind="ExternalOutput")
        dbg["siluz"] = nc.dram_tensor("dbg_siluz", [NCH, 128, L], bf16, kind="ExternalOutput")

    with tile.TileContext(nc, num_cores=8) as tc:
        _build_tile_program(
            nc, tc, tile, mybir, f32, f32r, bf16, AF, OP,
            hT, winxT, winzT, woutT_d, sel_d, ident_d, wx_d, wdt_d, A_d, cw_d,
            cb_d, db_d, D_d, pout, dbg, collective,
        )

    _split_sync_waits(nc, mybir)
    return nc


def _build_tile_program(
    nc, tc, tile, mybir, f32, f32r, bf16, AF, OP,
    hT, winxT, winzT, woutT_d, sel_d, ident_d, wx_d, wdt_d, A_d, cw_d, cb_d,
    db_d, D_d, pout, dbg, collective=True,
):
    from contextlib import ExitStack

    MM = nc.tensor.matmul
    ACT = nc.scalar.activation
    TT = nc.vector.tensor_tensor
    STT = nc.vector.scalar_tensor_tensor
    TSMUL = nc.vector.tensor_scalar_mul

    def veng(code):
        return nc.vector if code == "V" else nc.gpsimd

    def bcopy(code, out, in_):
        """PSUM f32 -> SBUF copy on the chosen engine."""
        if code == "A":
            ACT(out, in_, AF.Copy)
        else:
            veng(code).tensor_copy(out, in_)

    ctx = ExitStack()
    with ctx:
        # -------- persistent pools --------
        pers = ctx.enter_context(tc.tile_pool(name="pers", bufs=1))
        dram = ctx.enter_context(tc.tile_pool(name="dram", bufs=1, space="DRAM"))

        wout_sb = pers.tile([128, NCH, DM], bf16)
        nc.sync.dma_start(wout_sb[:], woutT_d.ap().rearrange("(k p) n -> p k n", p=128))
        sel_sb = pers.tile([48, DS * 128], bf16)
        nc.sync.dma_start(sel_sb[:], sel_d[:])
        ident_sb = pers.tile([128, 128], bf16)
        nc.sync.dma_start(ident_sb[:], ident_d[:])
        xdbl = {}      # bf16 [128, L]: rows [0:16]=B, [32:48]=C, [64:128]=dt-rank
        carry = {}
        wdt_sb = {}
        A_sb = {}
        db_sb = {}
        D_sb = {}
        for d in ("f", "r"):
            xdbl[d] = pers.tile([128, L], bf16, name=f"xdbl_{d}")
            carry[d] = pers.tile([128, NCH, DS], bf16, name=f"carry_{d}")
            nc.vector.memset(carry[d][:], 0.0)
            wdt_sb[d] = pers.tile([128, CH], bf16, name=f"wdt_sb_{d}")
            nc.sync.dma_start(wdt_sb[d][DR:128, :], wdt_d[d][:])
            A_sb[d] = pers.tile([128, NCH, DS], f32, name=f"A_sb_{d}")
            nc.sync.dma_start(A_sb[d][:], A_d[d].ap().rearrange("p (m s) -> p m s", m=NCH))
            db_sb[d] = pers.tile([128, NCH], f32, name=f"db_sb_{d}")
            nc.sync.dma_start(db_sb[d][:], db_d[d][:])
            D_sb[d] = pers.tile([128, NCH], f32, name=f"D_sb_{d}")
            nc.sync.dma_start(D_sb[d][:], D_d[d][:])
        ones = pers.tile([128, 1], f32)
        nc.vector.memset(ones[:], 1.0)

        # DRAM spill buffers (per-core local HBM); all in ORIGINAL time order
        # for the forward direction; xr/sz are original-time too (pass 2r
        # flips with reversed SBUF reads).  ygr is in flipped time.
        xf_dram = dram.tile([NCH, 128, L], bf16)
        xr_dram = dram.tile([NCH, 128, L], bf16)
        sz_dram = dram.tile([NCH, 128, L], bf16)
        ygr_dram = dram.tile([NCH, 128, L], bf16)
        # AllReduce staging: [dir, 128 rows, L] f32; rows as xdbl layout.
        # dir 0 = forward (original time), dir 1 = reverse (flipped time).
        ar_in = dram.tile([2, 128, L], f32)
        ar_out = dram.tile([2, 128, L], f32)

        # ================= PASS 1: in_proj + conv + silu + partial x_dbl ====
        with tc.tile_pool(name="p1", bufs=1) as p1, \
             tc.tile_pool(name="p1psum", bufs=1, space="PSUM") as p1psum:
            winx_sb = p1.tile([128, DM // 128, CH], bf16)
            nc.sync.dma_start(winx_sb[:], winxT.ap().rearrange("(k p) n -> p k n", p=128))
            winz_sb = p1.tile([128, DM // 128, CH], bf16)
            nc.sync.dma_start(winz_sb[:], winzT.ap().rearrange("(k p) n -> p k n", p=128))
            wx_sb = {}
            cw_sb = {}
            cb_sb = {}
            for d in ("f", "r"):
                wx_sb[d] = p1.tile([128, NCH, 128], bf16, name=f"wx_sb_{d}")
                nc.sync.dma_start(wx_sb[d][:], wx_d[d].ap().rearrange("(m p) n -> p m n", p=128))
                cw_sb[d] = p1.tile([128, NCH, DC], f32, name=f"cw_sb_{d}")
                nc.sync.dma_start(cw_sb[d][:], cw_d[d].ap().rearrange("p (m j) -> p m j", m=NCH))
                cb_sb[d] = p1.tile([128, NCH], f32, name=f"cb_sb_{d}")
                nc.sync.dma_start(cb_sb[d][:], cb_d[d][:])

            hT_r = hT.ap().rearrange("(k p) l -> p k l", p=128)
            prev_xe = [None] * NCH

            def conv_dir(cc, d, xe_list):
                """Causal (d=f) / anti-causal (d=r) depthwise conv + silu on
                original-time chunk cc, using extended tiles [3|T1|3].
                Conv runs on DVE (tap0 as 4x tensor_scalar, taps 1-3 as
                STT accumulate).  Returns bf16 silu'd tiles."""
                out = []
                for m in range(NCH):
                    acc = p1.tile([128, T1], f32, tag=f"cacc{m}", bufs=2,
                                  name=f"cacc{m}_{d}_{cc}")
                    xe = xe_list[m]
                    for j in range(DC):
                        off = j if d == "f" else (6 - j)
                        src = xe[:, off : off + T1]
                        wj = cw_sb[d][:, m, j : j + 1]
                        if j == 0:
                            TSMUL(acc[:], src, wj)
                        else:
                            STT(acc[:], src, wj, acc[:], OP.mult, OP.add)
                    xcb = p1.tile([128, T1], bf16, tag=f"xcb{m}_{d}", bufs=2,
                                  name=f"xcb{m}_{d}_{cc}")
                    ACT(xcb[:], acc[:], AF.Silu, bias=cb_sb[d][:, m : m + 1])
                    out.append(xcb)
                return out

            def xdbl_chunk(cc, d, xc_tiles):
                # psum rows laid out as [B 0:16 | C 32:48 | dt 64:128]
                # (W_x rows reordered+padded on host); full 128 rows go to AR.
                ps = p1psum.tile([128, T1], f32, tag="psx", bufs=2,
                                 name=f"psx_{d}_{cc}")
                for m in range(NCH):
                    MM(ps[:], wx_sb[d][:, m, :], xc_tiles[m][:],
                       start=(m == 0), stop=(m == NCH - 1))
                stage = p1.tile([128, T1], f32, tag="arstage", bufs=2,
                                name=f"arstage_{d}_{cc}")
                if d == "f":
                    ACT(stage[:], ps[:], AF.Copy)
                    nc.sync.dma_start(
                        ar_in[0, :, cc * T1 : (cc + 1) * T1], stage[:]
                    )
                else:
                    nc.vector.tensor_copy(stage[:], ps[:, ::-1])
                    nc.sync.dma_start(
                        ar_in[1, :, L - (cc + 1) * T1 : L - cc * T1], stage[:]
                    )

            def spill_chunk(cc, d, xc_tiles):
                x_dram = xf_dram if d == "f" else xr_dram
                for m in range(NCH):
                    nc.sync.dma_start(
                        x_dram[m, :, cc * T1 : (cc + 1) * T1], xc_tiles[m][:]
                    )
                    if dbg:
                        key = "xc_f" if d == "f" else "xc_r"
                        nc.sync.dma_start(
                            dbg[key][m, :, cc * T1 : (cc + 1) * T1], xc_tiles[m][:]
                        )

            def finish_reverse(cc, xe_list):
                xcr = conv_dir(cc, "r", xe_list)
                xdbl_chunk(cc, "r", xcr)
                spill_chunk(cc, "r", xcr)

            for c in range(NC1):
                hTt = p1.tile([128, DM // 128, T1], bf16, tag="hTt", bufs=2,
                              name=f"hTt_{c}")
                nc.sync.dma_start(hTt[:], hT_r[:, :, c * T1 : (c + 1) * T1])

                # x part (extended with halos) and z part (-> silu -> spill)
                cur_xe = []
                for m in range(NCH):
                    ps = p1psum.tile([128, T1], f32, tag="ps_ip", bufs=2,
                                     name=f"psx_{c}_{m}")
                    for ko in range(DM // 128):
                        MM(ps[:], winx_sb[:, ko, m * 128 : (m + 1) * 128],
                           hTt[:, ko, :], start=(ko == 0), stop=(ko == DM // 128 - 1))
                    xe = p1.tile([128, T1 + 6], bf16, tag=f"xe{m}", bufs=3,
                                 name=f"xe{m}_{c}")
                    ACT(xe[:, 3 : 3 + T1], ps[:], AF.Copy)
                    if c == 0:
                        nc.vector.memset(xe[:, 0:3], 0.0)
                    else:
                        nc.vector.tensor_copy(xe[:, 0:3], prev_xe[m][:, T1 : T1 + 3])
                    cur_xe.append(xe)
                for m in range(NCH):
                    ps = p1psum.tile([128, T1], f32, tag="ps_ip", bufs=2,
                                     name=f"psz_{c}_{m}")
                    for ko in range(DM // 128):
                        MM(ps[:], winz_sb[:, ko, m * 128 : (m + 1) * 128],
                           hTt[:, ko, :], start=(ko == 0), stop=(ko == DM // 128 - 1))
                    zs = p1.tile([128, T1], bf16, tag=f"zs{m}", bufs=2,
                                 name=f"zs{m}_{c}")
                    ACT(zs[:], ps[:], AF.Silu)
                    nc.sync.dma_start(sz_dram[m, :, c * T1 : (c + 1) * T1], zs[:])
                    if dbg:
                        nc.sync.dma_start(
                            dbg["siluz"][m, :, c * T1 : (c + 1) * T1], zs[:]
                        )

                if c > 0:
                    # fill previous chunk's right halo, then do its reverse conv
                    for m in range(NCH):
                        nc.vector.tensor_copy(
                            prev_xe[m][:, T1 + 3 : T1 + 6], cur_xe[m][:, 3:6]
                        )
                    finish_reverse(c - 1, prev_xe)

                # forward conv on current chunk
                xcf = conv_dir(c, "f", cur_xe)
                xdbl_chunk(c, "f", xcf)
                spill_chunk(c, "f", xcf)

                prev_xe = cur_xe

            for m in range(NCH):
                nc.vector.memset(prev_xe[m][:, T1 + 3 : T1 + 6], 0.0)
            finish_reverse(NC1 - 1, prev_xe)

            # -------- AllReduce of x_dbl over the 4 cores of this batch ----
            if collective:
                nc.gpsimd.collective_compute(
                    "AllReduce", OP.add,
                    replica_groups=[[0, 1, 2, 3], [4, 5, 6, 7]],
                    ins=[ar_in[:].opt()], outs=[ar_out[:].opt()],
                )
            else:
                nc.gpsimd.dma_start(ar_out[:], ar_in[:])
            # cast-readback f32 -> bf16 into SBUF (gpsimd DMAs may cast)
            for di, d in enumerate(("f", "r")):
                nc.gpsimd.dma_start(xdbl[d][:], ar_out[di, :, :])
            if dbg:
                for di, d in enumerate(("f", "r")):
                    nc.sync.dma_start(dbg[f"xdbl_{d}"][0:64, :], ar_out[di, 64:128, :])
                    nc.sync.dma_start(dbg[f"xdbl_{d}"][64:80, :], ar_out[di, 0:16, :])
                    nc.sync.dma_start(dbg[f"xdbl_{d}"][80:96, :], ar_out[di, 32:48, :])

        # ================= PASS 2: dt + selective scan (+gating, out_proj) ==
        def scan_pass(d, p2, p2psum, ytot_cb, mmt_bufs=3):
            """d: 'f' or 'r'.  'r' reads x/sz spills (original time) with
            reversed SBUF access; everything else runs in flipped time.
            ytot_cb(c2, yg_tiles): consumes gated y tiles for chunk c2."""
            x_dram = xf_dram if d == "f" else xr_dram
            rev = (lambda ap: ap) if d == "f" else (lambda ap: ap[:, ::-1])
            for c2 in range(NC2):
                sl = slice(c2 * T2, (c2 + 1) * T2)
                osl = sl if d == "f" else slice(L - (c2 + 1) * T2, L - c2 * T2)
                # ---- dt projection + softplus (f32 path) ----
                dt_sb = []
                for m in range(NCH):
                    psd = p2psum.tile([128, T2], f32, tag="mmt", bufs=mmt_bufs,
                                      name=f"psd_{d}_{c2}_{m}")
                    MM(psd[:], wdt_sb[d][DR:128, m * 128 : (m + 1) * 128],
                       xdbl[d][DR:128, sl], start=True, stop=True)
                    et = p2.tile([128, T2], f32, tag="et", bufs=2,
                                 name=f"et_{d}_{c2}_{m}")
                    ACT(et[:], psd[:], AF.Exp, bias=db_sb[d][:, m : m + 1])
                    dt = p2.tile([128, T2], f32, tag=f"dt{m}", bufs=2,
                                 name=f"dt{m}_{d}_{c2}")
                    ACT(dt[:], et[:], AF.Ln, bias=ones[:])
                    dt_sb.append(dt)
                    if dbg and d == "f":
                        nc.sync.dma_start(dbg["dt_f"][m, :, sl], dt[:])
                # ---- x load (bf16) + wd = dt*x + silu(z) load ----
                xd = []
                wd = []
                szt = []
                for m in range(NCH):
                    xt = p2.tile([128, T2], bf16, tag=f"xd{m}", bufs=2,
                                 name=f"xd{m}_{d}_{c2}")
                    nc.sync.dma_start(xt[:], x_dram[m, :, osl])
                    xd.append(xt)
                    wt = p2.tile([128, T2], bf16, tag=f"wd{m}", bufs=2,
                                 name=f"wd{m}_{d}_{c2}")
                    TT(wt[:], dt_sb[m][:], rev(xt[:]), OP.mult)
                    wd.append(wt)
                    sz = p2.tile([128, T2], bf16, tag=f"sz{m}", bufs=2,
                                 name=f"sz{m}_{d}_{c2}")
                    nc.sync.dma_start(sz[:], sz_dram[m, :, osl])
                    szt.append(sz)
                # ---- selective scan over 16 states ----
                yps = [p2psum.tile([128, T2], f32, tag=f"yp{m}", bufs=1,
                                   name=f"yp{m}_{d}_{c2}") for m in range(NCH)]
                for s in range(DS):
                    Bbp = p2psum.tile([128, T2], f32, tag="mmt", bufs=mmt_bufs,
                                      name=f"Bbp_{d}_{c2}_{s}")
                    MM(Bbp[:], sel_sb[0:DS, s * 128 : (s + 1) * 128],
                       xdbl[d][0:DS, sl], start=True, stop=True)
                    Bb = p2.tile([128, T2], bf16, tag="Bbs", bufs=CFG["hotbufs"],
                                 name=f"Bb_{d}_{c2}_{s}")
                    bcopy(CFG["bcopy"][s], Bb[:], Bbp[:])
                    Cbp = p2psum.tile([128, T2], f32, tag="mmt", bufs=mmt_bufs,
                                      name=f"Cbp_{d}_{c2}_{s}")
                    MM(Cbp[:], sel_sb[32 : 32 + DS, s * 128 : (s + 1) * 128],
                       xdbl[d][32 : 32 + DS, sl], start=True, stop=True)
                    Cb = p2.tile([128, T2], bf16, tag="Cbs", bufs=CFG["hotbufs"],
                                 name=f"Cb_{d}_{c2}_{s}")
                    bcopy(CFG["ccopy"][s], Cb[:], Cbp[:])
                    bt = []
                    for m in range(NCH):
                        b = p2.tile([128, T2], bf16, tag=f"bt{m}", bufs=CFG["hotbufs"],
                                    name=f"bt_{d}_{c2}_{s}_{m}")
                        veng(CFG["bt"][s * NCH + m]).tensor_tensor(
                            b[:], wd[m][:], Bb[:], OP.mult)
                        bt.append(b)
                    dAs = []
                    for m in range(NCH):
                        dA = p2.tile([128, T2], f32, tag=f"dA{m}", bufs=CFG["hotbufs"],
                                     name=f"dA_{d}_{c2}_{s}_{m}")
                        ACT(dA[:], dt_sb[m][:], AF.Exp,
                            scale=A_sb[d][:, m, s : s + 1])
                        dAs.append(dA)
                    # per-state hs tile holding all 4 channel groups, so the
                    # chunk-boundary carry is ONE strided copy per state
                    hs = p2.tile([128, NCH, T2], bf16, tag="hs", bufs=2,
                                 name=f"hs_{d}_{c2}_{s}")
                    for m in range(NCH):
                        nc.vector.tensor_tensor_scan(
                            hs[:, m, :], dAs[m][:], bt[m][:],
                            carry[d][:, m, s : s + 1], OP.mult, OP.add)
                    veng(CFG["carry"]).tensor_copy(
                        carry[d][:, :, s : s + 1], hs[:, :, T2 - 1 : T2])
                    for m in range(NCH):
                        cm = p2.tile([128, T2], bf16, tag=f"cm{m}", bufs=CFG["hotbufs"],
                                     name=f"cm_{d}_{c2}_{s}_{m}")
                        veng(CFG["cm"][s * NCH + m]).tensor_tensor(
                            cm[:], hs[:, m, :], Cb[:], OP.mult)
                        MM(yps[m][:], ident_sb[:], cm[:],
                           start=(s == 0), stop=(s == DS - 1))
                # ---- gating: y = (ypsum + x*D) * silu(z) ----
                yg = []
                for m in range(NCH):
                    y1 = p2.tile([128, T2], bf16, tag=f"y1{m}", bufs=2,
                                 name=f"y1_{d}_{c2}_{m}")
                    STT(y1[:], rev(xd[m][:]), D_sb[d][:, m : m + 1], yps[m][:],
                        OP.mult, OP.add)
                    yt = p2.tile([128, T2], bf16, tag=f"yg{m}", bufs=2,
                                 name=f"yg_{d}_{c2}_{m}")
                    TT(yt[:], y1[:], rev(szt[m][:]), OP.mult)
                    yg.append(yt)
                ytot_cb(c2, yg)

        # ---- pass 2r: reverse direction, spill gated y (flipped time) ----
        with tc.tile_pool(name="p2r", bufs=1) as p2r, \
             tc.tile_pool(name="p2rpsum", bufs=1, space="PSUM") as p2rpsum:

            def spill_ygr(c2, yg):
                for m in range(NCH):
                    nc.sync.dma_start(
                        ygr_dram[m, :, c2 * T2 : (c2 + 1) * T2], yg[m][:]
                    )

            scan_pass("r", p2r, p2rpsum, spill_ygr)

        # ---- pass 2f: forward + combine + out_proj ----
        with tc.tile_pool(name="p2f", bufs=1) as p2f, \
             tc.tile_pool(name="p2fpsum", bufs=1, space="PSUM") as p2fpsum:

            def combine_out(c2, yg):
                ytot = []
                for m in range(NCH):
                    ygr_t = p2f.tile([128, T2], bf16, tag=f"ygr{m}", bufs=2,
                                     name=f"ygr{m}_{c2}")
                    nc.sync.dma_start(
                        ygr_t[:], ygr_dram[m, :, L - (c2 + 1) * T2 : L - c2 * T2]
                    )
                    yt2 = p2f.tile([128, T2], bf16, tag=f"ytot{m}", bufs=2,
                                   name=f"ytot{m}_{c2}")
                    TT(yt2[:], yg[m][:], ygr_t[:, ::-1], OP.add)
                    ytot.append(yt2)
                    if dbg:
                        nc.sync.dma_start(
                            dbg["y_f"][m, :, c2 * T2 : (c2 + 1) * T2], yg[m][:]
                        )
                for mt in range(T2 // 128):
                    ob = p2f.tile([128, DM], bf16, tag="ob", bufs=2,
                                  name=f"ob_{c2}_{mt}")
                    for nh in range(DM // 512):
                        po = p2fpsum.tile([128, 512], f32, tag="po", bufs=2,
                                          name=f"po_{c2}_{mt}_{nh}")
                        for k in range(NCH):
                            MM(po[:], ytot[k][:, mt * 128 : (mt + 1) * 128],
                               wout_sb[:, k, nh * 512 : (nh + 1) * 512],
                               start=(k == 0), stop=(k == NCH - 1))
                        ACT(ob[:, nh * 512 : (nh + 1) * 512], po[:], AF.Copy)
                    nc.sync.dma_start(
                        pout[c2 * T2 + mt * 128 : c2 * T2 + (mt + 1) * 128, :],
                        ob[:],
                    )

            scan_pass("f", p2f, p2fpsum, combine_out, mmt_bufs=2)


def _host_prep(inputs):
    """Slice/transpose the full inputs into the 8 per-core input maps."""
    import ml_dtypes
    bf = ml_dtypes.bfloat16

    h = np.asarray(inputs["hidden_states"], np.float32)
    W_in = np.asarray(inputs["W_in"], np.float32)
    W_out = np.asarray(inputs["W_out"], np.float32)

    sel = np.zeros((48, DS * 128), np.float32)
    for s in range(DS):
        sel[s, s * 128 : (s + 1) * 128] = 1.0
        sel[32 + s, s * 128 : (s + 1) * 128] = 1.0

    maps = []
    for core in range(8):
        b, g = divmod(core, 4)
        c0 = g * CH
        m = {
            "hT": np.ascontiguousarray(h[b].T).astype(bf),
            "winxT": np.ascontiguousarray(W_in[c0 : c0 + CH, :].T).astype(bf),
            "winzT": np.ascontiguousarray(W_in[DI + c0 : DI + c0 + CH, :].T).astype(bf),
            "woutT": np.ascontiguousarray(W_out[:, c0 : c0 + CH].T).astype(bf),
            "sel": sel.astype(bf),
            "ident": np.eye(128, dtype=np.float32).astype(bf),
        }
        for d in ("f", "r"):
            sfx = f"_{d}"
            W_x = np.asarray(inputs[f"W_x{sfx}"], np.float32)
            W_dt = np.asarray(inputs[f"W_dt{sfx}"], np.float32)
            A = -np.exp(np.asarray(inputs[f"A_log{sfx}"], np.float64)).astype(np.float32)
            cw = np.asarray(inputs[f"conv_w{sfx}"], np.float32)
            cb = np.asarray(inputs[f"conv_b{sfx}"], np.float32)
            db = np.asarray(inputs[f"b_dt{sfx}"], np.float32)
            Dp = np.asarray(inputs[f"D{sfx}"], np.float32)
            wx_re = np.zeros((CH, 128), np.float32)
            wx_re[:, 0:DS] = W_x[DR : DR + DS, c0 : c0 + CH].T        # B rows
            wx_re[:, 32 : 32 + DS] = W_x[DR + DS : 96, c0 : c0 + CH].T  # C rows
            wx_re[:, DR:128] = W_x[0:DR, c0 : c0 + CH].T              # dt-rank rows
            m[f"wx{sfx}"] = wx_re.astype(bf)
            m[f"wdt{sfx}"] = np.ascontiguousarray(W_dt[c0 : c0 + CH, :].T).astype(bf)
            # (CH, DS) -> (128, NCH, DS) -> (128, NCH*DS)
            m[f"A{sfx}"] = np.ascontiguousarray(
                A[c0 : c0 + CH].reshape(NCH, 128, DS).transpose(1, 0, 2).reshape(128, NCH * DS)
            )
            m[f"cw{sfx}"] = np.ascontiguousarray(
                cw[c0 : c0 + CH].reshape(NCH, 128, DC).transpose(1, 0, 2).reshape(128, NCH * DC)
            )
            m[f"cb{sfx}"] = np.ascontiguousarray(
                cb[c0 : c0 + CH].reshape(NCH, 128).T
            )
            m[f"db{sfx}"] = np.ascontiguousarray(
                db[c0 : c0 + CH].reshape(NCH, 128).T
            )
            m[f"D{sfx}"] = np.ascontiguousarray(
                Dp[c0 : c0 + CH].reshape(NCH, 128).T
            )
        maps.append(m)
    return maps


def run(inputs, debug=False, trace=False):
    from concourse.bass_utils import run_bass_kernel_spmd

    if _COMPILED[0] is None or _COMPILED[0][1] != debug:
        _COMPILED[0] = (_build_program(debug=debug), debug)
    nc = _COMPILED[0][0]
    maps = _host_prep(inputs)
    res = run_bass_kernel_spmd(nc, maps, core_ids=list(range(8)), trace=trace)
    outs = [np.asarray(r["pout"], np.float32) for r in res.results]
    full = np.zeros((B, L, DM), np.float32)
    for core in range(8):
        b = core // 4
        full[b] += outs[core]
    return full, res


def kernel(**inputs):
    out, _ = run(inputs, debug=False, trace=False)
    return out


# revision 19
# speedup vs baseline: 1.1507x; 1.0187x over previous
"""BiMamba (bidirectional Mamba block) Trainium2 kernel.

Contract: kernel(**inputs) takes the full (unsharded) numpy inputs of the
reference and returns the full (2, 4096, 1024) float32 output.

Sharding: 8 cores = 2 batches x 4 channel-groups of 512 d_inner channels.
Each core runs both scan directions for its channel slice; the x_dbl
reduction over d_inner is an on-chip AllReduce within each batch's 4-core
group; the host sums the four partial out-projections per batch.

Key algebraic facts used:
  * xz for the reverse direction is the L-flip of the forward xz, so the
    input projection is computed once.
  * (y_f + flip(y_r)) @ W_out.T == out_f + flip(out_r), so one output
    projection suffices.

Performance structure (engine balance per scan chunk):
  * Pool (gpsimd) runs the 64 tensor_tensor_scan ops (the serial core).
  * DVE runs the bf16 TensorTensor mults (2x_1p packed mode).
  * Act runs the exp/softplus and most PSUM->SBUF broadcast copies.
  * PE accumulates y over the 16 states via identity matmuls into PSUM,
    plus the projections.
  * All DMA uses contiguous descriptors (reversals happen in SBUF reads).
"""

import os
import sys

import numpy as np

sys.path.insert(0, "/opt/trn_rl_repo")

B, L, DM, DI, DS, DR, DC = 2, 4096, 1024, 2048, 16, 64, 4
CH = 512          # d_inner channels per core
NCH = CH // 128   # channel tiles per core
T1 = 512          # pass-1 (projection/conv) token chunk
NC1 = L // T1
T2 = 512          # pass-2 (scan) token chunk
NC2 = L // T2

# engine assignment tuning: V=DVE, P=Pool(gpsimd), A=Act
# (scans must run on DVE: walrus cannot lower tensor_tensor_scan on Pool)
CFG = dict(
    bcopy=os.environ.get("CFG_BCOPY", "A" * 16),     # per s: B broadcast copy
    ccopy=os.environ.get("CFG_CCOPY", "A" * 16),     # per s: C broadcast copy
    carry=os.environ.get("CFG_CARRY", "P"),          # batched carry copies
    bt=os.environ.get("CFG_BT", ""),                 # per (s*NCH+m): bt engine
    cm=os.environ.get("CFG_CM", ""),                 # per (s*NCH+m): cmul engine
    hotbufs=int(os.environ.get("CFG_HOTBUFS", "2")),  # bufs for s-loop tags
    wd=os.environ.get("CFG_WD", "V"),
    skip=os.environ.get("CFG_SKIP", "V"),
    gate=os.environ.get("CFG_GATE", "V"),
    comb=os.environ.get("CFG_COMB", "V"),
    conv=os.environ.get("CFG_CONV", "V"),            # V=DVE STT, E=PE diag-mm
    obcopy=os.environ.get("CFG_OBCOPY", "A"),        # out_proj PSUM->SBUF copy
    flip=os.environ.get("CFG_FLIP", "A"),            # AR reverse stage copy
)
def _bres(k, n=64):
    out = []
    acc = 0
    for _ in range(n):
        acc += k
        if acc >= n:
            acc -= n
            out.append("V")
        else:
            out.append("P")
    return "".join(out)


if not CFG["bt"]:
    CFG["bt"] = _bres(39)
if not CFG["cm"]:
    CFG["cm"] = _bres(39)

_COMPILED = [None]


def _split_sync_waits(nc, mybir, max_waits=1):
    """walrus in this environment rejects >1 sync wait per instruction;
    hoist excess waits onto dedicated same-engine NOPs."""
    uid = [0]
    for f in nc.m.functions:
        for bb in f.blocks:
            new = []
            dirty = False
            for inst in bb.instructions:
                si = inst.sync_info
                if si is not None and len(si.on_wait) > max_waits:
                    waits = list(si.on_wait)
                    keep = waits[len(waits) - max_waits:]
                    hoist = waits[: len(waits) - max_waits]
                    for i in range(0, len(hoist), max_waits):
                        uid[0] += 1
                        nop = mybir.InstNoOp(
                            name=f"splitwait-{id(nc)}-{uid[0]}", engine=inst.engine
                        )
                        nop.sync_info = mybir.SyncInfo(
                            on_wait=hoist[i : i + max_waits], on_update=[]
                        )
                        nc.register_instruction(nop, overwrite=True)
                        new.append(nop)
                    inst.sync_info = mybir.SyncInfo(
                        on_wait=keep, on_update=list(si.on_update)
                    )
                    dirty = True
                new.append(inst)
            if dirty:
                bb.instructions = new


def _build_program(debug=False, collective=True):
    import concourse.bass as bass
    import concourse.tile as tile
    from concourse import mybir

    f32 = mybir.dt.float32
    f32r = mybir.dt.float32r
    bf16 = mybir.dt.bfloat16
    AF = mybir.ActivationFunctionType
    OP = mybir.AluOpType

    nc = bass.Bass("TRN2", target_bir_lowering=False, debug=False, num_devices=8)

    # ---- external inputs (per-core shards prepared on host) ----
    hT = nc.dram_tensor("hT", [DM, L], bf16, kind="ExternalInput")
    winxT = nc.dram_tensor("winxT", [DM, CH], bf16, kind="ExternalInput")
    winzT = nc.dram_tensor("winzT", [DM, CH], bf16, kind="ExternalInput")
    woutT_d = nc.dram_tensor("woutT", [CH, DM], bf16, kind="ExternalInput")
    sel_d = nc.dram_tensor("sel", [48, DS * 128], bf16, kind="ExternalInput")
    ident_d = nc.dram_tensor("ident", [128, 128], bf16, kind="ExternalInput")
    wx_d = {}
    wdt_d = {}
    A_d = {}
    cw_d = {}
    cwdiag_d = {}
    cb_d = {}
    db_d = {}
    D_d = {}
    for d in ("f", "r"):
        wx_d[d] = nc.dram_tensor(f"wx_{d}", [CH, 128], bf16, kind="ExternalInput")
        wdt_d[d] = nc.dram_tensor(f"wdt_{d}", [DR, CH], bf16, kind="ExternalInput")
        A_d[d] = nc.dram_tensor(f"A_{d}", [128, NCH * DS], f32, kind="ExternalInput")
        cw_d[d] = nc.dram_tensor(f"cw_{d}", [128, NCH * DC], f32,
                                 kind="ExternalInput")
        cwdiag_d[d] = nc.dram_tensor(f"cwdiag_{d}", [NCH * DC, 128, 128], bf16,
                                     kind="ExternalInput")
        cb_d[d] = nc.dram_tensor(f"cb_{d}", [128, NCH], f32, kind="ExternalInput")
        db_d[d] = nc.dram_tensor(f"db_{d}", [128, NCH], f32, kind="ExternalInput")
        D_d[d] = nc.dram_tensor(f"D_{d}", [128, NCH], f32, kind="ExternalInput")

    pout = nc.dram_tensor("pout", [L, DM], bf16, kind="ExternalOutput")
    dbg = {}
    if debug:
        dbg["xc_f"] = nc.dram_tensor("dbg_xc_f", [NCH, 128, L], bf16, kind="ExternalOutput")
        dbg["xc_r"] = nc.dram_tensor("dbg_xc_r", [NCH, 128, L], bf16, kind="ExternalOutput")
        dbg["xdbl_f"] = nc.dram_tensor("dbg_xdbl_f", [96, L], f32, kind="ExternalOutput")
        dbg["xdbl_r"] = nc.dram_tensor("dbg_xdbl_r", [96, L], f32, kind="ExternalOutput")
        dbg["dt_f"] = nc.dram_tensor("dbg_dt_f", [NCH, 128, L], f32, kind="ExternalOutput")
        dbg["y_f"] = nc.dram_tensor("dbg_y_f", [NCH, 128, L], bf16, kind="ExternalOutput")
        dbg["siluz"] = nc.dram_tensor("dbg_siluz", [NCH, 128, L], bf16, kind="ExternalOutput")

    with tile.TileContext(nc, num_cores=8) as tc:
        _build_tile_program(
            nc, tc, tile, mybir, f32, f32r, bf16, AF, OP,
            hT, winxT, winzT, woutT_d, sel_d, ident_d, wx_d, wdt_d, A_d, cw_d,
            cwdiag_d, cb_d, db_d, D_d, pout, dbg, collective,
        )

    _split_sync_waits(nc, mybir)
    return nc


def _build_tile_program(
    nc, tc, tile, mybir, f32, f32r, bf16, AF, OP,
    hT, winxT, winzT, woutT_d, sel_d, ident_d, wx_d, wdt_d, A_d, cw_d,
    cwdiag_d, cb_d, db_d, D_d, pout, dbg, collective=True,
):
    from contextlib import ExitStack

    MM = nc.tensor.matmul
    ACT = nc.scalar.activation
    TT = nc.vector.tensor_tensor
    STT = nc.vector.scalar_tensor_tensor
    TSMUL = nc.vector.tensor_scalar_mul

    def veng(code):
        return nc.vector if code == "V" else nc.gpsimd

    def bcopy(code, out, in_):
        """PSUM f32 -> SBUF copy on the chosen engine."""
        if code == "A":
            ACT(out, in_, AF.Copy)
        else:
            veng(code).tensor_copy(out, in_)

    ctx = ExitStack()
    with ctx:
        # -------- persistent pools --------
        pers = ctx.enter_context(tc.tile_pool(name="pers", bufs=1))
        dram = ctx.enter_context(tc.tile_pool(name="dram", bufs=1, space="DRAM"))

        wout_sb = pers.tile([128, NCH, DM], bf16)
        nc.sync.dma_start(wout_sb[:], woutT_d.ap().rearrange("(k p) n -> p k n", p=128))
        sel_sb = pers.tile([48, DS * 128], bf16)
        nc.sync.dma_start(sel_sb[:], sel_d[:])
        ident_sb = pers.tile([128, 128], bf16)
        nc.sync.dma_start(ident_sb[:], ident_d[:])
        xdbl = {}      # bf16 [128, L]: rows [0:16]=B, [32:48]=C, [64:128]=dt-rank
        carry = {}
        wdt_sb = {}
        A_sb = {}
        db_sb = {}
        D_sb = {}
        for d in ("f", "r"):
            xdbl[d] = pers.tile([128, L], bf16, name=f"xdbl_{d}")
            carry[d] = pers.tile([128, NCH, DS], bf16, name=f"carry_{d}")
            nc.vector.memset(carry[d][:], 0.0)
            wdt_sb[d] = pers.tile([128, CH], bf16, name=f"wdt_sb_{d}")
            nc.sync.dma_start(wdt_sb[d][DR:128, :], wdt_d[d][:])
            A_sb[d] = pers.tile([128, NCH, DS], f32, name=f"A_sb_{d}")
            nc.sync.dma_start(A_sb[d][:], A_d[d].ap().rearrange("p (m s) -> p m s", m=NCH))
            db_sb[d] = pers.tile([128, NCH], f32, name=f"db_sb_{d}")
            nc.sync.dma_start(db_sb[d][:], db_d[d][:])
            D_sb[d] = pers.tile([128, NCH], f32, name=f"D_sb_{d}")
            nc.sync.dma_start(D_sb[d][:], D_d[d][:])
        ones = pers.tile([128, 1], f32)
        nc.vector.memset(ones[:], 1.0)

        # DRAM spill buffers (per-core local HBM); all in ORIGINAL time order
        # for the forward direction; xr/sz are original-time too (pass 2r
        # flips with reversed SBUF reads).  ygr is in flipped time.
        xf_dram = dram.tile([NCH, 128, L], bf16)
        xr_dram = dram.tile([NCH, 128, L], bf16)
        sz_dram = dram.tile([NCH, 128, L], bf16)
        ygr_dram = dram.tile([NCH, 128, L], bf16)
        # AllReduce staging: [dir, 128 rows, L] f32; rows as xdbl layout.
        # dir 0 = forward (original time), dir 1 = reverse (flipped time).
        ar_in = dram.tile([2, 128, L], f32)
        ar_out = dram.tile([2, 128, L], f32)

        # ================= PASS 1: in_proj + conv + silu + partial x_dbl ====
        with tc.tile_pool(name="p1", bufs=1) as p1, \
             tc.tile_pool(name="p1psum", bufs=1, space="PSUM") as p1psum:
            winx_sb = p1.tile([128, DM // 128, CH], bf16)
            nc.sync.dma_start(winx_sb[:], winxT.ap().rearrange("(k p) n -> p k n", p=128))
            winz_sb = p1.tile([128, DM // 128, CH], bf16)
            nc.sync.dma_start(winz_sb[:], winzT.ap().rearrange("(k p) n -> p k n", p=128))
            wx_sb = {}
            cw_sb = {}
            cb_sb = {}
            for d in ("f", "r"):
                wx_sb[d] = p1.tile([128, NCH, 128], bf16, name=f"wx_sb_{d}")
                nc.sync.dma_start(wx_sb[d][:], wx_d[d].ap().rearrange("(m p) n -> p m n", p=128))
                if CFG["conv"] == "E":
                    cw_sb[d] = p1.tile([128, NCH * DC, 128], bf16,
                                       name=f"cw_sb_{d}")
                    nc.sync.dma_start(
                        cw_sb[d][:], cwdiag_d[d].ap().rearrange("k p n -> p k n"))
                else:
                    cw_sb[d] = p1.tile([128, NCH, DC], f32, name=f"cw_sb_{d}")
                    nc.sync.dma_start(
                        cw_sb[d][:], cw_d[d].ap().rearrange("p (m j) -> p m j", m=NCH))
                cb_sb[d] = p1.tile([128, NCH], f32, name=f"cb_sb_{d}")
                nc.sync.dma_start(cb_sb[d][:], cb_d[d][:])

            hT_r = hT.ap().rearrange("(k p) l -> p k l", p=128)
            prev_xe = [None] * NCH

            def conv_dir(cc, d, xe_list):
                """Causal (d=f) / anti-causal (d=r) depthwise conv + silu on
                original-time chunk cc, using extended tiles [3|T1|3].
                Conv runs on DVE (tap0 as 4x tensor_scalar, taps 1-3 as
                STT accumulate).  Returns bf16 silu'd tiles."""
                out = []
                for m in range(NCH):
                    xe = xe_list[m]
                    if CFG["conv"] == "E":
                        acc = p1psum.tile([128, T1], f32, tag="cps", bufs=2,
                                          name=f"cps{m}_{d}_{cc}")
                        for j in range(DC):
                            off = j if d == "f" else (6 - j)
                            MM(acc[:], cw_sb[d][:, m * DC + j, :],
                               xe[:, off : off + T1],
                               start=(j == 0), stop=(j == DC - 1))
                    else:
                        acc = p1.tile([128, T1], f32, tag=f"cacc{m}", bufs=2,
                                      name=f"cacc{m}_{d}_{cc}")
                        for j in range(DC):
                            off = j if d == "f" else (6 - j)
                            src = xe[:, off : off + T1]
                            wj = cw_sb[d][:, m, j : j + 1]
                            if j == 0:
                                TSMUL(acc[:], src, wj)
                            else:
                                STT(acc[:], src, wj, acc[:], OP.mult, OP.add)
                    xcb = p1.tile([128, T1], bf16, tag=f"xcb{m}_{d}", bufs=2,
                                  name=f"xcb{m}_{d}_{cc}")
                    ACT(xcb[:], acc[:], AF.Silu, bias=cb_sb[d][:, m : m + 1])
                    out.append(xcb)
                return out

            def xdbl_chunk(cc, d, xc_tiles):
                # psum rows laid out as [B 0:16 | C 32:48 | dt 64:128]
                # (W_x rows reordered+padded on host); full 128 rows go to AR.
                ps = p1psum.tile([128, T1], f32, tag="psx", bufs=2,
                                 name=f"psx_{d}_{cc}")
                for m in range(NCH):
                    MM(ps[:], wx_sb[d][:, m, :], xc_tiles[m][:],
                       start=(m == 0), stop=(m == NCH - 1))
                stage = p1.tile([128, T1], f32, tag="arstage", bufs=2,
                                name=f"arstage_{d}_{cc}")
                if d == "f":
                    ACT(stage[:], ps[:], AF.Copy)
                    nc.sync.dma_start(
                        ar_in[0, :, cc * T1 : (cc + 1) * T1], stage[:]
                    )
                else:
                    if CFG["flip"] == "A":
                        ACT(stage[:], ps[:, ::-1], AF.Copy)
                    else:
                        nc.vector.tensor_copy(stage[:], ps[:, ::-1])
                    nc.sync.dma_start(
                        ar_in[1, :, L - (cc + 1) * T1 : L - cc * T1], stage[:]
                    )

            def spill_chunk(cc, d, xc_tiles):
                x_dram = xf_dram if d == "f" else xr_dram
                for m in range(NCH):
                    nc.sync.dma_start(
                        x_dram[m, :, cc * T1 : (cc + 1) * T1], xc_tiles[m][:]
                    )
                    if dbg:
                        key = "xc_f" if d == "f" else "xc_r"
                        nc.sync.dma_start(
                            dbg[key][m, :, cc * T1 : (cc + 1) * T1], xc_tiles[m][:]
                        )

            def finish_reverse(cc, xe_list):
                xcr = conv_dir(cc, "r", xe_list)
                xdbl_chunk(cc, "r", xcr)
                spill_chunk(cc, "r", xcr)

            for c in range(NC1):
                hTt = p1.tile([128, DM // 128, T1], bf16, tag="hTt", bufs=2,
                              name=f"hTt_{c}")
                nc.sync.dma_start(hTt[:], hT_r[:, :, c * T1 : (c + 1) * T1])

                # x part (extended with halos) and z part (-> silu -> spill)
                cur_xe = []
                for m in range(NCH):
                    ps = p1psum.tile([128, T1], f32, tag="ps_ip", bufs=2,
                                     name=f"psx_{c}_{m}")
                    for ko in range(DM // 128):
                        MM(ps[:], winx_sb[:, ko, m * 128 : (m + 1) * 128],
                           hTt[:, ko, :], start=(ko == 0), stop=(ko == DM // 128 - 1))
                    xe = p1.tile([128, T1 + 6], bf16, tag=f"xe{m}", bufs=3,
                                 name=f"xe{m}_{c}")
                    ACT(xe[:, 3 : 3 + T1], ps[:], AF.Copy)
                    if c == 0:
                        nc.vector.memset(xe[:, 0:3], 0.0)
                    else:
                        nc.vector.tensor_copy(xe[:, 0:3], prev_xe[m][:, T1 : T1 + 3])
                    cur_xe.append(xe)
                for m in range(NCH):
                    ps = p1psum.tile([128, T1], f32, tag="ps_ip", bufs=2,
                                     name=f"psz_{c}_{m}")
                    for ko in range(DM // 128):
                        MM(ps[:], winz_sb[:, ko, m * 128 : (m + 1) * 128],
                           hTt[:, ko, :], start=(ko == 0), stop=(ko == DM // 128 - 1))
                    zs = p1.tile([128, T1], bf16, tag=f"zs{m}", bufs=2,
                                 name=f"zs{m}_{c}")
                    ACT(zs[:], ps[:], AF.Silu)
                    nc.sync.dma_start(sz_dram[m, :, c * T1 : (c + 1) * T1], zs[:])
                    if dbg:
                        nc.sync.dma_start(
                            dbg["siluz"][m, :, c * T1 : (c + 1) * T1], zs[:]
                        )

                if c > 0:
                    # fill previous chunk's right halo, then do its reverse conv
                    for m in range(NCH):
                        nc.vector.tensor_copy(
                            prev_xe[m][:, T1 + 3 : T1 + 6], cur_xe[m][:, 3:6]
                        )
                    finish_reverse(c - 1, prev_xe)

                # forward conv on current chunk
                xcf = conv_dir(c, "f", cur_xe)
                xdbl_chunk(c, "f", xcf)
                spill_chunk(c, "f", xcf)

                prev_xe = cur_xe

            for m in range(NCH):
                nc.vector.memset(prev_xe[m][:, T1 + 3 : T1 + 6], 0.0)
            finish_reverse(NC1 - 1, prev_xe)

            # -------- AllReduce of x_dbl over the 4 cores of this batch ----
            # reverse direction first: pass 2r starts as soon as its rows
            # are reduced, overlapping the forward AR
            for di, d in ((1, "r"), (0, "f")):
                if collective:
                    nc.gpsimd.collective_compute(
                        "AllReduce", OP.add,
                        replica_groups=[[0, 1, 2, 3], [4, 5, 6, 7]],
                        ins=[ar_in[di].opt()], outs=[ar_out[di].opt()],
                    )
                else:
                    nc.gpsimd.dma_start(ar_out[di], ar_in[di])
                # cast-readback f32 -> bf16 into SBUF (gpsimd DMAs may cast)
                nc.gpsimd.dma_start(xdbl[d][:], ar_out[di, :, :])
            if dbg:
                for di, d in enumerate(("f", "r")):
                    nc.sync.dma_start(dbg[f"xdbl_{d}"][0:64, :], ar_out[di, 64:128, :])
                    nc.sync.dma_start(dbg[f"xdbl_{d}"][64:80, :], ar_out[di, 0:16, :])
                    nc.sync.dma_start(dbg[f"xdbl_{d}"][80:96, :], ar_out[di, 32:48, :])

        # ================= PASS 2: dt + selective scan (+gating, out_proj) ==
        def scan_pass(d, p2, p2psum, ytot_cb, mmt_bufs=3):
            """d: 'f' or 'r'.  'r' reads x/sz spills (original time) with
            reversed SBUF access; everything else runs in flipped time.
            ytot_cb(c2, yg_tiles): consumes gated y tiles for chunk c2."""
            x_dram = xf_dram if d == "f" else xr_dram
            rev = (lambda ap: ap) if d == "f" else (lambda ap: ap[:, ::-1])
            for c2 in range(NC2):
                sl = slice(c2 * T2, (c2 + 1) * T2)
                osl = sl if d == "f" else slice(L - (c2 + 1) * T2, L - c2 * T2)
                # ---- dt projection + softplus (f32 path) ----
                dt_sb = []
                for m in range(NCH):
                    psd = p2psum.tile([128, T2], f32, tag="mmt", bufs=mmt_bufs,
                                      name=f"psd_{d}_{c2}_{m}")
                    MM(psd[:], wdt_sb[d][DR:128, m * 128 : (m + 1) * 128],
                       xdbl[d][DR:128, sl], start=True, stop=True)
                    et = p2.tile([128, T2], f32, tag="et", bufs=2,
                                 name=f"et_{d}_{c2}_{m}")
                    ACT(et[:], psd[:], AF.Exp, bias=db_sb[d][:, m : m + 1])
                    dt = p2.tile([128, T2], bf16, tag=f"dt{m}", bufs=2,
                                 name=f"dt{m}_{d}_{c2}")
                    ACT(dt[:], et[:], AF.Ln, bias=ones[:])
                    dt_sb.append(dt)
                    if dbg and d == "f":
                        nc.sync.dma_start(dbg["dt_f"][m, :, sl], dt[:])
                # ---- x load (bf16) + wd = dt*x + silu(z) load ----
                xd = []
                wd = []
                szt = []
                for m in range(NCH):
                    xt = p2.tile([128, T2], bf16, tag=f"xd{m}", bufs=2,
                                 name=f"xd{m}_{d}_{c2}")
                    nc.sync.dma_start(xt[:], x_dram[m, :, osl])
                    xd.append(xt)
                    wt = p2.tile([128, T2], bf16, tag=f"wd{m}", bufs=2,
                                 name=f"wd{m}_{d}_{c2}")
                    veng(CFG["wd"]).tensor_tensor(
                        wt[:], dt_sb[m][:], rev(xt[:]), OP.mult)
                    wd.append(wt)
                    sz = p2.tile([128, T2], bf16, tag=f"sz{m}", bufs=2,
                                 name=f"sz{m}_{d}_{c2}")
                    nc.sync.dma_start(sz[:], sz_dram[m, :, osl])
                    szt.append(sz)
                # ---- selective scan over 16 states ----
                yps = [p2psum.tile([128, T2], f32, tag=f"yp{m}", bufs=1,
                                   name=f"yp{m}_{d}_{c2}") for m in range(NCH)]
                for s in range(DS):
                    Bbp = p2psum.tile([128, T2], f32, tag="mmt", bufs=mmt_bufs,
                                      name=f"Bbp_{d}_{c2}_{s}")
                    MM(Bbp[:], sel_sb[0:DS, s * 128 : (s + 1) * 128],
                       xdbl[d][0:DS, sl], start=True, stop=True)
                    Bb = p2.tile([128, T2], bf16, tag="Bbs", bufs=CFG["hotbufs"],
                                 name=f"Bb_{d}_{c2}_{s}")
                    bcopy(CFG["bcopy"][s], Bb[:], Bbp[:])
                    Cbp = p2psum.tile([128, T2], f32, tag="mmt", bufs=mmt_bufs,
                                      name=f"Cbp_{d}_{c2}_{s}")
                    MM(Cbp[:], sel_sb[32 : 32 + DS, s * 128 : (s + 1) * 128],
                       xdbl[d][32 : 32 + DS, sl], start=True, stop=True)
                    Cb = p2.tile([128, T2], bf16, tag="Cbs", bufs=CFG["hotbufs"],
                                 name=f"Cb_{d}_{c2}_{s}")
                    bcopy(CFG["ccopy"][s], Cb[:], Cbp[:])
                    bt = []
                    for m in range(NCH):
                        b = p2.tile([128, T2], bf16, tag=f"bt{m}", bufs=CFG["hotbufs"],
                                    name=f"bt_{d}_{c2}_{s}_{m}")
                        veng(CFG["bt"][s * NCH + m]).tensor_tensor(
                            b[:], wd[m][:], Bb[:], OP.mult)
                        bt.append(b)
                    dAs = []
                    for m in range(NCH):
                        dA = p2.tile([128, T2], f32, tag=f"dA{m}", bufs=CFG["hotbufs"],
                                     name=f"dA_{d}_{c2}_{s}_{m}")
                        ACT(dA[:], dt_sb[m][:], AF.Exp,
                            scale=A_sb[d][:, m, s : s + 1])
                        dAs.append(dA)
                    # per-state hs tile holding all 4 channel groups, so the
                    # chunk-boundary carry is ONE strided copy per state
                    hs = p2.tile([128, NCH, T2], bf16, tag="hs", bufs=2,
                                 name=f"hs_{d}_{c2}_{s}")
                    for m in range(NCH):
                        nc.vector.tensor_tensor_scan(
                            hs[:, m, :], dAs[m][:], bt[m][:],
                            carry[d][:, m, s : s + 1], OP.mult, OP.add)
                    veng(CFG["carry"]).tensor_copy(
                        carry[d][:, :, s : s + 1], hs[:, :, T2 - 1 : T2])
                    for m in range(NCH):
                        cm = p2.tile([128, T2], bf16, tag=f"cm{m}", bufs=CFG["hotbufs"],
                                     name=f"cm_{d}_{c2}_{s}_{m}")
                        veng(CFG["cm"][s * NCH + m]).tensor_tensor(
                            cm[:], hs[:, m, :], Cb[:], OP.mult)
                        MM(yps[m][:], ident_sb[:], cm[:],
                           start=(s == 0), stop=(s == DS - 1))
                # ---- gating: y = (ypsum + x*D) * silu(z) ----
                yg = []
                for m in range(NCH):
                    y1 = p2.tile([128, T2], bf16, tag=f"y1{m}", bufs=2,
                                 name=f"y1_{d}_{c2}_{m}")
                    veng(CFG["skip"]).scalar_tensor_tensor(
                        y1[:], rev(xd[m][:]), D_sb[d][:, m : m + 1], yps[m][:],
                        OP.mult, OP.add)
                    yt = p2.tile([128, T2], bf16, tag=f"yg{m}", bufs=2,
                                 name=f"yg_{d}_{c2}_{m}")
                    veng(CFG["gate"]).tensor_tensor(
                        yt[:], y1[:], rev(szt[m][:]), OP.mult)
                    yg.append(yt)
                ytot_cb(c2, yg)

        # ---- pass 2: reverse (spill gated y), then forward (combine +
        # out_proj); one shared pool so the passes overlap at the seam ----
        with tc.tile_pool(name="p2", bufs=1) as p2f, \
             tc.tile_pool(name="p2psum", bufs=1, space="PSUM") as p2fpsum:

            def spill_ygr(c2, yg):
                for m in range(NCH):
                    nc.sync.dma_start(
                        ygr_dram[m, :, c2 * T2 : (c2 + 1) * T2], yg[m][:]
                    )

            scan_pass("r", p2f, p2fpsum, spill_ygr, mmt_bufs=2)

            def combine_out(c2, yg):
                ytot = []
                for m in range(NCH):
                    ygr_t = p2f.tile([128, T2], bf16, tag=f"ygr{m}", bufs=2,
                                     name=f"ygr{m}_{c2}")
                    nc.sync.dma_start(
                        ygr_t[:], ygr_dram[m, :, L - (c2 + 1) * T2 : L - c2 * T2]
                    )
                    yt2 = p2f.tile([128, T2], bf16, tag=f"ytot{m}", bufs=2,
                                   name=f"ytot{m}_{c2}")
                    veng(CFG["comb"]).tensor_tensor(
                        yt2[:], yg[m][:], ygr_t[:, ::-1], OP.add)
                    ytot.append(yt2)
                    if dbg:
                        nc.sync.dma_start(
                            dbg["y_f"][m, :, c2 * T2 : (c2 + 1) * T2], yg[m][:]
                        )
                for mt in range(T2 // 128):
                    ob = p2f.tile([128, DM], bf16, tag="ob", bufs=2,
                                  name=f"ob_{c2}_{mt}")
                    for nh in range(DM // 512):
                        po = p2fpsum.tile([128, 512], f32, tag="po", bufs=2,
                                          name=f"po_{c2}_{mt}_{nh}")
                        for k in range(NCH):
                            MM(po[:], ytot[k][:, mt * 128 : (mt + 1) * 128],
                               wout_sb[:, k, nh * 512 : (nh + 1) * 512],
                               start=(k == 0), stop=(k == NCH - 1))
                        bcopy(CFG["obcopy"], ob[:, nh * 512 : (nh + 1) * 512],
                              po[:])
                    nc.sync.dma_start(
                        pout[c2 * T2 + mt * 128 : c2 * T2 + (mt + 1) * 128, :],
                        ob[:],
                    )

            scan_pass("f", p2f, p2fpsum, combine_out, mmt_bufs=2)


def _host_prep(inputs):
    """Slice/transpose the full inputs into the 8 per-core input maps."""
    import ml_dtypes
    bf = ml_dtypes.bfloat16

    h = np.asarray(inputs["hidden_states"], np.float32)
    W_in = np.asarray(inputs["W_in"], np.float32)
    W_out = np.asarray(inputs["W_out"], np.float32)

    sel = np.zeros((48, DS * 128), np.float32)
    for s in range(DS):
        sel[s, s * 128 : (s + 1) * 128] = 1.0
        sel[32 + s, s * 128 : (s + 1) * 128] = 1.0

    maps = []
    for core in range(8):
        b, g = divmod(core, 4)
        c0 = g * CH
        m = {
            "hT": np.ascontiguousarray(h[b].T).astype(bf),
            "winxT": np.ascontiguousarray(W_in[c0 : c0 + CH, :].T).astype(bf),
            "winzT": np.ascontiguousarray(W_in[DI + c0 : DI + c0 + CH, :].T).astype(bf),
            "woutT": np.ascontiguousarray(W_out[:, c0 : c0 + CH].T).astype(bf),
            "sel": sel.astype(bf),
            "ident": np.eye(128, dtype=np.float32).astype(bf),
        }
        for d in ("f", "r"):
            sfx = f"_{d}"
            W_x = np.asarray(inputs[f"W_x{sfx}"], np.float32)
            W_dt = np.asarray(inputs[f"W_dt{sfx}"], np.float32)
            A = -np.exp(np.asarray(inputs[f"A_log{sfx}"], np.float64)).astype(np.float32)
            cw = np.asarray(inputs[f"conv_w{sfx}"], np.float32)
            cb = np.asarray(inputs[f"conv_b{sfx}"], np.float32)
            db = np.asarray(inputs[f"b_dt{sfx}"], np.float32)
            Dp = np.asarray(inputs[f"D{sfx}"], np.float32)
            wx_re = np.zeros((CH, 128), np.float32)
            wx_re[:, 0:DS] = W_x[DR : DR + DS, c0 : c0 + CH].T        # B rows
            wx_re[:, 32 : 32 + DS] = W_x[DR + DS : 96, c0 : c0 + CH].T  # C rows
            wx_re[:, DR:128] = W_x[0:DR, c0 : c0 + CH].T              # dt-rank rows
            m[f"wx{sfx}"] = wx_re.astype(bf)
            m[f"wdt{sfx}"] = np.ascontiguousarray(W_dt[c0 : c0 + CH, :].T).astype(bf)
            # (CH, DS) -> (128, NCH, DS) -> (128, NCH*DS)
            m[f"A{sfx}"] = np.ascontiguousarray(
                A[c0 : c0 + CH].reshape(NCH, 128, DS).transpose(1, 0, 2).reshape(128, NCH * DS)
            )
            m[f"cw{sfx}"] = np.ascontiguousarray(
                cw[c0 : c0 + CH].reshape(NCH, 128, DC).transpose(1, 0, 2).reshape(128, NCH * DC)
            )
            cwd = np.zeros((NCH * DC, 128, 128), np.float32)
            cwc = cw[c0 : c0 + CH].reshape(NCH, 128, DC)
            for mm_ in range(NCH):
                for j in range(DC):
                    np.fill_diagonal(cwd[mm_ * DC + j], cwc[mm_, :, j])
            m[f"cwdiag{sfx}"] = cwd.astype(bf)
            m[f"cb{sfx}"] = np.ascontiguousarray(
                cb[c0 : c0 + CH].reshape(NCH, 128).T
            )
            m[f"db{sfx}"] = np.ascontiguousarray(
                db[c0 : c0 + CH].reshape(NCH, 128).T
            )
            m[f"D{sfx}"] = np.ascontiguousarray(
                Dp[c0 : c0 + CH].reshape(NCH, 128).T
            )
        maps.append(m)
    return maps


def run(inputs, debug=False, trace=False):
    from concourse.bass_utils import run_bass_kernel_spmd

    if _COMPILED[0] is None or _COMPILED[0][1] != debug:
        _COMPILED[0] = (_build_program(debug=debug), debug)
    nc = _COMPILED[0][0]
    maps = _host_prep(inputs)
    res = run_bass_kernel_spmd(nc, maps, core_ids=list(range(8)), trace=trace)
    outs = [np.asarray(r["pout"], np.float32) for r in res.results]
    full = np.zeros((B, L, DM), np.float32)
    for core in range(8):
        b = core // 4
        full[b] += outs[core]
    return full, res


def kernel(**inputs):
    out, _ = run(inputs, debug=False, trace=False)
    return out


# revision 21
# speedup vs baseline: 1.1523x; 1.0014x over previous
"""BiMamba (bidirectional Mamba block) Trainium2 kernel.

Contract: kernel(**inputs) takes the full (unsharded) numpy inputs of the
reference and returns the full (2, 4096, 1024) float32 output.

Sharding: 8 cores = 2 batches x 4 channel-groups of 512 d_inner channels.
Each core runs both scan directions for its channel slice; the x_dbl
reduction over d_inner is an on-chip AllReduce within each batch's 4-core
group; the host sums the four partial out-projections per batch.

Key algebraic facts used:
  * xz for the reverse direction is the L-flip of the forward xz, so the
    input projection is computed once.
  * (y_f + flip(y_r)) @ W_out.T == out_f + flip(out_r), so one output
    projection suffices.

Performance structure (engine balance per scan chunk):
  * Pool (gpsimd) runs the 64 tensor_tensor_scan ops (the serial core).
  * DVE runs the bf16 TensorTensor mults (2x_1p packed mode).
  * Act runs the exp/softplus and most PSUM->SBUF broadcast copies.
  * PE accumulates y over the 16 states via identity matmuls into PSUM,
    plus the projections.
  * All DMA uses contiguous descriptors (reversals happen in SBUF reads).
"""

import os
import sys

import numpy as np

sys.path.insert(0, "/opt/trn_rl_repo")

B, L, DM, DI, DS, DR, DC = 2, 4096, 1024, 2048, 16, 64, 4
CH = 512          # d_inner channels per core
NCH = CH // 128   # channel tiles per core
T1 = 512          # pass-1 (projection/conv) token chunk
NC1 = L // T1
T2 = 512          # pass-2 (scan) token chunk
NC2 = L // T2

# engine assignment tuning: V=DVE, P=Pool(gpsimd), A=Act
# (scans must run on DVE: walrus cannot lower tensor_tensor_scan on Pool)
CFG = dict(
    bcopy=os.environ.get("CFG_BCOPY", "A" * 16),     # per s: B broadcast copy
    ccopy=os.environ.get("CFG_CCOPY", "A" * 16),     # per s: C broadcast copy
    carry=os.environ.get("CFG_CARRY", "P"),          # batched carry copies
    bt=os.environ.get("CFG_BT", ""),                 # per (s*NCH+m): bt engine
    cm=os.environ.get("CFG_CM", ""),                 # per (s*NCH+m): cmul engine
    hotbufs=int(os.environ.get("CFG_HOTBUFS", "3")),  # bufs for s-loop tags
    wd=os.environ.get("CFG_WD", "V"),
    skip=os.environ.get("CFG_SKIP", "V"),
    gate=os.environ.get("CFG_GATE", "V"),
    comb=os.environ.get("CFG_COMB", "V"),
    conv=os.environ.get("CFG_CONV", "V"),            # V=DVE STT, E=PE diag-mm
    obcopy=os.environ.get("CFG_OBCOPY", "A"),        # out_proj PSUM->SBUF copy
    flip=os.environ.get("CFG_FLIP", "A"),            # AR reverse stage copy
    p1bufs=int(os.environ.get("CFG_P1BUFS", "2")),   # pass-1 in_proj psum bufs
)
def _bres(k, n=64):
    out = []
    acc = 0
    for _ in range(n):
        acc += k
        if acc >= n:
            acc -= n
            out.append("V")
        else:
            out.append("P")
    return "".join(out)


if not CFG["bt"]:
    CFG["bt"] = _bres(39)
if not CFG["cm"]:
    CFG["cm"] = _bres(39)

_COMPILED = [None]


def _split_sync_waits(nc, mybir, max_waits=1):
    """walrus in this environment rejects >1 sync wait per instruction;
    hoist excess waits onto dedicated same-engine NOPs."""
    uid = [0]
    for f in nc.m.functions:
        for bb in f.blocks:
            new = []
            dirty = False
            for inst in bb.instructions:
                si = inst.sync_info
                if si is not None and len(si.on_wait) > max_waits:
                    waits = list(si.on_wait)
                    keep = waits[len(waits) - max_waits:]
                    hoist = waits[: len(waits) - max_waits]
                    for i in range(0, len(hoist), max_waits):
                        uid[0] += 1
                        nop = mybir.InstNoOp(
                            name=f"splitwait-{id(nc)}-{uid[0]}", engine=inst.engine
                        )
                        nop.sync_info = mybir.SyncInfo(
                            on_wait=hoist[i : i + max_waits], on_update=[]
                        )
                        nc.register_instruction(nop, overwrite=True)
                        new.append(nop)
                    inst.sync_info = mybir.SyncInfo(
                        on_wait=keep, on_update=list(si.on_update)
                    )
                    dirty = True
                new.append(inst)
            if dirty:
                bb.instructions = new


def _build_program(debug=False, collective=True):
    import concourse.bass as bass
    import concourse.tile as tile
    from concourse import mybir

    f32 = mybir.dt.float32
    f32r = mybir.dt.float32r
    bf16 = mybir.dt.bfloat16
    AF = mybir.ActivationFunctionType
    OP = mybir.AluOpType

    nc = bass.Bass("TRN2", target_bir_lowering=False, debug=False, num_devices=8)

    # ---- external inputs (per-core shards prepared on host) ----
    hT = nc.dram_tensor("hT", [DM, L], bf16, kind="ExternalInput")
    winxT = nc.dram_tensor("winxT", [DM, CH], bf16, kind="ExternalInput")
    winzT = nc.dram_tensor("winzT", [DM, CH], bf16, kind="ExternalInput")
    woutT_d = nc.dram_tensor("woutT", [CH, DM], bf16, kind="ExternalInput")
    sel_d = nc.dram_tensor("sel", [48, DS * 128], bf16, kind="ExternalInput")
    ident_d = nc.dram_tensor("ident", [128, 128], bf16, kind="ExternalInput")
    wx_d = {}
    wdt_d = {}
    A_d = {}
    cw_d = {}
    cwdiag_d = {}
    cb_d = {}
    db_d = {}
    D_d = {}
    for d in ("f", "r"):
        wx_d[d] = nc.dram_tensor(f"wx_{d}", [CH, 128], bf16, kind="ExternalInput")
        wdt_d[d] = nc.dram_tensor(f"wdt_{d}", [DR, CH], bf16, kind="ExternalInput")
        A_d[d] = nc.dram_tensor(f"A_{d}", [128, NCH * DS], f32, kind="ExternalInput")
        cw_d[d] = nc.dram_tensor(f"cw_{d}", [128, NCH * DC], f32,
                                 kind="ExternalInput")
        cwdiag_d[d] = nc.dram_tensor(f"cwdiag_{d}", [NCH * DC, 128, 128], bf16,
                                     kind="ExternalInput")
        cb_d[d] = nc.dram_tensor(f"cb_{d}", [128, NCH], f32, kind="ExternalInput")
        db_d[d] = nc.dram_tensor(f"db_{d}", [128, NCH], f32, kind="ExternalInput")
        D_d[d] = nc.dram_tensor(f"D_{d}", [128, NCH], f32, kind="ExternalInput")

    pout = nc.dram_tensor("pout", [L, DM], bf16, kind="ExternalOutput")
    dbg = {}
    if debug:
        dbg["xc_f"] = nc.dram_tensor("dbg_xc_f", [NCH, 128, L], bf16, kind="ExternalOutput")
        dbg["xc_r"] = nc.dram_tensor("dbg_xc_r", [NCH, 128, L], bf16, kind="ExternalOutput")
        dbg["xdbl_f"] = nc.dram_tensor("dbg_xdbl_f", [96, L], f32, kind="ExternalOutput")
        dbg["xdbl_r"] = nc.dram_tensor("dbg_xdbl_r", [96, L], f32, kind="ExternalOutput")
        dbg["dt_f"] = nc.dram_tensor("dbg_dt_f", [NCH, 128, L], f32, kind="ExternalOutput")
        dbg["y_f"] = nc.dram_tensor("dbg_y_f", [NCH, 128, L], bf16, kind="ExternalOutput")
        dbg["siluz"] = nc.dram_tensor("dbg_siluz", [NCH, 128, L], bf16, kind="ExternalOutput")

    with tile.TileContext(nc, num_cores=8) as tc:
        _build_tile_program(
            nc, tc, tile, mybir, f32, f32r, bf16, AF, OP,
            hT, winxT, winzT, woutT_d, sel_d, ident_d, wx_d, wdt_d, A_d, cw_d,
            cwdiag_d, cb_d, db_d, D_d, pout, dbg, collective,
        )

    _split_sync_waits(nc, mybir)
    return nc


def _build_tile_program(
    nc, tc, tile, mybir, f32, f32r, bf16, AF, OP,
    hT, winxT, winzT, woutT_d, sel_d, ident_d, wx_d, wdt_d, A_d, cw_d,
    cwdiag_d, cb_d, db_d, D_d, pout, dbg, collective=True,
):
    from contextlib import ExitStack

    MM = nc.tensor.matmul
    ACT = nc.scalar.activation
    TT = nc.vector.tensor_tensor
    STT = nc.vector.scalar_tensor_tensor
    TSMUL = nc.vector.tensor_scalar_mul

    def veng(code):
        return nc.vector if code == "V" else nc.gpsimd

    def bcopy(code, out, in_):
        """PSUM f32 -> SBUF copy on the chosen engine."""
        if code == "A":
            ACT(out, in_, AF.Copy)
        else:
            veng(code).tensor_copy(out, in_)

    ctx = ExitStack()
    with ctx:
        # -------- persistent pools --------
        pers = ctx.enter_context(tc.tile_pool(name="pers", bufs=1))
        dram = ctx.enter_context(tc.tile_pool(name="dram", bufs=1, space="DRAM"))

        wout_sb = pers.tile([128, NCH, DM], bf16)
        nc.sync.dma_start(wout_sb[:], woutT_d.ap().rearrange("(k p) n -> p k n", p=128))
        sel_sb = pers.tile([48, DS * 128], bf16)
        nc.sync.dma_start(sel_sb[:], sel_d[:])
        ident_sb = pers.tile([128, 128], bf16)
        nc.sync.dma_start(ident_sb[:], ident_d[:])
        xdbl = {}      # bf16 [128, L]: rows [0:16]=B, [32:48]=C, [64:128]=dt-rank
        carry = {}
        wdt_sb = {}
        A_sb = {}
        db_sb = {}
        D_sb = {}
        for d in ("f", "r"):
            xdbl[d] = pers.tile([128, L], bf16, name=f"xdbl_{d}")
            carry[d] = pers.tile([128, NCH, DS], bf16, name=f"carry_{d}")
            nc.vector.memset(carry[d][:], 0.0)
            wdt_sb[d] = pers.tile([128, CH], bf16, name=f"wdt_sb_{d}")
            nc.sync.dma_start(wdt_sb[d][DR:128, :], wdt_d[d][:])
            A_sb[d] = pers.tile([128, NCH, DS], f32, name=f"A_sb_{d}")
            nc.sync.dma_start(A_sb[d][:], A_d[d].ap().rearrange("p (m s) -> p m s", m=NCH))
            db_sb[d] = pers.tile([128, NCH], f32, name=f"db_sb_{d}")
            nc.sync.dma_start(db_sb[d][:], db_d[d][:])
            D_sb[d] = pers.tile([128, NCH], f32, name=f"D_sb_{d}")
            nc.sync.dma_start(D_sb[d][:], D_d[d][:])
        ones = pers.tile([128, 1], f32)
        nc.vector.memset(ones[:], 1.0)

        # DRAM spill buffers (per-core local HBM); all in ORIGINAL time order
        # for the forward direction; xr/sz are original-time too (pass 2r
        # flips with reversed SBUF reads).  ygr is in flipped time.
        xf_dram = dram.tile([NCH, 128, L], bf16)
        xr_dram = dram.tile([NCH, 128, L], bf16)
        sz_dram = dram.tile([NCH, 128, L], bf16)
        ygr_dram = dram.tile([NCH, 128, L], bf16)
        # AllReduce staging: [dir, 128 rows, L] f32; rows as xdbl layout.
        # dir 0 = forward (original time), dir 1 = reverse (flipped time).
        ar_in = dram.tile([2, 128, L], f32)
        ar_out = dram.tile([2, 128, L], f32)

        # ================= PASS 1: in_proj + conv + silu + partial x_dbl ====
        with tc.tile_pool(name="p1", bufs=1) as p1, \
             tc.tile_pool(name="p1psum", bufs=1, space="PSUM") as p1psum:
            winx_sb = p1.tile([128, DM // 128, CH], bf16)
            nc.sync.dma_start(winx_sb[:], winxT.ap().rearrange("(k p) n -> p k n", p=128))
            winz_sb = p1.tile([128, DM // 128, CH], bf16)
            nc.sync.dma_start(winz_sb[:], winzT.ap().rearrange("(k p) n -> p k n", p=128))
            wx_sb = {}
            cw_sb = {}
            cb_sb = {}
            for d in ("f", "r"):
                wx_sb[d] = p1.tile([128, NCH, 128], bf16, name=f"wx_sb_{d}")
                nc.sync.dma_start(wx_sb[d][:], wx_d[d].ap().rearrange("(m p) n -> p m n", p=128))
                if CFG["conv"] == "E":
                    cw_sb[d] = p1.tile([128, NCH * DC, 128], bf16,
                                       name=f"cw_sb_{d}")
                    nc.sync.dma_start(
                        cw_sb[d][:], cwdiag_d[d].ap().rearrange("k p n -> p k n"))
                else:
                    cw_sb[d] = p1.tile([128, NCH, DC], f32, name=f"cw_sb_{d}")
                    nc.sync.dma_start(
                        cw_sb[d][:], cw_d[d].ap().rearrange("p (m j) -> p m j", m=NCH))
                cb_sb[d] = p1.tile([128, NCH], f32, name=f"cb_sb_{d}")
                nc.sync.dma_start(cb_sb[d][:], cb_d[d][:])

            hT_r = hT.ap().rearrange("(k p) l -> p k l", p=128)
            prev_xe = [None] * NCH

            def conv_dir(cc, d, xe_list):
                """Causal (d=f) / anti-causal (d=r) depthwise conv + silu on
                original-time chunk cc, using extended tiles [3|T1|3].
                Conv runs on DVE (tap0 as 4x tensor_scalar, taps 1-3 as
                STT accumulate).  Returns bf16 silu'd tiles."""
                out = []
                for m in range(NCH):
                    xe = xe_list[m]
                    if CFG["conv"] == "E":
                        acc = p1psum.tile([128, T1], f32, tag="cps", bufs=2,
                                          name=f"cps{m}_{d}_{cc}")
                        for j in range(DC):
                            off = j if d == "f" else (6 - j)
                            MM(acc[:], cw_sb[d][:, m * DC + j, :],
                               xe[:, off : off + T1],
                               start=(j == 0), stop=(j == DC - 1))
                    else:
                        acc = p1.tile([128, T1], f32, tag=f"cacc{m}", bufs=2,
                                      name=f"cacc{m}_{d}_{cc}")
                        for j in range(DC):
                            off = j if d == "f" else (6 - j)
                            src = xe[:, off : off + T1]
                            wj = cw_sb[d][:, m, j : j + 1]
                            if j == 0:
                                TSMUL(acc[:], src, wj)
                            else:
                                STT(acc[:], src, wj, acc[:], OP.mult, OP.add)
                    xcb = p1.tile([128, T1], bf16, tag=f"xcb{m}_{d}", bufs=2,
                                  name=f"xcb{m}_{d}_{cc}")
                    ACT(xcb[:], acc[:], AF.Silu, bias=cb_sb[d][:, m : m + 1])
                    out.append(xcb)
                return out

            def xdbl_chunk(cc, d, xc_tiles):
                # psum rows laid out as [B 0:16 | C 32:48 | dt 64:128]
                # (W_x rows reordered+padded on host); full 128 rows go to AR.
                ps = p1psum.tile([128, T1], f32, tag="psx", bufs=2,
                                 name=f"psx_{d}_{cc}")
                for m in range(NCH):
                    MM(ps[:], wx_sb[d][:, m, :], xc_tiles[m][:],
                       start=(m == 0), stop=(m == NCH - 1))
                stage = p1.tile([128, T1], f32, tag="arstage", bufs=2,
                                name=f"arstage_{d}_{cc}")
                if d == "f":
                    ACT(stage[:], ps[:], AF.Copy)
                    nc.sync.dma_start(
                        ar_in[0, :, cc * T1 : (cc + 1) * T1], stage[:]
                    )
                else:
                    if CFG["flip"] == "A":
                        ACT(stage[:], ps[:, ::-1], AF.Copy)
                    else:
                        nc.vector.tensor_copy(stage[:], ps[:, ::-1])
                    nc.sync.dma_start(
                        ar_in[1, :, L - (cc + 1) * T1 : L - cc * T1], stage[:]
                    )

            def spill_chunk(cc, d, xc_tiles):
                x_dram = xf_dram if d == "f" else xr_dram
                for m in range(NCH):
                    nc.sync.dma_start(
                        x_dram[m, :, cc * T1 : (cc + 1) * T1], xc_tiles[m][:]
                    )
                    if dbg:
                        key = "xc_f" if d == "f" else "xc_r"
                        nc.sync.dma_start(
                            dbg[key][m, :, cc * T1 : (cc + 1) * T1], xc_tiles[m][:]
                        )

            def finish_reverse(cc, xe_list):
                xcr = conv_dir(cc, "r", xe_list)
                xdbl_chunk(cc, "r", xcr)
                spill_chunk(cc, "r", xcr)

            for c in range(NC1):
                hTt = p1.tile([128, DM // 128, T1], bf16, tag="hTt", bufs=2,
                              name=f"hTt_{c}")
                nc.sync.dma_start(hTt[:], hT_r[:, :, c * T1 : (c + 1) * T1])

                # x part (extended with halos) and z part (-> silu -> spill)
                cur_xe = []
                for m in range(NCH):
                    ps = p1psum.tile([128, T1], f32, tag="ps_ip", bufs=CFG["p1bufs"],
                                     name=f"psx_{c}_{m}")
                    for ko in range(DM // 128):
                        MM(ps[:], winx_sb[:, ko, m * 128 : (m + 1) * 128],
                           hTt[:, ko, :], start=(ko == 0), stop=(ko == DM // 128 - 1))
                    xe = p1.tile([128, T1 + 6], bf16, tag=f"xe{m}", bufs=3,
                                 name=f"xe{m}_{c}")
                    ACT(xe[:, 3 : 3 + T1], ps[:], AF.Copy)
                    if c == 0:
                        nc.vector.memset(xe[:, 0:3], 0.0)
                    else:
                        nc.vector.tensor_copy(xe[:, 0:3], prev_xe[m][:, T1 : T1 + 3])
                    cur_xe.append(xe)
                for m in range(NCH):
                    ps = p1psum.tile([128, T1], f32, tag="ps_ip", bufs=CFG["p1bufs"],
                                     name=f"psz_{c}_{m}")
                    for ko in range(DM // 128):
                        MM(ps[:], winz_sb[:, ko, m * 128 : (m + 1) * 128],
                           hTt[:, ko, :], start=(ko == 0), stop=(ko == DM // 128 - 1))
                    zs = p1.tile([128, T1], bf16, tag=f"zs{m}", bufs=2,
                                 name=f"zs{m}_{c}")
                    ACT(zs[:], ps[:], AF.Silu)
                    nc.sync.dma_start(sz_dram[m, :, c * T1 : (c + 1) * T1], zs[:])
                    if dbg:
                        nc.sync.dma_start(
                            dbg["siluz"][m, :, c * T1 : (c + 1) * T1], zs[:]
                        )

                if c > 0:
                    # fill previous chunk's right halo, then do its reverse conv
                    for m in range(NCH):
                        nc.vector.tensor_copy(
                            prev_xe[m][:, T1 + 3 : T1 + 6], cur_xe[m][:, 3:6]
                        )
                    finish_reverse(c - 1, prev_xe)

                # forward conv on current chunk
                xcf = conv_dir(c, "f", cur_xe)
                xdbl_chunk(c, "f", xcf)
                spill_chunk(c, "f", xcf)

                prev_xe = cur_xe

            for m in range(NCH):
                nc.vector.memset(prev_xe[m][:, T1 + 3 : T1 + 6], 0.0)
            finish_reverse(NC1 - 1, prev_xe)

            # -------- AllReduce of x_dbl over the 4 cores of this batch ----
            # reverse direction first: pass 2r starts as soon as its rows
            # are reduced, overlapping the forward AR
            for di, d in ((1, "r"), (0, "f")):
                if collective:
                    nc.gpsimd.collective_compute(
                        "AllReduce", OP.add,
                        replica_groups=[[0, 1, 2, 3], [4, 5, 6, 7]],
                        ins=[ar_in[di].opt()], outs=[ar_out[di].opt()],
                    )
                else:
                    nc.gpsimd.dma_start(ar_out[di], ar_in[di])
                # cast-readback f32 -> bf16 into SBUF (gpsimd DMAs may cast)
                nc.gpsimd.dma_start(xdbl[d][:], ar_out[di, :, :])
            if dbg:
                for di, d in enumerate(("f", "r")):
                    nc.sync.dma_start(dbg[f"xdbl_{d}"][0:64, :], ar_out[di, 64:128, :])
                    nc.sync.dma_start(dbg[f"xdbl_{d}"][64:80, :], ar_out[di, 0:16, :])
                    nc.sync.dma_start(dbg[f"xdbl_{d}"][80:96, :], ar_out[di, 32:48, :])

        # ================= PASS 2: dt + selective scan (+gating, out_proj) ==
        def scan_pass(d, p2, p2psum, ytot_cb, mmt_bufs=3):
            """d: 'f' or 'r'.  'r' reads x/sz spills (original time) with
            reversed SBUF access; everything else runs in flipped time.
            ytot_cb(c2, yg_tiles): consumes gated y tiles for chunk c2."""
            x_dram = xf_dram if d == "f" else xr_dram
            rev = (lambda ap: ap) if d == "f" else (lambda ap: ap[:, ::-1])
            for c2 in range(NC2):
                sl = slice(c2 * T2, (c2 + 1) * T2)
                osl = sl if d == "f" else slice(L - (c2 + 1) * T2, L - c2 * T2)
                # ---- dt projection + softplus (f32 path) ----
                dt_sb = []
                for m in range(NCH):
                    psd = p2psum.tile([128, T2], f32, tag="mmt", bufs=mmt_bufs,
                                      name=f"psd_{d}_{c2}_{m}")
                    MM(psd[:], wdt_sb[d][DR:128, m * 128 : (m + 1) * 128],
                       xdbl[d][DR:128, sl], start=True, stop=True)
                    et = p2.tile([128, T2], f32, tag="et", bufs=2,
                                 name=f"et_{d}_{c2}_{m}")
                    ACT(et[:], psd[:], AF.Exp, bias=db_sb[d][:, m : m + 1])
                    dt = p2.tile([128, T2], bf16, tag=f"dt{m}", bufs=2,
                                 name=f"dt{m}_{d}_{c2}")
                    ACT(dt[:], et[:], AF.Ln, bias=ones[:])
                    dt_sb.append(dt)
                    if dbg and d == "f":
                        nc.sync.dma_start(dbg["dt_f"][m, :, sl], dt[:])
                # ---- x load (bf16) + wd = dt*x + silu(z) load ----
                xd = []
                wd = []
                szt = []
                for m in range(NCH):
                    xt = p2.tile([128, T2], bf16, tag=f"xd{m}", bufs=2,
                                 name=f"xd{m}_{d}_{c2}")
                    nc.sync.dma_start(xt[:], x_dram[m, :, osl])
                    xd.append(xt)
                    wt = p2.tile([128, T2], bf16, tag=f"wd{m}", bufs=2,
                                 name=f"wd{m}_{d}_{c2}")
                    veng(CFG["wd"]).tensor_tensor(
                        wt[:], dt_sb[m][:], rev(xt[:]), OP.mult)
                    wd.append(wt)
                    sz = p2.tile([128, T2], bf16, tag=f"sz{m}", bufs=2,
                                 name=f"sz{m}_{d}_{c2}")
                    nc.sync.dma_start(sz[:], sz_dram[m, :, osl])
                    szt.append(sz)
                # ---- selective scan over 16 states ----
                yps = [p2psum.tile([128, T2], f32, tag=f"yp{m}", bufs=1,
                                   name=f"yp{m}_{d}_{c2}") for m in range(NCH)]
                for s in range(DS):
                    Bbp = p2psum.tile([128, T2], f32, tag="mmt", bufs=mmt_bufs,
                                      name=f"Bbp_{d}_{c2}_{s}")
                    MM(Bbp[:], sel_sb[0:DS, s * 128 : (s + 1) * 128],
                       xdbl[d][0:DS, sl], start=True, stop=True)
                    Bb = p2.tile([128, T2], bf16, tag="Bbs", bufs=CFG["hotbufs"],
                                 name=f"Bb_{d}_{c2}_{s}")
                    bcopy(CFG["bcopy"][s], Bb[:], Bbp[:])
                    Cbp = p2psum.tile([128, T2], f32, tag="mmt", bufs=mmt_bufs,
                                      name=f"Cbp_{d}_{c2}_{s}")
                    MM(Cbp[:], sel_sb[32 : 32 + DS, s * 128 : (s + 1) * 128],
                       xdbl[d][32 : 32 + DS, sl], start=True, stop=True)
                    Cb = p2.tile([128, T2], bf16, tag="Cbs", bufs=CFG["hotbufs"],
                                 name=f"Cb_{d}_{c2}_{s}")
                    bcopy(CFG["ccopy"][s], Cb[:], Cbp[:])
                    bt = []
                    for m in range(NCH):
                        b = p2.tile([128, T2], bf16, tag=f"bt{m}", bufs=CFG["hotbufs"],
                                    name=f"bt_{d}_{c2}_{s}_{m}")
                        veng(CFG["bt"][s * NCH + m]).tensor_tensor(
                            b[:], wd[m][:], Bb[:], OP.mult)
                        bt.append(b)
                    dAs = []
                    for m in range(NCH):
                        dA = p2.tile([128, T2], f32, tag=f"dA{m}", bufs=CFG["hotbufs"],
                                     name=f"dA_{d}_{c2}_{s}_{m}")
                        ACT(dA[:], dt_sb[m][:], AF.Exp,
                            scale=A_sb[d][:, m, s : s + 1])
                        dAs.append(dA)
                    # per-state hs tile holding all 4 channel groups, so the
                    # chunk-boundary carry is ONE strided copy per state
                    hs = p2.tile([128, NCH, T2], bf16, tag="hs", bufs=2,
                                 name=f"hs_{d}_{c2}_{s}")
                    for m in range(NCH):
                        nc.vector.tensor_tensor_scan(
                            hs[:, m, :], dAs[m][:], bt[m][:],
                            carry[d][:, m, s : s + 1], OP.mult, OP.add)
                    veng(CFG["carry"]).tensor_copy(
                        carry[d][:, :, s : s + 1], hs[:, :, T2 - 1 : T2])
                    for m in range(NCH):
                        cm = p2.tile([128, T2], bf16, tag=f"cm{m}", bufs=CFG["hotbufs"],
                                     name=f"cm_{d}_{c2}_{s}_{m}")
                        veng(CFG["cm"][s * NCH + m]).tensor_tensor(
                            cm[:], hs[:, m, :], Cb[:], OP.mult)
                        MM(yps[m][:], ident_sb[:], cm[:],
                           start=(s == 0), stop=(s == DS - 1))
                # ---- gating: y = (ypsum + x*D) * silu(z) ----
                yg = []
                for m in range(NCH):
                    y1 = p2.tile([128, T2], bf16, tag=f"y1{m}", bufs=2,
                                 name=f"y1_{d}_{c2}_{m}")
                    veng(CFG["skip"]).scalar_tensor_tensor(
                        y1[:], rev(xd[m][:]), D_sb[d][:, m : m + 1], yps[m][:],
                        OP.mult, OP.add)
                    yt = p2.tile([128, T2], bf16, tag=f"yg{m}", bufs=2,
                                 name=f"yg_{d}_{c2}_{m}")
                    veng(CFG["gate"]).tensor_tensor(
                        yt[:], y1[:], rev(szt[m][:]), OP.mult)
                    yg.append(yt)
                ytot_cb(c2, yg)

        # ---- pass 2: reverse (spill gated y), then forward (combine +
        # out_proj); one shared pool so the passes overlap at the seam ----
        with tc.tile_pool(name="p2", bufs=1) as p2f, \
             tc.tile_pool(name="p2psum", bufs=1, space="PSUM") as p2fpsum:

            def spill_ygr(c2, yg):
                for m in range(NCH):
                    nc.sync.dma_start(
                        ygr_dram[m, :, c2 * T2 : (c2 + 1) * T2], yg[m][:]
                    )

            scan_pass("r", p2f, p2fpsum, spill_ygr, mmt_bufs=2)

            def combine_out(c2, yg):
                ytot = []
                for m in range(NCH):
                    ygr_t = p2f.tile([128, T2], bf16, tag=f"ygr{m}", bufs=2,
                                     name=f"ygr{m}_{c2}")
                    nc.sync.dma_start(
                        ygr_t[:], ygr_dram[m, :, L - (c2 + 1) * T2 : L - c2 * T2]
                    )
                    yt2 = p2f.tile([128, T2], bf16, tag=f"ytot{m}", bufs=2,
                                   name=f"ytot{m}_{c2}")
                    veng(CFG["comb"]).tensor_tensor(
                        yt2[:], yg[m][:], ygr_t[:, ::-1], OP.add)
                    ytot.append(yt2)
                    if dbg:
                        nc.sync.dma_start(
                            dbg["y_f"][m, :, c2 * T2 : (c2 + 1) * T2], yg[m][:]
                        )
                for mt in range(T2 // 128):
                    ob = p2f.tile([128, DM], bf16, tag="ob", bufs=2,
                                  name=f"ob_{c2}_{mt}")
                    for nh in range(DM // 512):
                        po = p2fpsum.tile([128, 512], f32, tag="po", bufs=2,
                                          name=f"po_{c2}_{mt}_{nh}")
                        for k in range(NCH):
                            MM(po[:], ytot[k][:, mt * 128 : (mt + 1) * 128],
                               wout_sb[:, k, nh * 512 : (nh + 1) * 512],
                               start=(k == 0), stop=(k == NCH - 1))
                        bcopy(CFG["obcopy"], ob[:, nh * 512 : (nh + 1) * 512],
                              po[:])
                    nc.sync.dma_start(
                        pout[c2 * T2 + mt * 128 : c2 * T2 + (mt + 1) * 128, :],
                        ob[:],
                    )

            scan_pass("f", p2f, p2fpsum, combine_out, mmt_bufs=2)


def _host_prep(inputs):
    """Slice/transpose the full inputs into the 8 per-core input maps."""
    import ml_dtypes
    bf = ml_dtypes.bfloat16

    h = np.asarray(inputs["hidden_states"], np.float32)
    W_in = np.asarray(inputs["W_in"], np.float32)
    W_out = np.asarray(inputs["W_out"], np.float32)

    sel = np.zeros((48, DS * 128), np.float32)
    for s in range(DS):
        sel[s, s * 128 : (s + 1) * 128] = 1.0
        sel[32 + s, s * 128 : (s + 1) * 128] = 1.0

    maps = []
    for core in range(8):
        b, g = divmod(core, 4)
        c0 = g * CH
        m = {
            "hT": np.ascontiguousarray(h[b].T).astype(bf),
            "winxT": np.ascontiguousarray(W_in[c0 : c0 + CH, :].T).astype(bf),
            "winzT": np.ascontiguousarray(W_in[DI + c0 : DI + c0 + CH, :].T).astype(bf),
            "woutT": np.ascontiguousarray(W_out[:, c0 : c0 + CH].T).astype(bf),
            "sel": sel.astype(bf),
            "ident": np.eye(128, dtype=np.float32).astype(bf),
        }
        for d in ("f", "r"):
            sfx = f"_{d}"
            W_x = np.asarray(inputs[f"W_x{sfx}"], np.float32)
            W_dt = np.asarray(inputs[f"W_dt{sfx}"], np.float32)
            A = -np.exp(np.asarray(inputs[f"A_log{sfx}"], np.float64)).astype(np.float32)
            cw = np.asarray(inputs[f"conv_w{sfx}"], np.float32)
            cb = np.asarray(inputs[f"conv_b{sfx}"], np.float32)
            db = np.asarray(inputs[f"b_dt{sfx}"], np.float32)
            Dp = np.asarray(inputs[f"D{sfx}"], np.float32)
            wx_re = np.zeros((CH, 128), np.float32)
            wx_re[:, 0:DS] = W_x[DR : DR + DS, c0 : c0 + CH].T        # B rows
            wx_re[:, 32 : 32 + DS] = W_x[DR + DS : 96, c0 : c0 + CH].T  # C rows
            wx_re[:, DR:128] = W_x[0:DR, c0 : c0 + CH].T              # dt-rank rows
            m[f"wx{sfx}"] = wx_re.astype(bf)
            m[f"wdt{sfx}"] = np.ascontiguousarray(W_dt[c0 : c0 + CH, :].T).astype(bf)
            # (CH, DS) -> (128, NCH, DS) -> (128, NCH*DS)
            m[f"A{sfx}"] = np.ascontiguousarray(
                A[c0 : c0 + CH].reshape(NCH, 128, DS).transpose(1, 0, 2).reshape(128, NCH * DS)
            )
            m[f"cw{sfx}"] = np.ascontiguousarray(
                cw[c0 : c0 + CH].reshape(NCH, 128, DC).transpose(1, 0, 2).reshape(128, NCH * DC)
            )
            cwd = np.zeros((NCH * DC, 128, 128), np.float32)
            cwc = cw[c0 : c0 + CH].reshape(NCH, 128, DC)
            for mm_ in range(NCH):
                for j in range(DC):
                    np.fill_diagonal(cwd[mm_ * DC + j], cwc[mm_, :, j])
            m[f"cwdiag{sfx}"] = cwd.astype(bf)
            m[f"cb{sfx}"] = np.ascontiguousarray(
                cb[c0 : c0 + CH].reshape(NCH, 128).T
            )
            m[f"db{sfx}"] = np.ascontiguousarray(
                db[c0 : c0 + CH].reshape(NCH, 128).T
            )
            m[f"D{sfx}"] = np.ascontiguousarray(
                Dp[c0 : c0 + CH].reshape(NCH, 128).T
            )
        maps.append(m)
    return maps


def run(inputs, debug=False, trace=False):
    from concourse.bass_utils import run_bass_kernel_spmd

    if _COMPILED[0] is None or _COMPILED[0][1] != debug:
        _COMPILED[0] = (_build_program(debug=debug), debug)
    nc = _COMPILED[0][0]
    maps = _host_prep(inputs)
    res = run_bass_kernel_spmd(nc, maps, core_ids=list(range(8)), trace=trace)
    outs = [np.asarray(r["pout"], np.float32) for r in res.results]
    full = np.zeros((B, L, DM), np.float32)
    for core in range(8):
        b = core // 4
        full[b] += outs[core]
    return full, res


def kernel(**inputs):
    out, _ = run(inputs, debug=False, trace=False)
    return out


# revision 22
# speedup vs baseline: 1.1524x; 1.0000x over previous
"""BiMamba (bidirectional Mamba block) Trainium2 kernel.

Contract: kernel(**inputs) takes the full (unsharded) numpy inputs of the
reference and returns the full (2, 4096, 1024) float32 output.

Sharding: 8 cores = 2 batches x 4 channel-groups of 512 d_inner channels.
Each core runs both scan directions for its channel slice; the x_dbl
reduction over d_inner is an on-chip AllReduce within each batch's 4-core
group; the host sums the four partial out-projections per batch.

Key algebraic facts used:
  * xz for the reverse direction is the L-flip of the forward xz, so the
    input projection is computed once.
  * (y_f + flip(y_r)) @ W_out.T == out_f + flip(out_r), so one output
    projection suffices.

Performance structure (engine balance per scan chunk):
  * Pool (gpsimd) runs the 64 tensor_tensor_scan ops (the serial core).
  * DVE runs the bf16 TensorTensor mults (2x_1p packed mode).
  * Act runs the exp/softplus and most PSUM->SBUF broadcast copies.
  * PE accumulates y over the 16 states via identity matmuls into PSUM,
    plus the projections.
  * All DMA uses contiguous descriptors (reversals happen in SBUF reads).
"""

import os
import sys

import numpy as np

sys.path.insert(0, "/opt/trn_rl_repo")

B, L, DM, DI, DS, DR, DC = 2, 4096, 1024, 2048, 16, 64, 4
CH = 512          # d_inner channels per core
NCH = CH // 128   # channel tiles per core
T1 = 512          # pass-1 (projection/conv) token chunk
NC1 = L // T1
T2 = 512          # pass-2 (scan) token chunk
NC2 = L // T2

# engine assignment tuning: V=DVE, P=Pool(gpsimd), A=Act
# (scans must run on DVE: walrus cannot lower tensor_tensor_scan on Pool)
CFG = dict(
    bcopy=os.environ.get("CFG_BCOPY", "A" * 16),     # per s: B broadcast copy
    ccopy=os.environ.get("CFG_CCOPY", "A" * 16),     # per s: C broadcast copy
    carry=os.environ.get("CFG_CARRY", "P"),          # batched carry copies
    bt=os.environ.get("CFG_BT", ""),                 # per (s*NCH+m): bt engine
    cm=os.environ.get("CFG_CM", ""),                 # per (s*NCH+m): cmul engine
    hotbufs=int(os.environ.get("CFG_HOTBUFS", "3")),  # bufs for s-loop tags
    wd=os.environ.get("CFG_WD", "V"),
    skip=os.environ.get("CFG_SKIP", "V"),
    gate=os.environ.get("CFG_GATE", "V"),
    comb=os.environ.get("CFG_COMB", "V"),
    conv=os.environ.get("CFG_CONV", "V"),            # V=DVE STT, E=PE diag-mm
    obcopy=os.environ.get("CFG_OBCOPY", "A"),        # out_proj PSUM->SBUF copy
    flip=os.environ.get("CFG_FLIP", "A"),            # AR reverse stage copy
    p1bufs=int(os.environ.get("CFG_P1BUFS", "2")),   # pass-1 in_proj psum bufs
)
def _bres(k, n=64):
    out = []
    acc = 0
    for _ in range(n):
        acc += k
        if acc >= n:
            acc -= n
            out.append("V")
        else:
            out.append("P")
    return "".join(out)


if not CFG["bt"]:
    CFG["bt"] = _bres(39)
if not CFG["cm"]:
    CFG["cm"] = _bres(39)

_COMPILED = [None]


def _split_sync_waits(nc, mybir, max_waits=1):
    """walrus in this environment rejects >1 sync wait per instruction;
    hoist excess waits onto dedicated same-engine NOPs."""
    uid = [0]
    for f in nc.m.functions:
        for bb in f.blocks:
            new = []
            dirty = False
            for inst in bb.instructions:
                si = inst.sync_info
                if si is not None and len(si.on_wait) > max_waits:
                    waits = list(si.on_wait)
                    keep = waits[len(waits) - max_waits:]
                    hoist = waits[: len(waits) - max_waits]
                    for i in range(0, len(hoist), max_waits):
                        uid[0] += 1
                        nop = mybir.InstNoOp(
                            name=f"splitwait-{id(nc)}-{uid[0]}", engine=inst.engine
                        )
                        nop.sync_info = mybir.SyncInfo(
                            on_wait=hoist[i : i + max_waits], on_update=[]
                        )
                        nc.register_instruction(nop, overwrite=True)
                        new.append(nop)
                    inst.sync_info = mybir.SyncInfo(
                        on_wait=keep, on_update=list(si.on_update)
                    )
                    dirty = True
                new.append(inst)
            if dirty:
                bb.instructions = new


def _build_program(debug=False, collective=True):
    import concourse.bass as bass
    import concourse.tile as tile
    from concourse import mybir

    f32 = mybir.dt.float32
    f32r = mybir.dt.float32r
    bf16 = mybir.dt.bfloat16
    AF = mybir.ActivationFunctionType
    OP = mybir.AluOpType

    nc = bass.Bass("TRN2", target_bir_lowering=False, debug=False, num_devices=8)

    # ---- external inputs (per-core shards prepared on host) ----
    hT = nc.dram_tensor("hT", [DM, L], bf16, kind="ExternalInput")
    winxT = nc.dram_tensor("winxT", [DM, CH], bf16, kind="ExternalInput")
    winzT = nc.dram_tensor("winzT", [DM, CH], bf16, kind="ExternalInput")
    woutT_d = nc.dram_tensor("woutT", [CH, DM], bf16, kind="ExternalInput")
    sel_d = nc.dram_tensor("sel", [48, DS * 128], bf16, kind="ExternalInput")
    ident_d = nc.dram_tensor("ident", [128, 128], bf16, kind="ExternalInput")
    wx_d = {}
    wdt_d = {}
    A_d = {}
    cw_d = {}
    cwdiag_d = {}
    cb_d = {}
    db_d = {}
    D_d = {}
    for d in ("f", "r"):
        wx_d[d] = nc.dram_tensor(f"wx_{d}", [CH, 128], bf16, kind="ExternalInput")
        wdt_d[d] = nc.dram_tensor(f"wdt_{d}", [DR, CH], bf16, kind="ExternalInput")
        A_d[d] = nc.dram_tensor(f"A_{d}", [128, NCH * DS], f32, kind="ExternalInput")
        cw_d[d] = nc.dram_tensor(f"cw_{d}", [128, NCH * DC], f32,
                                 kind="ExternalInput")
        cwdiag_d[d] = nc.dram_tensor(f"cwdiag_{d}", [NCH * DC, 128, 128], bf16,
                                     kind="ExternalInput")
        cb_d[d] = nc.dram_tensor(f"cb_{d}", [128, NCH], f32, kind="ExternalInput")
        db_d[d] = nc.dram_tensor(f"db_{d}", [128, NCH], f32, kind="ExternalInput")
        D_d[d] = nc.dram_tensor(f"D_{d}", [128, NCH], f32, kind="ExternalInput")

    pout = nc.dram_tensor("pout", [L, DM], bf16, kind="ExternalOutput")
    dbg = {}
    if debug:
        dbg["xc_f"] = nc.dram_tensor("dbg_xc_f", [NCH, 128, L], bf16, kind="ExternalOutput")
        dbg["xc_r"] = nc.dram_tensor("dbg_xc_r", [NCH, 128, L], bf16, kind="ExternalOutput")
        dbg["xdbl_f"] = nc.dram_tensor("dbg_xdbl_f", [96, L], f32, kind="ExternalOutput")
        dbg["xdbl_r"] = nc.dram_tensor("dbg_xdbl_r", [96, L], f32, kind="ExternalOutput")
        dbg["dt_f"] = nc.dram_tensor("dbg_dt_f", [NCH, 128, L], f32, kind="ExternalOutput")
        dbg["y_f"] = nc.dram_tensor("dbg_y_f", [NCH, 128, L], bf16, kind="ExternalOutput")
        dbg["siluz"] = nc.dram_tensor("dbg_siluz", [NCH, 128, L], bf16, kind="ExternalOutput")

    with tile.TileContext(nc, num_cores=8) as tc:
        _build_tile_program(
            nc, tc, tile, mybir, f32, f32r, bf16, AF, OP,
            hT, winxT, winzT, woutT_d, sel_d, ident_d, wx_d, wdt_d, A_d, cw_d,
            cwdiag_d, cb_d, db_d, D_d, pout, dbg, collective,
        )

    _split_sync_waits(nc, mybir)
    return nc


def _build_tile_program(
    nc, tc, tile, mybir, f32, f32r, bf16, AF, OP,
    hT, winxT, winzT, woutT_d, sel_d, ident_d, wx_d, wdt_d, A_d, cw_d,
    cwdiag_d, cb_d, db_d, D_d, pout, dbg, collective=True,
):
    from contextlib import ExitStack

    MM = nc.tensor.matmul
    ACT = nc.scalar.activation
    TT = nc.vector.tensor_tensor
    STT = nc.vector.scalar_tensor_tensor
    TSMUL = nc.vector.tensor_scalar_mul

    def veng(code):
        return nc.vector if code == "V" else nc.gpsimd

    def bcopy(code, out, in_):
        """PSUM f32 -> SBUF copy on the chosen engine."""
        if code == "A":
            ACT(out, in_, AF.Copy)
        else:
            veng(code).tensor_copy(out, in_)

    ctx = ExitStack()
    with ctx:
        # -------- persistent pools --------
        pers = ctx.enter_context(tc.tile_pool(name="pers", bufs=1))
        dram = ctx.enter_context(tc.tile_pool(name="dram", bufs=1, space="DRAM"))

        wout_sb = pers.tile([128, NCH, DM], bf16)
        nc.sync.dma_start(wout_sb[:], woutT_d.ap().rearrange("(k p) n -> p k n", p=128))
        sel_sb = pers.tile([48, DS * 128], bf16)
        nc.sync.dma_start(sel_sb[:], sel_d[:])
        ident_sb = pers.tile([128, 128], bf16)
        nc.sync.dma_start(ident_sb[:], ident_d[:])
        xdbl = {}      # bf16 [128, L]: rows [0:16]=B, [32:48]=C, [64:128]=dt-rank
        carry = {}
        wdt_sb = {}
        A_sb = {}
        db_sb = {}
        D_sb = {}
        for d in ("f", "r"):
            xdbl[d] = pers.tile([128, L], bf16, name=f"xdbl_{d}")
            carry[d] = pers.tile([128, NCH, DS], bf16, name=f"carry_{d}")
            nc.vector.memset(carry[d][:], 0.0)
            wdt_sb[d] = pers.tile([128, CH], bf16, name=f"wdt_sb_{d}")
            nc.sync.dma_start(wdt_sb[d][DR:128, :], wdt_d[d][:])
            A_sb[d] = pers.tile([128, NCH, DS], f32, name=f"A_sb_{d}")
            nc.sync.dma_start(A_sb[d][:], A_d[d].ap().rearrange("p (m s) -> p m s", m=NCH))
            db_sb[d] = pers.tile([128, NCH], f32, name=f"db_sb_{d}")
            nc.sync.dma_start(db_sb[d][:], db_d[d][:])
            D_sb[d] = pers.tile([128, NCH], f32, name=f"D_sb_{d}")
            nc.sync.dma_start(D_sb[d][:], D_d[d][:])
        ones = pers.tile([128, 1], f32)
        nc.vector.memset(ones[:], 1.0)

        # DRAM spill buffers (per-core local HBM); all in ORIGINAL time order
        # for the forward direction; xr/sz are original-time too (pass 2r
        # flips with reversed SBUF reads).  ygr is in flipped time.
        xf_dram = dram.tile([NCH, 128, L], bf16)
        xr_dram = dram.tile([NCH, 128, L], bf16)
        sz_dram = dram.tile([NCH, 128, L], bf16)
        ygr_dram = dram.tile([NCH, 128, L], bf16)
        # AllReduce staging: [dir, 128 rows, L] f32; rows as xdbl layout.
        # dir 0 = forward (original time), dir 1 = reverse (flipped time).
        ar_in = dram.tile([2, 128, L], f32)
        ar_out = dram.tile([2, 128, L], f32)

        # ================= PASS 1: in_proj + conv + silu + partial x_dbl ====
        with tc.tile_pool(name="p1", bufs=1) as p1, \
             tc.tile_pool(name="p1psum", bufs=1, space="PSUM") as p1psum:
            winx_sb = p1.tile([128, DM // 128, CH], bf16)
            nc.sync.dma_start(winx_sb[:], winxT.ap().rearrange("(k p) n -> p k n", p=128))
            winz_sb = p1.tile([128, DM // 128, CH], bf16)
            nc.sync.dma_start(winz_sb[:], winzT.ap().rearrange("(k p) n -> p k n", p=128))
            wx_sb = {}
            cw_sb = {}
            cb_sb = {}
            for d in ("f", "r"):
                wx_sb[d] = p1.tile([128, NCH, 128], bf16, name=f"wx_sb_{d}")
                nc.sync.dma_start(wx_sb[d][:], wx_d[d].ap().rearrange("(m p) n -> p m n", p=128))
                if CFG["conv"] == "E":
                    cw_sb[d] = p1.tile([128, NCH * DC, 128], bf16,
                                       name=f"cw_sb_{d}")
                    nc.sync.dma_start(
                        cw_sb[d][:], cwdiag_d[d].ap().rearrange("k p n -> p k n"))
                else:
                    cw_sb[d] = p1.tile([128, NCH, DC], f32, name=f"cw_sb_{d}")
                    nc.sync.dma_start(
                        cw_sb[d][:], cw_d[d].ap().rearrange("p (m j) -> p m j", m=NCH))
                cb_sb[d] = p1.tile([128, NCH], f32, name=f"cb_sb_{d}")
                nc.sync.dma_start(cb_sb[d][:], cb_d[d][:])

            hT_r = hT.ap().rearrange("(k p) l -> p k l", p=128)
            prev_xe = [None] * NCH

            def conv_dir(cc, d, xe_list):
                """Causal (d=f) / anti-causal (d=r) depthwise conv + silu on
                original-time chunk cc, using extended tiles [3|T1|3].
                Conv runs on DVE (tap0 as 4x tensor_scalar, taps 1-3 as
                STT accumulate).  Returns bf16 silu'd tiles."""
                out = []
                for m in range(NCH):
                    xe = xe_list[m]
                    if CFG["conv"] == "E":
                        acc = p1psum.tile([128, T1], f32, tag="cps", bufs=2,
                                          name=f"cps{m}_{d}_{cc}")
                        for j in range(DC):
                            off = j if d == "f" else (6 - j)
                            MM(acc[:], cw_sb[d][:, m * DC + j, :],
                               xe[:, off : off + T1],
                               start=(j == 0), stop=(j == DC - 1))
                    else:
                        acc = p1.tile([128, T1], f32, tag=f"cacc{m}", bufs=2,
                                      name=f"cacc{m}_{d}_{cc}")
                        for j in range(DC):
                            off = j if d == "f" else (6 - j)
                            src = xe[:, off : off + T1]
                            wj = cw_sb[d][:, m, j : j + 1]
                            if j == 0:
                                TSMUL(acc[:], src, wj)
                            else:
                                STT(acc[:], src, wj, acc[:], OP.mult, OP.add)
                    xcb = p1.tile([128, T1], bf16, tag=f"xcb{m}_{d}", bufs=2,
                                  name=f"xcb{m}_{d}_{cc}")
                    ACT(xcb[:], acc[:], AF.Silu, bias=cb_sb[d][:, m : m + 1])
                    out.append(xcb)
                return out

            def xdbl_chunk(cc, d, xc_tiles):
                # psum rows laid out as [B 0:16 | C 32:48 | dt 64:128]
                # (W_x rows reordered+padded on host); full 128 rows go to AR.
                ps = p1psum.tile([128, T1], f32, tag="psx", bufs=2,
                                 name=f"psx_{d}_{cc}")
                for m in range(NCH):
                    MM(ps[:], wx_sb[d][:, m, :], xc_tiles[m][:],
                       start=(m == 0), stop=(m == NCH - 1))
                stage = p1.tile([128, T1], f32, tag="arstage", bufs=2,
                                name=f"arstage_{d}_{cc}")
                if d == "f":
                    ACT(stage[:], ps[:], AF.Copy)
                    nc.sync.dma_start(
                        ar_in[0, :, cc * T1 : (cc + 1) * T1], stage[:]
                    )
                else:
                    if CFG["flip"] == "A":
                        ACT(stage[:], ps[:, ::-1], AF.Copy)
                    else:
                        nc.vector.tensor_copy(stage[:], ps[:, ::-1])
                    nc.sync.dma_start(
                        ar_in[1, :, L - (cc + 1) * T1 : L - cc * T1], stage[:]
                    )

            def spill_chunk(cc, d, xc_tiles):
                x_dram = xf_dram if d == "f" else xr_dram
                for m in range(NCH):
                    nc.sync.dma_start(
                        x_dram[m, :, cc * T1 : (cc + 1) * T1], xc_tiles[m][:]
                    )
                    if dbg:
                        key = "xc_f" if d == "f" else "xc_r"
                        nc.sync.dma_start(
                            dbg[key][m, :, cc * T1 : (cc + 1) * T1], xc_tiles[m][:]
                        )

            def finish_reverse(cc, xe_list):
                xcr = conv_dir(cc, "r", xe_list)
                xdbl_chunk(cc, "r", xcr)
                spill_chunk(cc, "r", xcr)

            for c in range(NC1):
                hTt = p1.tile([128, DM // 128, T1], bf16, tag="hTt", bufs=2,
                              name=f"hTt_{c}")
                nc.sync.dma_start(hTt[:], hT_r[:, :, c * T1 : (c + 1) * T1])

                # x part (extended with halos) and z part (-> silu -> spill)
                cur_xe = []
                for m in range(NCH):
                    ps = p1psum.tile([128, T1], f32, tag="ps_ip", bufs=CFG["p1bufs"],
                                     name=f"psx_{c}_{m}")
                    for ko in range(DM // 128):
                        MM(ps[:], winx_sb[:, ko, m * 128 : (m + 1) * 128],
                           hTt[:, ko, :], start=(ko == 0), stop=(ko == DM // 128 - 1))
                    xe = p1.tile([128, T1 + 6], bf16, tag=f"xe{m}", bufs=3,
                                 name=f"xe{m}_{c}")
                    ACT(xe[:, 3 : 3 + T1], ps[:], AF.Copy)
                    if c == 0:
                        nc.vector.memset(xe[:, 0:3], 0.0)
                    else:
                        nc.vector.tensor_copy(xe[:, 0:3], prev_xe[m][:, T1 : T1 + 3])
                    cur_xe.append(xe)
                if c > 0:
                    # fill previous chunk's right halo, then do its reverse conv
                    for m in range(NCH):
                        nc.vector.tensor_copy(
                            prev_xe[m][:, T1 + 3 : T1 + 6], cur_xe[m][:, 3:6]
                        )
                    finish_reverse(c - 1, prev_xe)

                # forward conv on current chunk
                xcf = conv_dir(c, "f", cur_xe)
                xdbl_chunk(c, "f", xcf)
                spill_chunk(c, "f", xcf)

                # z projection last: nothing in pass 1 depends on it
                for m in range(NCH):
                    ps = p1psum.tile([128, T1], f32, tag="ps_ip", bufs=CFG["p1bufs"],
                                     name=f"psz_{c}_{m}")
                    for ko in range(DM // 128):
                        MM(ps[:], winz_sb[:, ko, m * 128 : (m + 1) * 128],
                           hTt[:, ko, :], start=(ko == 0), stop=(ko == DM // 128 - 1))
                    zs = p1.tile([128, T1], bf16, tag=f"zs{m}", bufs=2,
                                 name=f"zs{m}_{c}")
                    ACT(zs[:], ps[:], AF.Silu)
                    nc.sync.dma_start(sz_dram[m, :, c * T1 : (c + 1) * T1], zs[:])
                    if dbg:
                        nc.sync.dma_start(
                            dbg["siluz"][m, :, c * T1 : (c + 1) * T1], zs[:]
                        )

                prev_xe = cur_xe

            for m in range(NCH):
                nc.vector.memset(prev_xe[m][:, T1 + 3 : T1 + 6], 0.0)
            finish_reverse(NC1 - 1, prev_xe)

            # -------- AllReduce of x_dbl over the 4 cores of this batch ----
            # reverse direction first: pass 2r starts as soon as its rows
            # are reduced, overlapping the forward AR
            for di, d in ((1, "r"), (0, "f")):
                if collective:
                    nc.gpsimd.collective_compute(
                        "AllReduce", OP.add,
                        replica_groups=[[0, 1, 2, 3], [4, 5, 6, 7]],
                        ins=[ar_in[di].opt()], outs=[ar_out[di].opt()],
                    )
                else:
                    nc.gpsimd.dma_start(ar_out[di], ar_in[di])
                # cast-readback f32 -> bf16 into SBUF (gpsimd DMAs may cast)
                nc.gpsimd.dma_start(xdbl[d][:], ar_out[di, :, :])
            if dbg:
                for di, d in enumerate(("f", "r")):
                    nc.sync.dma_start(dbg[f"xdbl_{d}"][0:64, :], ar_out[di, 64:128, :])
                    nc.sync.dma_start(dbg[f"xdbl_{d}"][64:80, :], ar_out[di, 0:16, :])
                    nc.sync.dma_start(dbg[f"xdbl_{d}"][80:96, :], ar_out[di, 32:48, :])

        # ================= PASS 2: dt + selective scan (+gating, out_proj) ==
        def scan_pass(d, p2, p2psum, ytot_cb, mmt_bufs=3):
            """d: 'f' or 'r'.  'r' reads x/sz spills (original time) with
            reversed SBUF access; everything else runs in flipped time.
            ytot_cb(c2, yg_tiles): consumes gated y tiles for chunk c2."""
            x_dram = xf_dram if d == "f" else xr_dram
            rev = (lambda ap: ap) if d == "f" else (lambda ap: ap[:, ::-1])
            for c2 in range(NC2):
                sl = slice(c2 * T2, (c2 + 1) * T2)
                osl = sl if d == "f" else slice(L - (c2 + 1) * T2, L - c2 * T2)
                # ---- dt projection + softplus (f32 path) ----
                dt_sb = []
                for m in range(NCH):
                    psd = p2psum.tile([128, T2], f32, tag="mmt", bufs=mmt_bufs,
                                      name=f"psd_{d}_{c2}_{m}")
                    MM(psd[:], wdt_sb[d][DR:128, m * 128 : (m + 1) * 128],
                       xdbl[d][DR:128, sl], start=True, stop=True)
                    et = p2.tile([128, T2], f32, tag="et", bufs=2,
                                 name=f"et_{d}_{c2}_{m}")
                    ACT(et[:], psd[:], AF.Exp, bias=db_sb[d][:, m : m + 1])
                    dt = p2.tile([128, T2], bf16, tag=f"dt{m}", bufs=2,
                                 name=f"dt{m}_{d}_{c2}")
                    ACT(dt[:], et[:], AF.Ln, bias=ones[:])
                    dt_sb.append(dt)
                    if dbg and d == "f":
                        nc.sync.dma_start(dbg["dt_f"][m, :, sl], dt[:])
                # ---- x load (bf16) + wd = dt*x + silu(z) load ----
                xd = []
                wd = []
                szt = []
                for m in range(NCH):
                    xt = p2.tile([128, T2], bf16, tag=f"xd{m}", bufs=2,
                                 name=f"xd{m}_{d}_{c2}")
                    nc.sync.dma_start(xt[:], x_dram[m, :, osl])
                    xd.append(xt)
                    wt = p2.tile([128, T2], bf16, tag=f"wd{m}", bufs=2,
                                 name=f"wd{m}_{d}_{c2}")
                    veng(CFG["wd"]).tensor_tensor(
                        wt[:], dt_sb[m][:], rev(xt[:]), OP.mult)
                    wd.append(wt)
                    sz = p2.tile([128, T2], bf16, tag=f"sz{m}", bufs=2,
                                 name=f"sz{m}_{d}_{c2}")
                    nc.sync.dma_start(sz[:], sz_dram[m, :, osl])
                    szt.append(sz)
                # ---- selective scan over 16 states ----
                yps = [p2psum.tile([128, T2], f32, tag=f"yp{m}", bufs=1,
                                   name=f"yp{m}_{d}_{c2}") for m in range(NCH)]
                for s in range(DS):
                    Bbp = p2psum.tile([128, T2], f32, tag="mmt", bufs=mmt_bufs,
                                      name=f"Bbp_{d}_{c2}_{s}")
                    MM(Bbp[:], sel_sb[0:DS, s * 128 : (s + 1) * 128],
                       xdbl[d][0:DS, sl], start=True, stop=True)
                    Bb = p2.tile([128, T2], bf16, tag="Bbs", bufs=CFG["hotbufs"],
                                 name=f"Bb_{d}_{c2}_{s}")
                    bcopy(CFG["bcopy"][s], Bb[:], Bbp[:])
                    Cbp = p2psum.tile([128, T2], f32, tag="mmt", bufs=mmt_bufs,
                                      name=f"Cbp_{d}_{c2}_{s}")
                    MM(Cbp[:], sel_sb[32 : 32 + DS, s * 128 : (s + 1) * 128],
                       xdbl[d][32 : 32 + DS, sl], start=True, stop=True)
                    Cb = p2.tile([128, T2], bf16, tag="Cbs", bufs=CFG["hotbufs"],
                                 name=f"Cb_{d}_{c2}_{s}")
                    bcopy(CFG["ccopy"][s], Cb[:], Cbp[:])
                    bt = []
                    for m in range(NCH):
                        b = p2.tile([128, T2], bf16, tag=f"bt{m}", bufs=CFG["hotbufs"],
                                    name=f"bt_{d}_{c2}_{s}_{m}")
                        veng(CFG["bt"][s * NCH + m]).tensor_tensor(
                            b[:], wd[m][:], Bb[:], OP.mult)
                        bt.append(b)
                    dAs = []
                    for m in range(NCH):
                        dA = p2.tile([128, T2], f32, tag=f"dA{m}", bufs=CFG["hotbufs"],
                                     name=f"dA_{d}_{c2}_{s}_{m}")
                        ACT(dA[:], dt_sb[m][:], AF.Exp,
                            scale=A_sb[d][:, m, s : s + 1])
                        dAs.append(dA)
                    # per-state hs tile holding all 4 channel groups, so the
                    # chunk-boundary carry is ONE strided copy per state
                    hs = p2.tile([128, NCH, T2], bf16, tag="hs", bufs=2,
                                 name=f"hs_{d}_{c2}_{s}")
                    for m in range(NCH):
                        nc.vector.tensor_tensor_scan(
                            hs[:, m, :], dAs[m][:], bt[m][:],
                            carry[d][:, m, s : s + 1], OP.mult, OP.add)
                    veng(CFG["carry"]).tensor_copy(
                        carry[d][:, :, s : s + 1], hs[:, :, T2 - 1 : T2])
                    for m in range(NCH):
                        cm = p2.tile([128, T2], bf16, tag=f"cm{m}", bufs=CFG["hotbufs"],
                                     name=f"cm_{d}_{c2}_{s}_{m}")
                        veng(CFG["cm"][s * NCH + m]).tensor_tensor(
                            cm[:], hs[:, m, :], Cb[:], OP.mult)
                        MM(yps[m][:], ident_sb[:], cm[:],
                           start=(s == 0), stop=(s == DS - 1))
                # ---- gating: y = (ypsum + x*D) * silu(z) ----
                yg = []
                for m in range(NCH):
                    y1 = p2.tile([128, T2], bf16, tag=f"y1{m}", bufs=2,
                                 name=f"y1_{d}_{c2}_{m}")
                    veng(CFG["skip"]).scalar_tensor_tensor(
                        y1[:], rev(xd[m][:]), D_sb[d][:, m : m + 1], yps[m][:],
                        OP.mult, OP.add)
                    yt = p2.tile([128, T2], bf16, tag=f"yg{m}", bufs=2,
                                 name=f"yg_{d}_{c2}_{m}")
                    veng(CFG["gate"]).tensor_tensor(
                        yt[:], y1[:], rev(szt[m][:]), OP.mult)
                    yg.append(yt)
                ytot_cb(c2, yg)

        # ---- pass 2: reverse (spill gated y), then forward (combine +
        # out_proj); one shared pool so the passes overlap at the seam ----
        with tc.tile_pool(name="p2", bufs=1) as p2f, \
             tc.tile_pool(name="p2psum", bufs=1, space="PSUM") as p2fpsum:

            def spill_ygr(c2, yg):
                for m in range(NCH):
                    nc.sync.dma_start(
                        ygr_dram[m, :, c2 * T2 : (c2 + 1) * T2], yg[m][:]
                    )

            scan_pass("r", p2f, p2fpsum, spill_ygr, mmt_bufs=2)

            def combine_out(c2, yg):
                ytot = []
                for m in range(NCH):
                    ygr_t = p2f.tile([128, T2], bf16, tag=f"ygr{m}", bufs=2,
                                     name=f"ygr{m}_{c2}")
                    nc.sync.dma_start(
                        ygr_t[:], ygr_dram[m, :, L - (c2 + 1) * T2 : L - c2 * T2]
                    )
                    yt2 = p2f.tile([128, T2], bf16, tag=f"ytot{m}", bufs=2,
                                   name=f"ytot{m}_{c2}")
                    veng(CFG["comb"]).tensor_tensor(
                        yt2[:], yg[m][:], ygr_t[:, ::-1], OP.add)
                    ytot.append(yt2)
                    if dbg:
                        nc.sync.dma_start(
                            dbg["y_f"][m, :, c2 * T2 : (c2 + 1) * T2], yg[m][:]
                        )
                for mt in range(T2 // 128):
                    ob = p2f.tile([128, DM], bf16, tag="ob", bufs=2,
                                  name=f"ob_{c2}_{mt}")
                    for nh in range(DM // 512):
                        po = p2fpsum.tile([128, 512], f32, tag="po", bufs=2,
                                          name=f"po_{c2}_{mt}_{nh}")
                        for k in range(NCH):
                            MM(po[:], ytot[k][:, mt * 128 : (mt + 1) * 128],
                               wout_sb[:, k, nh * 512 : (nh + 1) * 512],
                               start=(k == 0), stop=(k == NCH - 1))
                        bcopy(CFG["obcopy"], ob[:, nh * 512 : (nh + 1) * 512],
                              po[:])
                    nc.sync.dma_start(
                        pout[c2 * T2 + mt * 128 : c2 * T2 + (mt + 1) * 128, :],
                        ob[:],
                    )

            scan_pass("f", p2f, p2fpsum, combine_out, mmt_bufs=2)


def _host_prep(inputs):
    """Slice/transpose the full inputs into the 8 per-core input maps."""
    import ml_dtypes
    bf = ml_dtypes.bfloat16

    h = np.asarray(inputs["hidden_states"], np.float32)
    W_in = np.asarray(inputs["W_in"], np.float32)
    W_out = np.asarray(inputs["W_out"], np.float32)

    sel = np.zeros((48, DS * 128), np.float32)
    for s in range(DS):
        sel[s, s * 128 : (s + 1) * 128] = 1.0
        sel[32 + s, s * 128 : (s + 1) * 128] = 1.0

    maps = []
    for core in range(8):
        b, g = divmod(core, 4)
        c0 = g * CH
        m = {
            "hT": np.ascontiguousarray(h[b].T).astype(bf),
            "winxT": np.ascontiguousarray(W_in[c0 : c0 + CH, :].T).astype(bf),
            "winzT": np.ascontiguousarray(W_in[DI + c0 : DI + c0 + CH, :].T).astype(bf),
            "woutT": np.ascontiguousarray(W_out[:, c0 : c0 + CH].T).astype(bf),
            "sel": sel.astype(bf),
            "ident": np.eye(128, dtype=np.float32).astype(bf),
        }
        for d in ("f", "r"):
            sfx = f"_{d}"
            W_x = np.asarray(inputs[f"W_x{sfx}"], np.float32)
            W_dt = np.asarray(inputs[f"W_dt{sfx}"], np.float32)
            A = -np.exp(np.asarray(inputs[f"A_log{sfx}"], np.float64)).astype(np.float32)
            cw = np.asarray(inputs[f"conv_w{sfx}"], np.float32)
            cb = np.asarray(inputs[f"conv_b{sfx}"], np.float32)
            db = np.asarray(inputs[f"b_dt{sfx}"], np.float32)
            Dp = np.asarray(inputs[f"D{sfx}"], np.float32)
            wx_re = np.zeros((CH, 128), np.float32)
            wx_re[:, 0:DS] = W_x[DR : DR + DS, c0 : c0 + CH].T        # B rows
            wx_re[:, 32 : 32 + DS] = W_x[DR + DS : 96, c0 : c0 + CH].T  # C rows
            wx_re[:, DR:128] = W_x[0:DR, c0 : c0 + CH].T              # dt-rank rows
            m[f"wx{sfx}"] = wx_re.astype(bf)
            m[f"wdt{sfx}"] = np.ascontiguousarray(W_dt[c0 : c0 + CH, :].T).astype(bf)
            # (CH, DS) -> (128, NCH, DS) -> (128, NCH*DS)
            m[f"A{sfx}"] = np.ascontiguousarray(
                A[c0 : c0 + CH].reshape(NCH, 128, DS).transpose(1, 0, 2).reshape(128, NCH * DS)
            )
            m[f"cw{sfx}"] = np.ascontiguousarray(
                cw[c0 : c0 + CH].reshape(NCH, 128, DC).transpose(1, 0, 2).reshape(128, NCH * DC)
            )
            cwd = np.zeros((NCH * DC, 128, 128), np.float32)
            cwc = cw[c0 : c0 + CH].reshape(NCH, 128, DC)
            for mm_ in range(NCH):
                for j in range(DC):
                    np.fill_diagonal(cwd[mm_ * DC + j], cwc[mm_, :, j])
            m[f"cwdiag{sfx}"] = cwd.astype(bf)
            m[f"cb{sfx}"] = np.ascontiguousarray(
                cb[c0 : c0 + CH].reshape(NCH, 128).T
            )
            m[f"db{sfx}"] = np.ascontiguousarray(
                db[c0 : c0 + CH].reshape(NCH, 128).T
            )
            m[f"D{sfx}"] = np.ascontiguousarray(
                Dp[c0 : c0 + CH].reshape(NCH, 128).T
            )
        maps.append(m)
    return maps


def run(inputs, debug=False, trace=False):
    from concourse.bass_utils import run_bass_kernel_spmd

    if _COMPILED[0] is None or _COMPILED[0][1] != debug:
        _COMPILED[0] = (_build_program(debug=debug), debug)
    nc = _COMPILED[0][0]
    maps = _host_prep(inputs)
    res = run_bass_kernel_spmd(nc, maps, core_ids=list(range(8)), trace=trace)
    outs = [np.asarray(r["pout"], np.float32) for r in res.results]
    full = np.zeros((B, L, DM), np.float32)
    for core in range(8):
        b = core // 4
        full[b] += outs[core]
    return full, res


def kernel(**inputs):
    out, _ = run(inputs, debug=False, trace=False)
    return out


# revision 27
# speedup vs baseline: 1.1651x; 1.0110x over previous
"""BiMamba (bidirectional Mamba block) Trainium2 kernel.

Contract: kernel(**inputs) takes the full (unsharded) numpy inputs of the
reference and returns the full (2, 4096, 1024) float32 output.

Sharding: 8 cores = 2 batches x 4 channel-groups of 512 d_inner channels.
Each core runs both scan directions for its channel slice; the x_dbl
reduction over d_inner is an on-chip AllReduce within each batch's 4-core
group; the host sums the four partial out-projections per batch.

Key algebraic facts used:
  * xz for the reverse direction is the L-flip of the forward xz, so the
    input projection is computed once.
  * (y_f + flip(y_r)) @ W_out.T == out_f + flip(out_r), so one output
    projection suffices.

Performance structure (engine balance per scan chunk):
  * Pool (gpsimd) runs the 64 tensor_tensor_scan ops (the serial core).
  * DVE runs the bf16 TensorTensor mults (2x_1p packed mode).
  * Act runs the exp/softplus and most PSUM->SBUF broadcast copies.
  * PE accumulates y over the 16 states via identity matmuls into PSUM,
    plus the projections.
  * All DMA uses contiguous descriptors (reversals happen in SBUF reads).
"""

import os
import sys

import numpy as np

sys.path.insert(0, "/opt/trn_rl_repo")

B, L, DM, DI, DS, DR, DC = 2, 4096, 1024, 2048, 16, 64, 4
CH = 512          # d_inner channels per core
NCH = CH // 128   # channel tiles per core
T1 = 512          # pass-1 (projection/conv) token chunk
NC1 = L // T1
T2 = 512          # pass-2 (scan) token chunk
NC2 = L // T2

# engine assignment tuning: V=DVE, P=Pool(gpsimd), A=Act
# (scans must run on DVE: walrus cannot lower tensor_tensor_scan on Pool)
CFG = dict(
    bcopy=os.environ.get("CFG_BCOPY", "A" * 16),     # per s: B broadcast copy
    ccopy=os.environ.get("CFG_CCOPY", "A" * 16),     # per s: C broadcast copy
    carry=os.environ.get("CFG_CARRY", "P"),          # batched carry copies
    bt=os.environ.get("CFG_BT", ""),                 # per (s*NCH+m): bt engine
    cm=os.environ.get("CFG_CM", ""),                 # per (s*NCH+m): cmul engine
    hotbufs=int(os.environ.get("CFG_HOTBUFS", "3")),  # bufs for s-loop tags
    wd=os.environ.get("CFG_WD", "V"),
    skip=os.environ.get("CFG_SKIP", "V"),
    gate=os.environ.get("CFG_GATE", "V"),
    comb=os.environ.get("CFG_COMB", "V"),
    conv=os.environ.get("CFG_CONV", "V"),            # V=DVE STT, E=PE diag-mm
    convsplit=os.environ.get("CFG_CONVSPLIT", "VVVV"),  # per-m conv engine (V/P)
    xebufs=int(os.environ.get("CFG_XEBUFS", "3")),
    obcopy=os.environ.get("CFG_OBCOPY", "A"),        # out_proj PSUM->SBUF copy
    flip=os.environ.get("CFG_FLIP", "A"),            # AR reverse stage copy
    p1bufs=int(os.environ.get("CFG_P1BUFS", "2")),   # pass-1 in_proj psum bufs
)
def _bres(k, n=64):
    out = []
    acc = 0
    for _ in range(n):
        acc += k
        if acc >= n:
            acc -= n
            out.append("V")
        else:
            out.append("P")
    return "".join(out)


if not CFG["bt"]:
    CFG["bt"] = _bres(39)
if not CFG["cm"]:
    CFG["cm"] = _bres(39)

_COMPILED = [None]


def _split_sync_waits(nc, mybir, max_waits=1):
    """walrus in this environment rejects >1 sync wait per instruction;
    hoist excess waits onto dedicated same-engine NOPs."""
    uid = [0]
    for f in nc.m.functions:
        for bb in f.blocks:
            new = []
            dirty = False
            for inst in bb.instructions:
                si = inst.sync_info
                if si is not None and len(si.on_wait) > max_waits:
                    waits = list(si.on_wait)
                    keep = waits[len(waits) - max_waits:]
                    hoist = waits[: len(waits) - max_waits]
                    for i in range(0, len(hoist), max_waits):
                        uid[0] += 1
                        nop = mybir.InstNoOp(
                            name=f"splitwait-{id(nc)}-{uid[0]}", engine=inst.engine
                        )
                        nop.sync_info = mybir.SyncInfo(
                            on_wait=hoist[i : i + max_waits], on_update=[]
                        )
                        nc.register_instruction(nop, overwrite=True)
                        new.append(nop)
                    inst.sync_info = mybir.SyncInfo(
                        on_wait=keep, on_update=list(si.on_update)
                    )
                    dirty = True
                new.append(inst)
            if dirty:
                bb.instructions = new


def _build_program(debug=False, collective=True):
    import concourse.bass as bass
    import concourse.tile as tile
    from concourse import mybir

    f32 = mybir.dt.float32
    f32r = mybir.dt.float32r
    bf16 = mybir.dt.bfloat16
    AF = mybir.ActivationFunctionType
    OP = mybir.AluOpType

    nc = bass.Bass("TRN2", target_bir_lowering=False, debug=False, num_devices=8)

    # ---- external inputs (per-core shards prepared on host) ----
    hT = nc.dram_tensor("hT", [DM, L], bf16, kind="ExternalInput")
    winxT = nc.dram_tensor("winxT", [DM, CH], bf16, kind="ExternalInput")
    winzT = nc.dram_tensor("winzT", [DM, CH], bf16, kind="ExternalInput")
    woutT_d = nc.dram_tensor("woutT", [CH, DM], bf16, kind="ExternalInput")
    sel_d = nc.dram_tensor("sel", [48, DS * 128], bf16, kind="ExternalInput")
    ident_d = nc.dram_tensor("ident", [128, 128], bf16, kind="ExternalInput")
    wx_d = {}
    wdt_d = {}
    A_d = {}
    cw_d = {}
    cwdiag_d = {}
    cb_d = {}
    db_d = {}
    D_d = {}
    for d in ("f", "r"):
        wx_d[d] = nc.dram_tensor(f"wx_{d}", [CH, 128], bf16, kind="ExternalInput")
        wdt_d[d] = nc.dram_tensor(f"wdt_{d}", [DR, CH], bf16, kind="ExternalInput")
        A_d[d] = nc.dram_tensor(f"A_{d}", [128, NCH * DS], f32, kind="ExternalInput")
        cw_d[d] = nc.dram_tensor(f"cw_{d}", [128, NCH * DC], f32,
                                 kind="ExternalInput")
        cwdiag_d[d] = nc.dram_tensor(f"cwdiag_{d}", [NCH * DC, 128, 128], bf16,
                                     kind="ExternalInput")
        cb_d[d] = nc.dram_tensor(f"cb_{d}", [128, NCH], f32, kind="ExternalInput")
        db_d[d] = nc.dram_tensor(f"db_{d}", [128, NCH], f32, kind="ExternalInput")
        D_d[d] = nc.dram_tensor(f"D_{d}", [128, NCH], f32, kind="ExternalInput")

    pout = nc.dram_tensor("pout", [L, DM], bf16, kind="ExternalOutput")
    dbg = {}
    if debug:
        dbg["xc_f"] = nc.dram_tensor("dbg_xc_f", [NCH, 128, L], bf16, kind="ExternalOutput")
        dbg["xc_r"] = nc.dram_tensor("dbg_xc_r", [NCH, 128, L], bf16, kind="ExternalOutput")
        dbg["xdbl_f"] = nc.dram_tensor("dbg_xdbl_f", [96, L], f32, kind="ExternalOutput")
        dbg["xdbl_r"] = nc.dram_tensor("dbg_xdbl_r", [96, L], f32, kind="ExternalOutput")
        dbg["dt_f"] = nc.dram_tensor("dbg_dt_f", [NCH, 128, L], f32, kind="ExternalOutput")
        dbg["y_f"] = nc.dram_tensor("dbg_y_f", [NCH, 128, L], bf16, kind="ExternalOutput")
        dbg["siluz"] = nc.dram_tensor("dbg_siluz", [NCH, 128, L], bf16, kind="ExternalOutput")

    with tile.TileContext(nc, num_cores=8) as tc:
        _build_tile_program(
            nc, tc, tile, mybir, f32, f32r, bf16, AF, OP,
            hT, winxT, winzT, woutT_d, sel_d, ident_d, wx_d, wdt_d, A_d, cw_d,
            cwdiag_d, cb_d, db_d, D_d, pout, dbg, collective,
        )

    _split_sync_waits(nc, mybir)
    return nc


def _build_tile_program(
    nc, tc, tile, mybir, f32, f32r, bf16, AF, OP,
    hT, winxT, winzT, woutT_d, sel_d, ident_d, wx_d, wdt_d, A_d, cw_d,
    cwdiag_d, cb_d, db_d, D_d, pout, dbg, collective=True,
):
    from contextlib import ExitStack

    MM = nc.tensor.matmul
    ACT = nc.scalar.activation
    TT = nc.vector.tensor_tensor
    STT = nc.vector.scalar_tensor_tensor
    TSMUL = nc.vector.tensor_scalar_mul

    def veng(code):
        return nc.vector if code == "V" else nc.gpsimd

    def bcopy(code, out, in_):
        """PSUM f32 -> SBUF copy on the chosen engine."""
        if code == "A":
            ACT(out, in_, AF.Copy)
        else:
            veng(code).tensor_copy(out, in_)

    ctx = ExitStack()
    with ctx:
        # -------- persistent pools --------
        pers = ctx.enter_context(tc.tile_pool(name="pers", bufs=1))
        dram = ctx.enter_context(tc.tile_pool(name="dram", bufs=1, space="DRAM"))

        # pass-2-only parameters: tiles declared here, loads DEFERRED into
        # the AllReduce gap so pass-1's first-chunk loads go first
        wout_sb = pers.tile([128, NCH, DM], bf16)
        sel_sb = pers.tile([48, DS * 128], bf16)
        ident_sb = pers.tile([128, 128], bf16)
        xdbl = {}      # bf16 [128, L]: rows [0:16]=B, [32:48]=C, [64:128]=dt-rank
        carry = {}
        wdt_sb = {}
        A_sb = {}
        db_sb = {}
        D_sb = {}
        for d in ("f", "r"):
            xdbl[d] = pers.tile([128, L], bf16, name=f"xdbl_{d}")
            carry[d] = pers.tile([128, NCH, DS], bf16, name=f"carry_{d}")
            wdt_sb[d] = pers.tile([128, CH], bf16, name=f"wdt_sb_{d}")
            A_sb[d] = pers.tile([128, NCH, DS], f32, name=f"A_sb_{d}")
            db_sb[d] = pers.tile([128, NCH], f32, name=f"db_sb_{d}")
            D_sb[d] = pers.tile([128, NCH], f32, name=f"D_sb_{d}")
        ones = pers.tile([128, 1], f32)

        def load_pass2_params():
            nc.sync.dma_start(wout_sb[:],
                              woutT_d.ap().rearrange("(k p) n -> p k n", p=128))
            nc.sync.dma_start(sel_sb[:], sel_d[:])
            nc.sync.dma_start(ident_sb[:], ident_d[:])
            for d in ("f", "r"):
                nc.vector.memset(carry[d][:], 0.0)
                nc.sync.dma_start(wdt_sb[d][DR:128, :], wdt_d[d][:])
                nc.sync.dma_start(
                    A_sb[d][:], A_d[d].ap().rearrange("p (m s) -> p m s", m=NCH))
                nc.sync.dma_start(db_sb[d][:], db_d[d][:])
                nc.sync.dma_start(D_sb[d][:], D_d[d][:])
            nc.vector.memset(ones[:], 1.0)

        # DRAM spill buffers (per-core local HBM); all in ORIGINAL time order
        # for the forward direction; xr/sz are original-time too (pass 2r
        # flips with reversed SBUF reads).  ygr is in flipped time.
        xf_dram = dram.tile([NCH, 128, L], bf16)
        xr_dram = dram.tile([NCH, 128, L], bf16)
        sz_dram = dram.tile([NCH, 128, L], bf16)
        ygr_dram = dram.tile([NCH, 128, L], bf16)
        # AllReduce staging: [dir, 128 rows, L] f32; rows as xdbl layout.
        # dir 0 = forward (original time), dir 1 = reverse (flipped time).
        ar_in = dram.tile([2, 128, L], f32)
        ar_out = dram.tile([2, 128, L], f32)

        # ================= PASS 1: in_proj + conv + silu + partial x_dbl ====
        with tc.tile_pool(name="p1", bufs=1) as p1, \
             tc.tile_pool(name="p1psum", bufs=1, space="PSUM") as p1psum:
            winx_sb = p1.tile([128, DM // 128, CH], bf16)
            nc.sync.dma_start(winx_sb[:], winxT.ap().rearrange("(k p) n -> p k n", p=128))
            winz_sb = p1.tile([128, DM // 128, CH], bf16)
            wx_sb = {}
            cw_sb = {}
            cb_sb = {}
            for d in ("f", "r"):
                wx_sb[d] = p1.tile([128, NCH, 128], bf16, name=f"wx_sb_{d}")
                nc.sync.dma_start(wx_sb[d][:], wx_d[d].ap().rearrange("(m p) n -> p m n", p=128))
                if CFG["conv"] == "E":
                    cw_sb[d] = p1.tile([128, NCH * DC, 128], bf16,
                                       name=f"cw_sb_{d}")
                    nc.sync.dma_start(
                        cw_sb[d][:], cwdiag_d[d].ap().rearrange("k p n -> p k n"))
                else:
                    cw_sb[d] = p1.tile([128, NCH, DC], f32, name=f"cw_sb_{d}")
                    nc.sync.dma_start(
                        cw_sb[d][:], cw_d[d].ap().rearrange("p (m j) -> p m j", m=NCH))
                cb_sb[d] = p1.tile([128, NCH], f32, name=f"cb_sb_{d}")
                nc.sync.dma_start(cb_sb[d][:], cb_d[d][:])

            hT_r = hT.ap().rearrange("(k p) l -> p k l", p=128)
            prev_xe = [None] * NCH
            prev_hTt = None

            def emit_z(cc, hTt_cc):
                for m in range(NCH):
                    ps = p1psum.tile([128, T1], f32, tag="ps_ip",
                                     bufs=CFG["p1bufs"], name=f"psz_{cc}_{m}")
                    for ko in range(DM // 128):
                        MM(ps[:], winz_sb[:, ko, m * 128 : (m + 1) * 128],
                           hTt_cc[:, ko, :],
                           start=(ko == 0), stop=(ko == DM // 128 - 1))
                    zs = p1.tile([128, T1], bf16, tag=f"zs{m}", bufs=2,
                                 name=f"zs{m}_{cc}")
                    ACT(zs[:], ps[:], AF.Silu)
                    nc.sync.dma_start(sz_dram[m, :, cc * T1 : (cc + 1) * T1], zs[:])
                    if dbg:
                        nc.sync.dma_start(
                            dbg["siluz"][m, :, cc * T1 : (cc + 1) * T1], zs[:]
                        )

            def conv_dir(cc, d, xe_list):
                """Causal (d=f) / anti-causal (d=r) depthwise conv + silu on
                original-time chunk cc, using extended tiles [3|T1|3].
                Conv runs on DVE (tap0 as 4x tensor_scalar, taps 1-3 as
                STT accumulate).  Returns bf16 silu'd tiles."""
                out = []
                for m in range(NCH):
                    xe = xe_list[m]
                    if CFG["conv"] == "E":
                        acc = p1psum.tile([128, T1], f32, tag="cps", bufs=2,
                                          name=f"cps{m}_{d}_{cc}")
                        for j in range(DC):
                            off = j if d == "f" else (6 - j)
                            MM(acc[:], cw_sb[d][:, m * DC + j, :],
                               xe[:, off : off + T1],
                               start=(j == 0), stop=(j == DC - 1))
                    else:
                        ce = veng(CFG["convsplit"][m])
                        acc = p1.tile([128, T1], f32, tag=f"cacc{m}", bufs=2,
                                      name=f"cacc{m}_{d}_{cc}")
                        for j in range(DC):
                            off = j if d == "f" else (6 - j)
                            src = xe[:, off : off + T1]
                            wj = cw_sb[d][:, m, j : j + 1]
                            if j == 0:
                                ce.tensor_scalar_mul(acc[:], src, wj)
                            else:
                                ce.scalar_tensor_tensor(
                                    acc[:], src, wj, acc[:], OP.mult, OP.add)
                    xcb = p1.tile([128, T1], bf16, tag=f"xcb{m}_{d}", bufs=2,
                                  name=f"xcb{m}_{d}_{cc}")
                    ACT(xcb[:], acc[:], AF.Silu, bias=cb_sb[d][:, m : m + 1])
                    out.append(xcb)
                return out

            def xdbl_chunk(cc, d, xc_tiles):
                # psum rows laid out as [B 0:16 | C 32:48 | dt 64:128]
                # (W_x rows reordered+padded on host); full 128 rows go to AR.
                ps = p1psum.tile([128, T1], f32, tag="psx", bufs=2,
                                 name=f"psx_{d}_{cc}")
                for m in range(NCH):
                    MM(ps[:], wx_sb[d][:, m, :], xc_tiles[m][:],
                       start=(m == 0), stop=(m == NCH - 1))
                stage = p1.tile([128, T1], f32, tag="arstage", bufs=2,
                                name=f"arstage_{d}_{cc}")
                if d == "f":
                    ACT(stage[:], ps[:], AF.Copy)
                    nc.sync.dma_start(
                        ar_in[0, :, cc * T1 : (cc + 1) * T1], stage[:]
                    )
                else:
                    if CFG["flip"] == "A":
                        ACT(stage[:], ps[:, ::-1], AF.Copy)
                    else:
                        nc.vector.tensor_copy(stage[:], ps[:, ::-1])
                    nc.sync.dma_start(
                        ar_in[1, :, L - (cc + 1) * T1 : L - cc * T1], stage[:]
                    )

            def spill_chunk(cc, d, xc_tiles):
                x_dram = xf_dram if d == "f" else xr_dram
                for m in range(NCH):
                    nc.sync.dma_start(
                        x_dram[m, :, cc * T1 : (cc + 1) * T1], xc_tiles[m][:]
                    )
                    if dbg:
                        key = "xc_f" if d == "f" else "xc_r"
                        nc.sync.dma_start(
                            dbg[key][m, :, cc * T1 : (cc + 1) * T1], xc_tiles[m][:]
                        )

            def finish_reverse(cc, xe_list):
                xcr = conv_dir(cc, "r", xe_list)
                xdbl_chunk(cc, "r", xcr)
                spill_chunk(cc, "r", xcr)

            for c in range(NC1):
                hTt = p1.tile([128, DM // 128, T1], bf16, tag="hTt", bufs=3,
                              name=f"hTt_{c}")
                nc.sync.dma_start(hTt[:], hT_r[:, :, c * T1 : (c + 1) * T1])
                if c == 0:
                    # z weights are first needed one chunk later; keep the
                    # first hTt chunk ahead of them in the DMA queue
                    nc.sync.dma_start(
                        winz_sb[:],
                        winzT.ap().rearrange("(k p) n -> p k n", p=128))

                # x part (extended with halos) and z part (-> silu -> spill)
                cur_xe = []
                for m in range(NCH):
                    ps = p1psum.tile([128, T1], f32, tag="ps_ip", bufs=CFG["p1bufs"],
                                     name=f"psx_{c}_{m}")
                    for ko in range(DM // 128):
                        MM(ps[:], winx_sb[:, ko, m * 128 : (m + 1) * 128],
                           hTt[:, ko, :], start=(ko == 0), stop=(ko == DM // 128 - 1))
                    xe = p1.tile([128, T1 + 6], bf16, tag=f"xe{m}", bufs=CFG["xebufs"],
                                 name=f"xe{m}_{c}")
                    ACT(xe[:, 3 : 3 + T1], ps[:], AF.Copy)
                    if c == 0:
                        nc.vector.memset(xe[:, 0:3], 0.0)
                    else:
                        nc.vector.tensor_copy(xe[:, 0:3], prev_xe[m][:, T1 : T1 + 3])
                    cur_xe.append(xe)
                if c > 0:
                    # fill previous chunk's right halo, then do its reverse conv
                    for m in range(NCH):
                        nc.vector.tensor_copy(
                            prev_xe[m][:, T1 + 3 : T1 + 6], cur_xe[m][:, 3:6]
                        )
                    finish_reverse(c - 1, prev_xe)

                # forward conv on current chunk
                xcf = conv_dir(c, "f", cur_xe)
                xdbl_chunk(c, "f", xcf)
                spill_chunk(c, "f", xcf)

                # z projection deferred by one chunk: z is consumed only by
                # pass 2, so it stays off pass-1's critical path (the final
                # chunk's z runs inside the AllReduce gap)
                if c > 0:
                    emit_z(c - 1, prev_hTt)
                prev_xe = cur_xe
                prev_hTt = hTt

            for m in range(NCH):
                nc.vector.memset(prev_xe[m][:, T1 + 3 : T1 + 6], 0.0)
            finish_reverse(NC1 - 1, prev_xe)
            load_pass2_params()
            emit_z(NC1 - 1, prev_hTt)

            # -------- AllReduce of x_dbl over the 4 cores of this batch ----
            # reverse direction first: pass 2r starts as soon as its rows
            # are reduced, overlapping the forward AR
            for di, d in ((1, "r"), (0, "f")):
                if collective:
                    nc.gpsimd.collective_compute(
                        "AllReduce", OP.add,
                        replica_groups=[[0, 1, 2, 3], [4, 5, 6, 7]],
                        ins=[ar_in[di].opt()], outs=[ar_out[di].opt()],
                    )
                else:
                    nc.gpsimd.dma_start(ar_out[di], ar_in[di])
                # cast-readback f32 -> bf16 into SBUF (gpsimd DMAs may cast)
                nc.gpsimd.dma_start(xdbl[d][:], ar_out[di, :, :])
            if dbg:
                for di, d in enumerate(("f", "r")):
                    nc.sync.dma_start(dbg[f"xdbl_{d}"][0:64, :], ar_out[di, 64:128, :])
                    nc.sync.dma_start(dbg[f"xdbl_{d}"][64:80, :], ar_out[di, 0:16, :])
                    nc.sync.dma_start(dbg[f"xdbl_{d}"][80:96, :], ar_out[di, 32:48, :])

        # ================= PASS 2: dt + selective scan (+gating, out_proj) ==
        def scan_pass(d, p2, p2psum, ytot_cb, mmt_bufs=3):
            """d: 'f' or 'r'.  'r' reads x/sz spills (original time) with
            reversed SBUF access; everything else runs in flipped time.
            ytot_cb(c2, yg_tiles): consumes gated y tiles for chunk c2."""
            x_dram = xf_dram if d == "f" else xr_dram
            rev = (lambda ap: ap) if d == "f" else (lambda ap: ap[:, ::-1])
            for c2 in range(NC2):
                sl = slice(c2 * T2, (c2 + 1) * T2)
                osl = sl if d == "f" else slice(L - (c2 + 1) * T2, L - c2 * T2)
                # ---- dt projection + softplus (f32 path) ----
                dt_sb = []
                for m in range(NCH):
                    psd = p2psum.tile([128, T2], f32, tag="mmt", bufs=mmt_bufs,
                                      name=f"psd_{d}_{c2}_{m}")
                    MM(psd[:], wdt_sb[d][DR:128, m * 128 : (m + 1) * 128],
                       xdbl[d][DR:128, sl], start=True, stop=True)
                    et = p2.tile([128, T2], f32, tag="et", bufs=2,
                                 name=f"et_{d}_{c2}_{m}")
                    ACT(et[:], psd[:], AF.Exp, bias=db_sb[d][:, m : m + 1])
                    dt = p2.tile([128, T2], bf16, tag=f"dt{m}", bufs=2,
                                 name=f"dt{m}_{d}_{c2}")
                    ACT(dt[:], et[:], AF.Ln, bias=ones[:])
                    dt_sb.append(dt)
                    if dbg and d == "f":
                        nc.sync.dma_start(dbg["dt_f"][m, :, sl], dt[:])
                # ---- x load (bf16) + wd = dt*x + silu(z) load ----
                xd = []
                wd = []
                szt = []
                for m in range(NCH):
                    xt = p2.tile([128, T2], bf16, tag=f"xd{m}", bufs=2,
                                 name=f"xd{m}_{d}_{c2}")
                    nc.sync.dma_start(xt[:], x_dram[m, :, osl])
                    xd.append(xt)
                    wt = p2.tile([128, T2], bf16, tag=f"wd{m}", bufs=2,
                                 name=f"wd{m}_{d}_{c2}")
                    veng(CFG["wd"]).tensor_tensor(
                        wt[:], dt_sb[m][:], rev(xt[:]), OP.mult)
                    wd.append(wt)
                    sz = p2.tile([128, T2], bf16, tag=f"sz{m}", bufs=2,
                                 name=f"sz{m}_{d}_{c2}")
                    nc.sync.dma_start(sz[:], sz_dram[m, :, osl])
                    szt.append(sz)
                # ---- selective scan over 16 states ----
                yps = [p2psum.tile([128, T2], f32, tag=f"yp{m}", bufs=1,
                                   name=f"yp{m}_{d}_{c2}") for m in range(NCH)]

                def bc_bcast(s):
                    """sel-matmul broadcast + copy for state s -> (Bb, Cb)."""
                    Bbp = p2psum.tile([128, T2], f32, tag="mmt", bufs=mmt_bufs,
                                      name=f"Bbp_{d}_{c2}_{s}")
                    MM(Bbp[:], sel_sb[0:DS, s * 128 : (s + 1) * 128],
                       xdbl[d][0:DS, sl], start=True, stop=True)
                    Bb = p2.tile([128, T2], bf16, tag="Bbs", bufs=CFG["hotbufs"],
                                 name=f"Bb_{d}_{c2}_{s}")
                    bcopy(CFG["bcopy"][s], Bb[:], Bbp[:])
                    Cbp = p2psum.tile([128, T2], f32, tag="mmt", bufs=mmt_bufs,
                                      name=f"Cbp_{d}_{c2}_{s}")
                    MM(Cbp[:], sel_sb[32 : 32 + DS, s * 128 : (s + 1) * 128],
                       xdbl[d][32 : 32 + DS, sl], start=True, stop=True)
                    Cb = p2.tile([128, T2], bf16, tag="Cbs", bufs=CFG["hotbufs"],
                                 name=f"Cb_{d}_{c2}_{s}")
                    bcopy(CFG["ccopy"][s], Cb[:], Cbp[:])
                    return Bb, Cb

                nxt_bc = bc_bcast(0)
                for s in range(DS):
                    Bb, Cb = nxt_bc
                    bt = []
                    for m in range(NCH):
                        b = p2.tile([128, T2], bf16, tag=f"bt{m}", bufs=CFG["hotbufs"],
                                    name=f"bt_{d}_{c2}_{s}_{m}")
                        veng(CFG["bt"][s * NCH + m]).tensor_tensor(
                            b[:], wd[m][:], Bb[:], OP.mult)
                        bt.append(b)
                    dAs = []
                    for m in range(NCH):
                        dA = p2.tile([128, T2], f32, tag=f"dA{m}", bufs=CFG["hotbufs"],
                                     name=f"dA_{d}_{c2}_{s}_{m}")
                        ACT(dA[:], dt_sb[m][:], AF.Exp,
                            scale=A_sb[d][:, m, s : s + 1])
                        dAs.append(dA)
                    # lookahead: issue next state's broadcasts ahead of the
                    # yacc matmuls so PE's in-order queue can't stall them
                    # behind cmul-dependent work
                    if s + 1 < DS:
                        nxt_bc = bc_bcast(s + 1)
                    # per-state hs tile holding all 4 channel groups, so the
                    # chunk-boundary carry is ONE strided copy per state
                    hs = p2.tile([128, NCH, T2], bf16, tag="hs", bufs=2,
                                 name=f"hs_{d}_{c2}_{s}")
                    for m in range(NCH):
                        nc.vector.tensor_tensor_scan(
                            hs[:, m, :], dAs[m][:], bt[m][:],
                            carry[d][:, m, s : s + 1], OP.mult, OP.add)
                    veng(CFG["carry"]).tensor_copy(
                        carry[d][:, :, s : s + 1], hs[:, :, T2 - 1 : T2])
                    for m in range(NCH):
                        cm = p2.tile([128, T2], bf16, tag=f"cm{m}", bufs=CFG["hotbufs"],
                                     name=f"cm_{d}_{c2}_{s}_{m}")
                        veng(CFG["cm"][s * NCH + m]).tensor_tensor(
                            cm[:], hs[:, m, :], Cb[:], OP.mult)
                        MM(yps[m][:], ident_sb[:], cm[:],
                           start=(s == 0), stop=(s == DS - 1))
                # ---- gating: y = (ypsum + x*D) * silu(z) ----
                yg = []
                for m in range(NCH):
                    y1 = p2.tile([128, T2], bf16, tag=f"y1{m}", bufs=2,
                                 name=f"y1_{d}_{c2}_{m}")
                    veng(CFG["skip"]).scalar_tensor_tensor(
                        y1[:], rev(xd[m][:]), D_sb[d][:, m : m + 1], yps[m][:],
                        OP.mult, OP.add)
                    yt = p2.tile([128, T2], bf16, tag=f"yg{m}", bufs=2,
                                 name=f"yg_{d}_{c2}_{m}")
                    veng(CFG["gate"]).tensor_tensor(
                        yt[:], y1[:], rev(szt[m][:]), OP.mult)
                    yg.append(yt)
                ytot_cb(c2, yg)

        # ---- pass 2: reverse (spill gated y), then forward (combine +
        # out_proj); one shared pool so the passes overlap at the seam ----
        with tc.tile_pool(name="p2", bufs=1) as p2f, \
             tc.tile_pool(name="p2psum", bufs=1, space="PSUM") as p2fpsum:

            def spill_ygr(c2, yg):
                for m in range(NCH):
                    nc.sync.dma_start(
                        ygr_dram[m, :, c2 * T2 : (c2 + 1) * T2], yg[m][:]
                    )

            scan_pass("r", p2f, p2fpsum, spill_ygr, mmt_bufs=2)

            def combine_out(c2, yg):
                ytot = []
                for m in range(NCH):
                    ygr_t = p2f.tile([128, T2], bf16, tag=f"ygr{m}", bufs=2,
                                     name=f"ygr{m}_{c2}")
                    nc.sync.dma_start(
                        ygr_t[:], ygr_dram[m, :, L - (c2 + 1) * T2 : L - c2 * T2]
                    )
                    yt2 = p2f.tile([128, T2], bf16, tag=f"ytot{m}", bufs=2,
                                   name=f"ytot{m}_{c2}")
                    veng(CFG["comb"]).tensor_tensor(
                        yt2[:], yg[m][:], ygr_t[:, ::-1], OP.add)
                    ytot.append(yt2)
                    if dbg:
                        nc.sync.dma_start(
                            dbg["y_f"][m, :, c2 * T2 : (c2 + 1) * T2], yg[m][:]
                        )
                for mt in range(T2 // 128):
                    ob = p2f.tile([128, DM], bf16, tag="ob", bufs=2,
                                  name=f"ob_{c2}_{mt}")
                    for nh in range(DM // 512):
                        po = p2fpsum.tile([128, 512], f32, tag="po", bufs=2,
                                          name=f"po_{c2}_{mt}_{nh}")
                        for k in range(NCH):
                            MM(po[:], ytot[k][:, mt * 128 : (mt + 1) * 128],
                               wout_sb[:, k, nh * 512 : (nh + 1) * 512],
                               start=(k == 0), stop=(k == NCH - 1))
                        bcopy(CFG["obcopy"], ob[:, nh * 512 : (nh + 1) * 512],
                              po[:])
                    nc.sync.dma_start(
                        pout[c2 * T2 + mt * 128 : c2 * T2 + (mt + 1) * 128, :],
                        ob[:],
                    )

            scan_pass("f", p2f, p2fpsum, combine_out, mmt_bufs=2)


def _host_prep(inputs):
    """Slice/transpose the full inputs into the 8 per-core input maps."""
    import ml_dtypes
    bf = ml_dtypes.bfloat16

    h = np.asarray(inputs["hidden_states"], np.float32)
    W_in = np.asarray(inputs["W_in"], np.float32)
    W_out = np.asarray(inputs["W_out"], np.float32)

    sel = np.zeros((48, DS * 128), np.float32)
    for s in range(DS):
        sel[s, s * 128 : (s + 1) * 128] = 1.0
        sel[32 + s, s * 128 : (s + 1) * 128] = 1.0

    maps = []
    for core in range(8):
        b, g = divmod(core, 4)
        c0 = g * CH
        m = {
            "hT": np.ascontiguousarray(h[b].T).astype(bf),
            "winxT": np.ascontiguousarray(W_in[c0 : c0 + CH, :].T).astype(bf),
            "winzT": np.ascontiguousarray(W_in[DI + c0 : DI + c0 + CH, :].T).astype(bf),
            "woutT": np.ascontiguousarray(W_out[:, c0 : c0 + CH].T).astype(bf),
            "sel": sel.astype(bf),
            "ident": np.eye(128, dtype=np.float32).astype(bf),
        }
        for d in ("f", "r"):
            sfx = f"_{d}"
            W_x = np.asarray(inputs[f"W_x{sfx}"], np.float32)
            W_dt = np.asarray(inputs[f"W_dt{sfx}"], np.float32)
            A = -np.exp(np.asarray(inputs[f"A_log{sfx}"], np.float64)).astype(np.float32)
            cw = np.asarray(inputs[f"conv_w{sfx}"], np.float32)
            cb = np.asarray(inputs[f"conv_b{sfx}"], np.float32)
            db = np.asarray(inputs[f"b_dt{sfx}"], np.float32)
            Dp = np.asarray(inputs[f"D{sfx}"], np.float32)
            wx_re = np.zeros((CH, 128), np.float32)
            wx_re[:, 0:DS] = W_x[DR : DR + DS, c0 : c0 + CH].T        # B rows
            wx_re[:, 32 : 32 + DS] = W_x[DR + DS : 96, c0 : c0 + CH].T  # C rows
            wx_re[:, DR:128] = W_x[0:DR, c0 : c0 + CH].T              # dt-rank rows
            m[f"wx{sfx}"] = wx_re.astype(bf)
            m[f"wdt{sfx}"] = np.ascontiguousarray(W_dt[c0 : c0 + CH, :].T).astype(bf)
            # (CH, DS) -> (128, NCH, DS) -> (128, NCH*DS)
            m[f"A{sfx}"] = np.ascontiguousarray(
                A[c0 : c0 + CH].reshape(NCH, 128, DS).transpose(1, 0, 2).reshape(128, NCH * DS)
            )
            m[f"cw{sfx}"] = np.ascontiguousarray(
                cw[c0 : c0 + CH].reshape(NCH, 128, DC).transpose(1, 0, 2).reshape(128, NCH * DC)
            )
            cwd = np.zeros((NCH * DC, 128, 128), np.float32)
            cwc = cw[c0 : c0 + CH].reshape(NCH, 128, DC)
            for mm_ in range(NCH):
                for j in range(DC):
                    np.fill_diagonal(cwd[mm_ * DC + j], cwc[mm_, :, j])
            m[f"cwdiag{sfx}"] = cwd.astype(bf)
            m[f"cb{sfx}"] = np.ascontiguousarray(
                cb[c0 : c0 + CH].reshape(NCH, 128).T
            )
            m[f"db{sfx}"] = np.ascontiguousarray(
                db[c0 : c0 + CH].reshape(NCH, 128).T
            )
            m[f"D{sfx}"] = np.ascontiguousarray(
                Dp[c0 : c0 + CH].reshape(NCH, 128).T
            )
        maps.append(m)
    return maps


def run(inputs, debug=False, trace=False):
    from concourse.bass_utils import run_bass_kernel_spmd

    if _COMPILED[0] is None or _COMPILED[0][1] != debug:
        _COMPILED[0] = (_build_program(debug=debug), debug)
    nc = _COMPILED[0][0]
    maps = _host_prep(inputs)
    res = run_bass_kernel_spmd(nc, maps, core_ids=list(range(8)), trace=trace)
    outs = [np.asarray(r["pout"], np.float32) for r in res.results]
    full = np.zeros((B, L, DM), np.float32)
    for core in range(8):
        b = core // 4
        full[b] += outs[core]
    return full, res


def kernel(**inputs):
    out, _ = run(inputs, debug=False, trace=False)
    return out


# revision 31
# speedup vs baseline: 1.1812x; 1.0138x over previous
"""BiMamba (bidirectional Mamba block) Trainium2 kernel.

Contract: kernel(**inputs) takes the full (unsharded) numpy inputs of the
reference and returns the full (2, 4096, 1024) float32 output.

Sharding: 8 cores = 2 batches x 4 channel-groups of 512 d_inner channels.
Each core runs both scan directions for its channel slice; the x_dbl
reduction over d_inner is an on-chip AllReduce within each batch's 4-core
group; the host sums the four partial out-projections per batch.

Key algebraic facts used:
  * xz for the reverse direction is the L-flip of the forward xz, so the
    input projection is computed once.
  * (y_f + flip(y_r)) @ W_out.T == out_f + flip(out_r), so one output
    projection suffices.

Performance structure (engine balance per scan chunk):
  * Pool (gpsimd) runs the 64 tensor_tensor_scan ops (the serial core).
  * DVE runs the bf16 TensorTensor mults (2x_1p packed mode).
  * Act runs the exp/softplus and most PSUM->SBUF broadcast copies.
  * PE accumulates y over the 16 states via identity matmuls into PSUM,
    plus the projections.
  * All DMA uses contiguous descriptors (reversals happen in SBUF reads).
"""

import os
import sys

import numpy as np

sys.path.insert(0, "/opt/trn_rl_repo")

B, L, DM, DI, DS, DR, DC = 2, 4096, 1024, 2048, 16, 64, 4
CH = 512          # d_inner channels per core
NCH = CH // 128   # channel tiles per core
T1 = 512          # pass-1 (projection/conv) token chunk
NC1 = L // T1
T2 = 512          # pass-2 (scan) token chunk
NC2 = L // T2

# engine assignment tuning: V=DVE, P=Pool(gpsimd), A=Act
# (scans must run on DVE: walrus cannot lower tensor_tensor_scan on Pool)
CFG = dict(
    bcopy=os.environ.get("CFG_BCOPY", "A" * 16),     # per s: B broadcast copy
    ccopy=os.environ.get("CFG_CCOPY", "A" * 16),     # per s: C broadcast copy
    carry=os.environ.get("CFG_CARRY", "P"),          # batched carry copies
    bt=os.environ.get("CFG_BT", ""),                 # per (s*NCH+m): bt engine
    cm=os.environ.get("CFG_CM", ""),                 # per (s*NCH+m): cmul engine
    hotbufs=int(os.environ.get("CFG_HOTBUFS", "3")),  # bufs for s-loop tags
    wd=os.environ.get("CFG_WD", "V"),
    skip=os.environ.get("CFG_SKIP", "V"),  # y1 PSUM->SBUF copy (A/V: Pool cannot read PSUM)
    gate=os.environ.get("CFG_GATE", "V"),
    comb=os.environ.get("CFG_COMB", "V"),
    conv=os.environ.get("CFG_CONV", "V"),            # V=DVE STT, E=PE diag-mm
    convsplit=os.environ.get("CFG_CONVSPLIT", "VVVV"),  # per-m conv engine (V/P)
    convtree=os.environ.get("CFG_CONVTREE", "0"),    # 1: bf16 product tree, j1 on Act
    xebufs=int(os.environ.get("CFG_XEBUFS", "3")),
    obcopy=os.environ.get("CFG_OBCOPY", "A"),        # out_proj PSUM->SBUF copy
    flip=os.environ.get("CFG_FLIP", "A"),            # AR reverse stage copy
    p1bufs=int(os.environ.get("CFG_P1BUFS", "2")),   # pass-1 in_proj psum bufs
)
def _bres(k, n=64):
    out = []
    acc = 0
    for _ in range(n):
        acc += k
        if acc >= n:
            acc -= n
            out.append("V")
        else:
            out.append("P")
    return "".join(out)


if not CFG["bt"]:
    CFG["bt"] = _bres(39)
if not CFG["cm"]:
    CFG["cm"] = _bres(39)

_COMPILED = [None]


def _split_sync_waits(nc, mybir, max_waits=1):
    """walrus in this environment rejects >1 sync wait per instruction;
    hoist excess waits onto dedicated same-engine NOPs."""
    uid = [0]
    for f in nc.m.functions:
        for bb in f.blocks:
            new = []
            dirty = False
            for inst in bb.instructions:
                si = inst.sync_info
                if si is not None and len(si.on_wait) > max_waits:
                    waits = list(si.on_wait)
                    keep = waits[len(waits) - max_waits:]
                    hoist = waits[: len(waits) - max_waits]
                    for i in range(0, len(hoist), max_waits):
                        uid[0] += 1
                        nop = mybir.InstNoOp(
                            name=f"splitwait-{id(nc)}-{uid[0]}", engine=inst.engine
                        )
                        nop.sync_info = mybir.SyncInfo(
                            on_wait=hoist[i : i + max_waits], on_update=[]
                        )
                        nc.register_instruction(nop, overwrite=True)
                        new.append(nop)
                    inst.sync_info = mybir.SyncInfo(
                        on_wait=keep, on_update=list(si.on_update)
                    )
                    dirty = True
                new.append(inst)
            if dirty:
                bb.instructions = new


def _build_program(debug=False, collective=True):
    import concourse.bass as bass
    import concourse.tile as tile
    from concourse import mybir

    f32 = mybir.dt.float32
    f32r = mybir.dt.float32r
    bf16 = mybir.dt.bfloat16
    AF = mybir.ActivationFunctionType
    OP = mybir.AluOpType

    nc = bass.Bass("TRN2", target_bir_lowering=False, debug=False, num_devices=8)

    # ---- external inputs (per-core shards prepared on host) ----
    hT = nc.dram_tensor("hT", [DM, L], bf16, kind="ExternalInput")
    winxT = nc.dram_tensor("winxT", [DM, CH], bf16, kind="ExternalInput")
    winzT = nc.dram_tensor("winzT", [DM, CH], bf16, kind="ExternalInput")
    woutT_d = nc.dram_tensor("woutT", [CH, DM], bf16, kind="ExternalInput")
    sel_d = nc.dram_tensor("sel", [48, DS * 128], bf16, kind="ExternalInput")
    ident_d = nc.dram_tensor("ident", [128, 128], bf16, kind="ExternalInput")
    wx_d = {}
    wdt_d = {}
    A_d = {}
    cw_d = {}
    cwdiag_d = {}
    cb_d = {}
    db_d = {}
    D_d = {}
    Ddiag_d = {}
    for d in ("f", "r"):
        wx_d[d] = nc.dram_tensor(f"wx_{d}", [CH, 128], bf16, kind="ExternalInput")
        wdt_d[d] = nc.dram_tensor(f"wdt_{d}", [DR, CH], bf16, kind="ExternalInput")
        A_d[d] = nc.dram_tensor(f"A_{d}", [128, NCH * DS], f32, kind="ExternalInput")
        cw_d[d] = nc.dram_tensor(f"cw_{d}", [128, NCH * DC], f32,
                                 kind="ExternalInput")
        cwdiag_d[d] = nc.dram_tensor(f"cwdiag_{d}", [NCH * DC, 128, 128], bf16,
                                     kind="ExternalInput")
        cb_d[d] = nc.dram_tensor(f"cb_{d}", [128, NCH], f32, kind="ExternalInput")
        db_d[d] = nc.dram_tensor(f"db_{d}", [128, NCH], f32, kind="ExternalInput")
        D_d[d] = nc.dram_tensor(f"D_{d}", [128, NCH], f32, kind="ExternalInput")
        Ddiag_d[d] = nc.dram_tensor(f"Ddiag_{d}", [NCH, 128, 128], bf16,
                                    kind="ExternalInput")

    pout = nc.dram_tensor("pout", [L, DM], bf16, kind="ExternalOutput")
    dbg = {}
    if debug:
        dbg["xc_f"] = nc.dram_tensor("dbg_xc_f", [NCH, 128, L], bf16, kind="ExternalOutput")
        dbg["xc_r"] = nc.dram_tensor("dbg_xc_r", [NCH, 128, L], bf16, kind="ExternalOutput")
        dbg["xdbl_f"] = nc.dram_tensor("dbg_xdbl_f", [96, L], f32, kind="ExternalOutput")
        dbg["xdbl_r"] = nc.dram_tensor("dbg_xdbl_r", [96, L], f32, kind="ExternalOutput")
        dbg["dt_f"] = nc.dram_tensor("dbg_dt_f", [NCH, 128, L], f32, kind="ExternalOutput")
        dbg["y_f"] = nc.dram_tensor("dbg_y_f", [NCH, 128, L], bf16, kind="ExternalOutput")
        dbg["siluz"] = nc.dram_tensor("dbg_siluz", [NCH, 128, L], bf16, kind="ExternalOutput")

    with tile.TileContext(nc, num_cores=8) as tc:
        _build_tile_program(
            nc, tc, tile, mybir, f32, f32r, bf16, AF, OP,
            hT, winxT, winzT, woutT_d, sel_d, ident_d, wx_d, wdt_d, A_d, cw_d,
            cwdiag_d, cb_d, db_d, D_d, Ddiag_d, pout, dbg, collective,
        )

    _split_sync_waits(nc, mybir)
    return nc


def _build_tile_program(
    nc, tc, tile, mybir, f32, f32r, bf16, AF, OP,
    hT, winxT, winzT, woutT_d, sel_d, ident_d, wx_d, wdt_d, A_d, cw_d,
    cwdiag_d, cb_d, db_d, D_d, Ddiag_d, pout, dbg, collective=True,
):
    from contextlib import ExitStack

    MM = nc.tensor.matmul
    ACT = nc.scalar.activation
    TT = nc.vector.tensor_tensor
    STT = nc.vector.scalar_tensor_tensor
    TSMUL = nc.vector.tensor_scalar_mul

    def veng(code):
        return nc.vector if code == "V" else nc.gpsimd

    def bcopy(code, out, in_):
        """PSUM f32 -> SBUF copy on the chosen engine."""
        if code == "A":
            ACT(out, in_, AF.Copy)
        else:
            veng(code).tensor_copy(out, in_)

    ctx = ExitStack()
    with ctx:
        # -------- persistent pools --------
        pers = ctx.enter_context(tc.tile_pool(name="pers", bufs=1))
        dram = ctx.enter_context(tc.tile_pool(name="dram", bufs=1, space="DRAM"))

        # pass-2-only parameters: tiles declared here, loads DEFERRED into
        # the AllReduce gap so pass-1's first-chunk loads go first
        wout_sb = pers.tile([128, NCH, DM], bf16)
        sel_sb = pers.tile([48, DS * 128], bf16)
        ident_sb = pers.tile([128, 128], bf16)
        xdbl = {}      # bf16 [128, L]: rows [0:16]=B, [32:48]=C, [64:128]=dt-rank
        carry = {}
        wdt_sb = {}
        A_sb = {}
        db_sb = {}
        D_sb = {}
        Ddiag_sb = {}
        for d in ("f", "r"):
            xdbl[d] = pers.tile([128, L], bf16, name=f"xdbl_{d}")
            carry[d] = pers.tile([128, NCH, DS], bf16, name=f"carry_{d}")
            wdt_sb[d] = pers.tile([128, CH], bf16, name=f"wdt_sb_{d}")
            A_sb[d] = pers.tile([128, NCH, DS], f32, name=f"A_sb_{d}")
            db_sb[d] = pers.tile([128, NCH], f32, name=f"db_sb_{d}")
            D_sb[d] = pers.tile([128, NCH], f32, name=f"D_sb_{d}")
            Ddiag_sb[d] = pers.tile([128, NCH, 128], bf16, name=f"Ddiag_sb_{d}")
        ones = pers.tile([128, 1], f32)

        def load_pass2_params():
            nc.sync.dma_start(wout_sb[:],
                              woutT_d.ap().rearrange("(k p) n -> p k n", p=128))
            nc.sync.dma_start(sel_sb[:], sel_d[:])
            nc.sync.dma_start(ident_sb[:], ident_d[:])
            for d in ("f", "r"):
                nc.vector.memset(carry[d][:], 0.0)
                nc.sync.dma_start(wdt_sb[d][DR:128, :], wdt_d[d][:])
                nc.sync.dma_start(
                    A_sb[d][:], A_d[d].ap().rearrange("p (m s) -> p m s", m=NCH))
                nc.sync.dma_start(db_sb[d][:], db_d[d][:])
                nc.sync.dma_start(D_sb[d][:], D_d[d][:])
                nc.sync.dma_start(
                    Ddiag_sb[d][:],
                    Ddiag_d[d].ap().rearrange("m p n -> p m n"))
            nc.vector.memset(ones[:], 1.0)

        # DRAM spill buffers (per-core local HBM); all in ORIGINAL time order
        # for the forward direction; xr/sz are original-time too (pass 2r
        # flips with reversed SBUF reads).  ygr is in flipped time.
        xf_dram = dram.tile([NCH, 128, L], bf16)
        xr_dram = dram.tile([NCH, 128, L], bf16)
        sz_dram = dram.tile([NCH, 128, L], bf16)
        ygr_dram = dram.tile([NCH, 128, L], bf16)
        # AllReduce staging: [dir, 128 rows, L] f32; rows as xdbl layout.
        # dir 0 = forward (original time), dir 1 = reverse (flipped time).
        ar_in = dram.tile([2, 128, L], f32)
        ar_out = dram.tile([2, 128, L], f32)

        # ================= PASS 1: in_proj + conv + silu + partial x_dbl ====
        with tc.tile_pool(name="p1", bufs=1) as p1, \
             tc.tile_pool(name="p1psum", bufs=1, space="PSUM") as p1psum:
            winx_sb = p1.tile([128, DM // 128, CH], bf16)
            nc.sync.dma_start(winx_sb[:], winxT.ap().rearrange("(k p) n -> p k n", p=128))
            winz_sb = p1.tile([128, DM // 128, CH], bf16)
            wx_sb = {}
            cw_sb = {}
            cb_sb = {}
            for d in ("f", "r"):
                wx_sb[d] = p1.tile([128, NCH, 128], bf16, name=f"wx_sb_{d}")
                nc.sync.dma_start(wx_sb[d][:], wx_d[d].ap().rearrange("(m p) n -> p m n", p=128))
                if CFG["conv"] == "E":
                    cw_sb[d] = p1.tile([128, NCH * DC, 128], bf16,
                                       name=f"cw_sb_{d}")
                    nc.sync.dma_start(
                        cw_sb[d][:], cwdiag_d[d].ap().rearrange("k p n -> p k n"))
                else:
                    cw_sb[d] = p1.tile([128, NCH, DC], f32, name=f"cw_sb_{d}")
                    nc.sync.dma_start(
                        cw_sb[d][:], cw_d[d].ap().rearrange("p (m j) -> p m j", m=NCH))
                cb_sb[d] = p1.tile([128, NCH], f32, name=f"cb_sb_{d}")
                nc.sync.dma_start(cb_sb[d][:], cb_d[d][:])

            hT_r = hT.ap().rearrange("(k p) l -> p k l", p=128)
            prev_xe = [None] * NCH
            prev_hTt = None

            def emit_z(cc, hTt_cc):
                for m in range(NCH):
                    ps = p1psum.tile([128, T1], f32, tag="ps_ip",
                                     bufs=CFG["p1bufs"], name=f"psz_{cc}_{m}")
                    for ko in range(DM // 128):
                        MM(ps[:], winz_sb[:, ko, m * 128 : (m + 1) * 128],
                           hTt_cc[:, ko, :],
                           start=(ko == 0), stop=(ko == DM // 128 - 1))
                    zs = p1.tile([128, T1], bf16, tag=f"zs{m}", bufs=2,
                                 name=f"zs{m}_{cc}")
                    ACT(zs[:], ps[:], AF.Silu)
                    nc.sync.dma_start(sz_dram[m, :, cc * T1 : (cc + 1) * T1], zs[:])
                    if dbg:
                        nc.sync.dma_start(
                            dbg["siluz"][m, :, cc * T1 : (cc + 1) * T1], zs[:]
                        )

            def conv_dir(cc, d, xe_list):
                """Causal (d=f) / anti-causal (d=r) depthwise conv + silu on
                original-time chunk cc, using extended tiles [3|T1|3].
                Conv runs on DVE (tap0 as 4x tensor_scalar, taps 1-3 as
                STT accumulate).  Returns bf16 silu'd tiles."""
                out = []
                for m in range(NCH):
                    xe = xe_list[m]
                    if CFG["conv"] == "E":
                        acc = p1psum.tile([128, T1], f32, tag="cps", bufs=2,
                                          name=f"cps{m}_{d}_{cc}")
                        for j in range(DC):
                            off = j if d == "f" else (6 - j)
                            MM(acc[:], cw_sb[d][:, m * DC + j, :],
                               xe[:, off : off + T1],
                               start=(j == 0), stop=(j == DC - 1))
                    elif CFG["convtree"] == "1":
                        # bf16 product tree: taps 0/2/3 as 4x TSMUL on DVE,
                        # tap 1 as per-partition scaled copy on Act, then
                        # three 2x bf16 adds on DVE
                        tp = []
                        for j in range(DC):
                            off = j if d == "f" else (6 - j)
                            src = xe[:, off : off + T1]
                            wj = cw_sb[d][:, m, j : j + 1]
                            t = p1.tile([128, T1], bf16, tag=f"ct{m}_{j}",
                                        bufs=2, name=f"ct{m}_{j}_{d}_{cc}")
                            if j == 1:
                                ACT(t[:], src, AF.Copy, scale=wj)
                            else:
                                TSMUL(t[:], src, wj)
                            tp.append(t)
                        acc = p1.tile([128, T1], bf16, tag=f"cacc{m}", bufs=2,
                                      name=f"cacc{m}_{d}_{cc}")
                        TT(tp[0][:], tp[0][:], tp[1][:], OP.add)
                        TT(tp[2][:], tp[2][:], tp[3][:], OP.add)
                        TT(acc[:], tp[0][:], tp[2][:], OP.add)
                    else:
                        ce = veng(CFG["convsplit"][m])
                        acc = p1.tile([128, T1], f32, tag=f"cacc{m}", bufs=2,
                                      name=f"cacc{m}_{d}_{cc}")
                        for j in range(DC):
                            off = j if d == "f" else (6 - j)
                            src = xe[:, off : off + T1]
                            wj = cw_sb[d][:, m, j : j + 1]
                            if j == 0:
                                ce.tensor_scalar_mul(acc[:], src, wj)
                            else:
                                ce.scalar_tensor_tensor(
                                    acc[:], src, wj, acc[:], OP.mult, OP.add)
                    xcb = p1.tile([128, T1], bf16, tag=f"xcb{m}_{d}", bufs=2,
                                  name=f"xcb{m}_{d}_{cc}")
                    # reverse direction: write silu output time-flipped so the
                    # spill/x_dbl/pass-2r all see flipped time with contiguous
                    # DMAs and unreversed matmul operands
                    dst = xcb[:, ::-1] if d == "r" else xcb[:]
                    ACT(dst, acc[:], AF.Silu, bias=cb_sb[d][:, m : m + 1])
                    out.append(xcb)
                return out

            def xdbl_chunk(cc, d, xc_tiles):
                # psum rows laid out as [B 0:16 | C 32:48 | dt 64:128]
                # (W_x rows reordered+padded on host); full 128 rows go to AR.
                ps = p1psum.tile([128, T1], f32, tag="psx", bufs=2,
                                 name=f"psx_{d}_{cc}")
                for m in range(NCH):
                    MM(ps[:], wx_sb[d][:, m, :], xc_tiles[m][:],
                       start=(m == 0), stop=(m == NCH - 1))
                stage = p1.tile([128, T1], f32, tag="arstage", bufs=2,
                                name=f"arstage_{d}_{cc}")
                if d == "f":
                    ACT(stage[:], ps[:], AF.Copy)
                    nc.sync.dma_start(
                        ar_in[0, :, cc * T1 : (cc + 1) * T1], stage[:]
                    )
                else:
                    ACT(stage[:], ps[:], AF.Copy)
                    nc.sync.dma_start(
                        ar_in[1, :, L - (cc + 1) * T1 : L - cc * T1], stage[:]
                    )

            def spill_chunk(cc, d, xc_tiles):
                x_dram = xf_dram if d == "f" else xr_dram
                for m in range(NCH):
                    dsl = (slice(cc * T1, (cc + 1) * T1) if d == "f"
                           else slice(L - (cc + 1) * T1, L - cc * T1))
                    nc.sync.dma_start(x_dram[m, :, dsl], xc_tiles[m][:])
                    if dbg:
                        key = "xc_f" if d == "f" else "xc_r"
                        nc.sync.dma_start(
                            dbg[key][m, :, cc * T1 : (cc + 1) * T1], xc_tiles[m][:]
                        )

            def finish_reverse(cc, xe_list):
                xcr = conv_dir(cc, "r", xe_list)
                xdbl_chunk(cc, "r", xcr)
                spill_chunk(cc, "r", xcr)

            for c in range(NC1):
                hTt = p1.tile([128, DM // 128, T1], bf16, tag="hTt", bufs=3,
                              name=f"hTt_{c}")
                nc.sync.dma_start(hTt[:], hT_r[:, :, c * T1 : (c + 1) * T1])
                if c == 0:
                    # z weights are first needed one chunk later; keep the
                    # first hTt chunk ahead of them in the DMA queue
                    nc.sync.dma_start(
                        winz_sb[:],
                        winzT.ap().rearrange("(k p) n -> p k n", p=128))

                # x part (extended with halos) and z part (-> silu -> spill)
                cur_xe = []
                for m in range(NCH):
                    ps = p1psum.tile([128, T1], f32, tag="ps_ip", bufs=CFG["p1bufs"],
                                     name=f"psx_{c}_{m}")
                    for ko in range(DM // 128):
                        MM(ps[:], winx_sb[:, ko, m * 128 : (m + 1) * 128],
                           hTt[:, ko, :], start=(ko == 0), stop=(ko == DM // 128 - 1))
                    xe = p1.tile([128, T1 + 6], bf16, tag=f"xe{m}", bufs=CFG["xebufs"],
                                 name=f"xe{m}_{c}")
                    ACT(xe[:, 3 : 3 + T1], ps[:], AF.Copy)
                    if c == 0:
                        nc.vector.memset(xe[:, 0:3], 0.0)
                    else:
                        nc.vector.tensor_copy(xe[:, 0:3], prev_xe[m][:, T1 : T1 + 3])
                    cur_xe.append(xe)
                if c > 0:
                    # fill previous chunk's right halo, then do its reverse conv
                    for m in range(NCH):
                        nc.vector.tensor_copy(
                            prev_xe[m][:, T1 + 3 : T1 + 6], cur_xe[m][:, 3:6]
                        )
                    finish_reverse(c - 1, prev_xe)

                # forward conv on current chunk
                xcf = conv_dir(c, "f", cur_xe)
                xdbl_chunk(c, "f", xcf)
                spill_chunk(c, "f", xcf)

                # z projection deferred by one chunk: z is consumed only by
                # pass 2, so it stays off pass-1's critical path (the final
                # chunk's z runs inside the AllReduce gap)
                if c > 0:
                    emit_z(c - 1, prev_hTt)
                prev_xe = cur_xe
                prev_hTt = hTt

            for m in range(NCH):
                nc.vector.memset(prev_xe[m][:, T1 + 3 : T1 + 6], 0.0)
            finish_reverse(NC1 - 1, prev_xe)
            load_pass2_params()
            emit_z(NC1 - 1, prev_hTt)

            # -------- AllReduce of x_dbl over the 4 cores of this batch ----
            # reverse direction first: pass 2r starts as soon as its rows
            # are reduced, overlapping the forward AR
            for di, d in ((1, "r"), (0, "f")):
                if collective:
                    nc.gpsimd.collective_compute(
                        "AllReduce", OP.add,
                        replica_groups=[[0, 1, 2, 3], [4, 5, 6, 7]],
                        ins=[ar_in[di].opt()], outs=[ar_out[di].opt()],
                    )
                else:
                    nc.gpsimd.dma_start(ar_out[di], ar_in[di])
                # cast-readback f32 -> bf16 into SBUF (gpsimd DMAs may cast)
                nc.gpsimd.dma_start(xdbl[d][:], ar_out[di, :, :])
            if dbg:
                for di, d in enumerate(("f", "r")):
                    nc.sync.dma_start(dbg[f"xdbl_{d}"][0:64, :], ar_out[di, 64:128, :])
                    nc.sync.dma_start(dbg[f"xdbl_{d}"][64:80, :], ar_out[di, 0:16, :])
                    nc.sync.dma_start(dbg[f"xdbl_{d}"][80:96, :], ar_out[di, 32:48, :])

        # ================= PASS 2: dt + selective scan (+gating, out_proj) ==
        def scan_pass(d, p2, p2psum, ytot_cb, mmt_bufs=3):
            """d: 'f' or 'r'.  'r' reads x/sz spills (original time) with
            reversed SBUF access; everything else runs in flipped time.
            ytot_cb(c2, yg_tiles): consumes gated y tiles for chunk c2."""
            x_dram = xf_dram if d == "f" else xr_dram
            rev = (lambda ap: ap) if d == "f" else (lambda ap: ap[:, ::-1])
            for c2 in range(NC2):
                sl = slice(c2 * T2, (c2 + 1) * T2)
                osl = sl if d == "f" else slice(L - (c2 + 1) * T2, L - c2 * T2)
                # ---- dt projection + softplus (f32 path) ----
                dt_sb = []
                for m in range(NCH):
                    psd = p2psum.tile([128, T2], f32, tag="mmt", bufs=mmt_bufs,
                                      name=f"psd_{d}_{c2}_{m}")
                    MM(psd[:], wdt_sb[d][DR:128, m * 128 : (m + 1) * 128],
                       xdbl[d][DR:128, sl], start=True, stop=True)
                    et = p2.tile([128, T2], f32, tag="et", bufs=2,
                                 name=f"et_{d}_{c2}_{m}")
                    ACT(et[:], psd[:], AF.Exp, bias=db_sb[d][:, m : m + 1])
                    dt = p2.tile([128, T2], bf16, tag=f"dt{m}", bufs=2,
                                 name=f"dt{m}_{d}_{c2}")
                    ACT(dt[:], et[:], AF.Ln, bias=ones[:])
                    dt_sb.append(dt)
                    if dbg and d == "f":
                        nc.sync.dma_start(dbg["dt_f"][m, :, sl], dt[:])
                # ---- x load (bf16) + wd = dt*x + silu(z) load ----
                xd = []
                wd = []
                szt = []
                for m in range(NCH):
                    xt = p2.tile([128, T2], bf16, tag=f"xd{m}", bufs=2,
                                 name=f"xd{m}_{d}_{c2}")
                    # xr spill is already in flipped time: load the pass-time
                    # slice directly, no reversed reads needed
                    nc.sync.dma_start(xt[:], x_dram[m, :, sl])
                    xd.append(xt)
                    wt = p2.tile([128, T2], bf16, tag=f"wd{m}", bufs=2,
                                 name=f"wd{m}_{d}_{c2}")
                    veng(CFG["wd"]).tensor_tensor(
                        wt[:], dt_sb[m][:], xt[:], OP.mult)
                    wd.append(wt)
                    sz = p2.tile([128, T2], bf16, tag=f"sz{m}", bufs=2,
                                 name=f"sz{m}_{d}_{c2}")
                    nc.sync.dma_start(sz[:], sz_dram[m, :, osl])
                    szt.append(sz)
                # ---- selective scan over 16 states ----
                yps = [p2psum.tile([128, T2], f32, tag=f"yp{m}", bufs=1,
                                   name=f"yp{m}_{d}_{c2}") for m in range(NCH)]

                def bc_bcast(s):
                    """sel-matmul broadcast + copy for state s -> (Bb, Cb)."""
                    Bbp = p2psum.tile([128, T2], f32, tag="mmt", bufs=mmt_bufs,
                                      name=f"Bbp_{d}_{c2}_{s}")
                    MM(Bbp[:], sel_sb[0:DS, s * 128 : (s + 1) * 128],
                       xdbl[d][0:DS, sl], start=True, stop=True)
                    Bb = p2.tile([128, T2], bf16, tag="Bbs", bufs=CFG["hotbufs"],
                                 name=f"Bb_{d}_{c2}_{s}")
                    bcopy(CFG["bcopy"][s], Bb[:], Bbp[:])
                    Cbp = p2psum.tile([128, T2], f32, tag="mmt", bufs=mmt_bufs,
                                      name=f"Cbp_{d}_{c2}_{s}")
                    MM(Cbp[:], sel_sb[32 : 32 + DS, s * 128 : (s + 1) * 128],
                       xdbl[d][32 : 32 + DS, sl], start=True, stop=True)
                    Cb = p2.tile([128, T2], bf16, tag="Cbs", bufs=CFG["hotbufs"],
                                 name=f"Cb_{d}_{c2}_{s}")
                    bcopy(CFG["ccopy"][s], Cb[:], Cbp[:])
                    return Bb, Cb

                nxt_bc = bc_bcast(0)
                for s in range(DS):
                    Bb, Cb = nxt_bc
                    bt = []
                    for m in range(NCH):
                        b = p2.tile([128, T2], bf16, tag=f"bt{m}", bufs=CFG["hotbufs"],
                                    name=f"bt_{d}_{c2}_{s}_{m}")
                        veng(CFG["bt"][s * NCH + m]).tensor_tensor(
                            b[:], wd[m][:], Bb[:], OP.mult)
                        bt.append(b)
                    dAs = []
                    for m in range(NCH):
                        dA = p2.tile([128, T2], f32, tag=f"dA{m}", bufs=CFG["hotbufs"],
                                     name=f"dA_{d}_{c2}_{s}_{m}")
                        ACT(dA[:], dt_sb[m][:], AF.Exp,
                            scale=A_sb[d][:, m, s : s + 1])
                        dAs.append(dA)
                    # lookahead: issue next state's broadcasts ahead of the
                    # yacc matmuls so PE's in-order queue can't stall them
                    # behind cmul-dependent work
                    if s + 1 < DS:
                        nxt_bc = bc_bcast(s + 1)
                    # per-state hs tile holding all 4 channel groups, so the
                    # chunk-boundary carry is ONE strided copy per state
                    hs = p2.tile([128, NCH, T2], bf16, tag="hs", bufs=2,
                                 name=f"hs_{d}_{c2}_{s}")
                    for m in range(NCH):
                        nc.vector.tensor_tensor_scan(
                            hs[:, m, :], dAs[m][:], bt[m][:],
                            carry[d][:, m, s : s + 1], OP.mult, OP.add)
                    veng(CFG["carry"]).tensor_copy(
                        carry[d][:, :, s : s + 1], hs[:, :, T2 - 1 : T2])
                    for m in range(NCH):
                        cm = p2.tile([128, T2], bf16, tag=f"cm{m}", bufs=CFG["hotbufs"],
                                     name=f"cm_{d}_{c2}_{s}_{m}")
                        veng(CFG["cm"][s * NCH + m]).tensor_tensor(
                            cm[:], hs[:, m, :], Cb[:], OP.mult)
                        MM(yps[m][:], ident_sb[:], cm[:],
                           start=(s == 0), stop=False)
                # ---- gating: y = (ypsum + x*D) * silu(z) ----
                yg = []
                for m in range(NCH):
                    # x*D skip joins the PSUM accumulation as a diag matmul
                    MM(yps[m][:], Ddiag_sb[d][:, m, :], xd[m][:],
                       start=False, stop=True)
                    yt = p2.tile([128, T2], bf16, tag=f"yg{m}", bufs=2,
                                 name=f"yg_{d}_{c2}_{m}")
                    veng(CFG["gate"]).tensor_tensor(
                        yt[:], yps[m][:], rev(szt[m][:]), OP.mult)
                    yg.append(yt)
                ytot_cb(c2, yg)

        # ---- pass 2: reverse (spill gated y), then forward (combine +
        # out_proj); one shared pool so the passes overlap at the seam ----
        with tc.tile_pool(name="p2", bufs=1) as p2f, \
             tc.tile_pool(name="p2psum", bufs=1, space="PSUM") as p2fpsum:

            def spill_ygr(c2, yg):
                for m in range(NCH):
                    nc.sync.dma_start(
                        ygr_dram[m, :, c2 * T2 : (c2 + 1) * T2], yg[m][:]
                    )

            scan_pass("r", p2f, p2fpsum, spill_ygr, mmt_bufs=2)

            def combine_out(c2, yg):
                ytot = []
                for m in range(NCH):
                    ygr_t = p2f.tile([128, T2], bf16, tag=f"ygr{m}", bufs=2,
                                     name=f"ygr{m}_{c2}")
                    nc.sync.dma_start(
                        ygr_t[:], ygr_dram[m, :, L - (c2 + 1) * T2 : L - c2 * T2]
                    )
                    yt2 = p2f.tile([128, T2], bf16, tag=f"ytot{m}", bufs=2,
                                   name=f"ytot{m}_{c2}")
                    veng(CFG["comb"]).tensor_tensor(
                        yt2[:], yg[m][:], ygr_t[:, ::-1], OP.add)
                    ytot.append(yt2)
                    if dbg:
                        nc.sync.dma_start(
                            dbg["y_f"][m, :, c2 * T2 : (c2 + 1) * T2], yg[m][:]
                        )
                for mt in range(T2 // 128):
                    ob = p2f.tile([128, DM], bf16, tag="ob", bufs=2,
                                  name=f"ob_{c2}_{mt}")
                    for nh in range(DM // 512):
                        po = p2fpsum.tile([128, 512], f32, tag="po", bufs=2,
                                          name=f"po_{c2}_{mt}_{nh}")
                        for k in range(NCH):
                            MM(po[:], ytot[k][:, mt * 128 : (mt + 1) * 128],
                               wout_sb[:, k, nh * 512 : (nh + 1) * 512],
                               start=(k == 0), stop=(k == NCH - 1))
                        bcopy(CFG["obcopy"], ob[:, nh * 512 : (nh + 1) * 512],
                              po[:])
                    nc.sync.dma_start(
                        pout[c2 * T2 + mt * 128 : c2 * T2 + (mt + 1) * 128, :],
                        ob[:],
                    )

            scan_pass("f", p2f, p2fpsum, combine_out, mmt_bufs=2)


def _host_prep(inputs):
    """Slice/transpose the full inputs into the 8 per-core input maps."""
    import ml_dtypes
    bf = ml_dtypes.bfloat16

    h = np.asarray(inputs["hidden_states"], np.float32)
    W_in = np.asarray(inputs["W_in"], np.float32)
    W_out = np.asarray(inputs["W_out"], np.float32)

    sel = np.zeros((48, DS * 128), np.float32)
    for s in range(DS):
        sel[s, s * 128 : (s + 1) * 128] = 1.0
        sel[32 + s, s * 128 : (s + 1) * 128] = 1.0

    maps = []
    for core in range(8):
        b, g = divmod(core, 4)
        c0 = g * CH
        m = {
            "hT": np.ascontiguousarray(h[b].T).astype(bf),
            "winxT": np.ascontiguousarray(W_in[c0 : c0 + CH, :].T).astype(bf),
            "winzT": np.ascontiguousarray(W_in[DI + c0 : DI + c0 + CH, :].T).astype(bf),
            "woutT": np.ascontiguousarray(W_out[:, c0 : c0 + CH].T).astype(bf),
            "sel": sel.astype(bf),
            "ident": np.eye(128, dtype=np.float32).astype(bf),
        }
        for d in ("f", "r"):
            sfx = f"_{d}"
            W_x = np.asarray(inputs[f"W_x{sfx}"], np.float32)
            W_dt = np.asarray(inputs[f"W_dt{sfx}"], np.float32)
            A = -np.exp(np.asarray(inputs[f"A_log{sfx}"], np.float64)).astype(np.float32)
            cw = np.asarray(inputs[f"conv_w{sfx}"], np.float32)
            cb = np.asarray(inputs[f"conv_b{sfx}"], np.float32)
            db = np.asarray(inputs[f"b_dt{sfx}"], np.float32)
            Dp = np.asarray(inputs[f"D{sfx}"], np.float32)
            wx_re = np.zeros((CH, 128), np.float32)
            wx_re[:, 0:DS] = W_x[DR : DR + DS, c0 : c0 + CH].T        # B rows
            wx_re[:, 32 : 32 + DS] = W_x[DR + DS : 96, c0 : c0 + CH].T  # C rows
            wx_re[:, DR:128] = W_x[0:DR, c0 : c0 + CH].T              # dt-rank rows
            m[f"wx{sfx}"] = wx_re.astype(bf)
            m[f"wdt{sfx}"] = np.ascontiguousarray(W_dt[c0 : c0 + CH, :].T).astype(bf)
            # (CH, DS) -> (128, NCH, DS) -> (128, NCH*DS)
            m[f"A{sfx}"] = np.ascontiguousarray(
                A[c0 : c0 + CH].reshape(NCH, 128, DS).transpose(1, 0, 2).reshape(128, NCH * DS)
            )
            m[f"cw{sfx}"] = np.ascontiguousarray(
                cw[c0 : c0 + CH].reshape(NCH, 128, DC).transpose(1, 0, 2).reshape(128, NCH * DC)
            )
            cwd = np.zeros((NCH * DC, 128, 128), np.float32)
            cwc = cw[c0 : c0 + CH].reshape(NCH, 128, DC)
            for mm_ in range(NCH):
                for j in range(DC):
                    np.fill_diagonal(cwd[mm_ * DC + j], cwc[mm_, :, j])
            m[f"cwdiag{sfx}"] = cwd.astype(bf)
            m[f"cb{sfx}"] = np.ascontiguousarray(
                cb[c0 : c0 + CH].reshape(NCH, 128).T
            )
            m[f"db{sfx}"] = np.ascontiguousarray(
                db[c0 : c0 + CH].reshape(NCH, 128).T
            )
            m[f"D{sfx}"] = np.ascontiguousarray(
                Dp[c0 : c0 + CH].reshape(NCH, 128).T
            )
            dd = np.zeros((NCH, 128, 128), np.float32)
            for mm_ in range(NCH):
                np.fill_diagonal(dd[mm_], Dp[c0 + mm_ * 128 : c0 + (mm_ + 1) * 128])
            m[f"Ddiag{sfx}"] = dd.astype(bf)
        maps.append(m)
    return maps


def run(inputs, debug=False, trace=False):
    from concourse.bass_utils import run_bass_kernel_spmd

    if _COMPILED[0] is None or _COMPILED[0][1] != debug:
        _COMPILED[0] = (_build_program(debug=debug), debug)
    nc = _COMPILED[0][0]
    maps = _host_prep(inputs)
    res = run_bass_kernel_spmd(nc, maps, core_ids=list(range(8)), trace=trace)
    outs = [np.asarray(r["pout"], np.float32) for r in res.results]
    full = np.zeros((B, L, DM), np.float32)
    for core in range(8):
        b = core // 4
        full[b] += outs[core]
    return full, res


def kernel(**inputs):
    out, _ = run(inputs, debug=False, trace=False)
    return out


# revision 32
# speedup vs baseline: 1.1983x; 1.0145x over previous
"""BiMamba (bidirectional Mamba block) Trainium2 kernel.

Contract: kernel(**inputs) takes the full (unsharded) numpy inputs of the
reference and returns the full (2, 4096, 1024) float32 output.

Sharding: 8 cores = 2 batches x 4 channel-groups of 512 d_inner channels.
Each core runs both scan directions for its channel slice; the x_dbl
reduction over d_inner is an on-chip AllReduce within each batch's 4-core
group; the host sums the four partial out-projections per batch.

Key algebraic facts used:
  * xz for the reverse direction is the L-flip of the forward xz, so the
    input projection is computed once.
  * (y_f + flip(y_r)) @ W_out.T == out_f + flip(out_r), so one output
    projection suffices.

Performance structure (engine balance per scan chunk):
  * Pool (gpsimd) runs the 64 tensor_tensor_scan ops (the serial core).
  * DVE runs the bf16 TensorTensor mults (2x_1p packed mode).
  * Act runs the exp/softplus and most PSUM->SBUF broadcast copies.
  * PE accumulates y over the 16 states via identity matmuls into PSUM,
    plus the projections.
  * All DMA uses contiguous descriptors (reversals happen in SBUF reads).
"""

import os
import sys

import numpy as np

sys.path.insert(0, "/opt/trn_rl_repo")

B, L, DM, DI, DS, DR, DC = 2, 4096, 1024, 2048, 16, 64, 4
CH = 512          # d_inner channels per core
NCH = CH // 128   # channel tiles per core
T1 = 512          # pass-1 (projection/conv) token chunk
NC1 = L // T1
T2 = 512          # pass-2 (scan) token chunk
NC2 = L // T2

# engine assignment tuning: V=DVE, P=Pool(gpsimd), A=Act
# (scans must run on DVE: walrus cannot lower tensor_tensor_scan on Pool)
CFG = dict(
    bcopy=os.environ.get("CFG_BCOPY", "A" * 16),     # per s: B broadcast copy
    ccopy=os.environ.get("CFG_CCOPY", "A" * 16),     # per s: C broadcast copy
    carry=os.environ.get("CFG_CARRY", "P"),          # batched carry copies
    bt=os.environ.get("CFG_BT", ""),                 # per (s*NCH+m): bt engine
    cm=os.environ.get("CFG_CM", ""),                 # per (s*NCH+m): cmul engine
    hotbufs=int(os.environ.get("CFG_HOTBUFS", "3")),  # bufs for s-loop tags
    wd=os.environ.get("CFG_WD", "V"),
    skip=os.environ.get("CFG_SKIP", "V"),  # y1 PSUM->SBUF copy (A/V: Pool cannot read PSUM)
    gate=os.environ.get("CFG_GATE", "V"),
    comb=os.environ.get("CFG_COMB", "V"),
    conv=os.environ.get("CFG_CONV", "V"),            # V=DVE STT, E=PE diag-mm
    convsplit=os.environ.get("CFG_CONVSPLIT", "VVVV"),  # per-m conv engine (V/P)
    convtree=os.environ.get("CFG_CONVTREE", "0"),    # 1: bf16 product tree, j1 on Act
    xebufs=int(os.environ.get("CFG_XEBUFS", "3")),
    obcopy=os.environ.get("CFG_OBCOPY", "A"),        # out_proj PSUM->SBUF copy
    flip=os.environ.get("CFG_FLIP", "A"),            # AR reverse stage copy
    p1bufs=int(os.environ.get("CFG_P1BUFS", "2")),   # pass-1 in_proj psum bufs
)
def _bres(k, n=64):
    out = []
    acc = 0
    for _ in range(n):
        acc += k
        if acc >= n:
            acc -= n
            out.append("V")
        else:
            out.append("P")
    return "".join(out)


if not CFG["bt"]:
    CFG["bt"] = _bres(44)
if not CFG["cm"]:
    CFG["cm"] = _bres(32)

_COMPILED = [None]


def _split_sync_waits(nc, mybir, max_waits=1):
    """walrus in this environment rejects >1 sync wait per instruction;
    hoist excess waits onto dedicated same-engine NOPs."""
    uid = [0]
    for f in nc.m.functions:
        for bb in f.blocks:
            new = []
            dirty = False
            for inst in bb.instructions:
                si = inst.sync_info
                if si is not None and len(si.on_wait) > max_waits:
                    waits = list(si.on_wait)
                    keep = waits[len(waits) - max_waits:]
                    hoist = waits[: len(waits) - max_waits]
                    for i in range(0, len(hoist), max_waits):
                        uid[0] += 1
                        nop = mybir.InstNoOp(
                            name=f"splitwait-{id(nc)}-{uid[0]}", engine=inst.engine
                        )
                        nop.sync_info = mybir.SyncInfo(
                            on_wait=hoist[i : i + max_waits], on_update=[]
                        )
                        nc.register_instruction(nop, overwrite=True)
                        new.append(nop)
                    inst.sync_info = mybir.SyncInfo(
                        on_wait=keep, on_update=list(si.on_update)
                    )
                    dirty = True
                new.append(inst)
            if dirty:
                bb.instructions = new


def _build_program(debug=False, collective=True):
    import concourse.bass as bass
    import concourse.tile as tile
    from concourse import mybir

    f32 = mybir.dt.float32
    f32r = mybir.dt.float32r
    bf16 = mybir.dt.bfloat16
    AF = mybir.ActivationFunctionType
    OP = mybir.AluOpType

    nc = bass.Bass("TRN2", target_bir_lowering=False, debug=False, num_devices=8)

    # ---- external inputs (per-core shards prepared on host) ----
    hT = nc.dram_tensor("hT", [DM, L], bf16, kind="ExternalInput")
    winxT = nc.dram_tensor("winxT", [DM, CH], bf16, kind="ExternalInput")
    winzT = nc.dram_tensor("winzT", [DM, CH], bf16, kind="ExternalInput")
    woutT_d = nc.dram_tensor("woutT", [CH, DM], bf16, kind="ExternalInput")
    sel_d = nc.dram_tensor("sel", [48, DS * 128], bf16, kind="ExternalInput")
    ident_d = nc.dram_tensor("ident", [128, 128], bf16, kind="ExternalInput")
    wx_d = {}
    wdt_d = {}
    A_d = {}
    cw_d = {}
    cwdiag_d = {}
    cb_d = {}
    db_d = {}
    D_d = {}
    Ddiag_d = {}
    for d in ("f", "r"):
        wx_d[d] = nc.dram_tensor(f"wx_{d}", [CH, 128], bf16, kind="ExternalInput")
        wdt_d[d] = nc.dram_tensor(f"wdt_{d}", [DR, CH], bf16, kind="ExternalInput")
        A_d[d] = nc.dram_tensor(f"A_{d}", [128, NCH * DS], f32, kind="ExternalInput")
        cw_d[d] = nc.dram_tensor(f"cw_{d}", [128, NCH * DC], f32,
                                 kind="ExternalInput")
        cwdiag_d[d] = nc.dram_tensor(f"cwdiag_{d}", [NCH * DC, 128, 128], bf16,
                                     kind="ExternalInput")
        cb_d[d] = nc.dram_tensor(f"cb_{d}", [128, NCH], f32, kind="ExternalInput")
        db_d[d] = nc.dram_tensor(f"db_{d}", [128, NCH], f32, kind="ExternalInput")
        D_d[d] = nc.dram_tensor(f"D_{d}", [128, NCH], f32, kind="ExternalInput")
        Ddiag_d[d] = nc.dram_tensor(f"Ddiag_{d}", [NCH, 128, 128], bf16,
                                    kind="ExternalInput")

    pout = nc.dram_tensor("pout", [L, DM], bf16, kind="ExternalOutput")
    dbg = {}
    if debug:
        dbg["xc_f"] = nc.dram_tensor("dbg_xc_f", [NCH, 128, L], bf16, kind="ExternalOutput")
        dbg["xc_r"] = nc.dram_tensor("dbg_xc_r", [NCH, 128, L], bf16, kind="ExternalOutput")
        dbg["xdbl_f"] = nc.dram_tensor("dbg_xdbl_f", [96, L], f32, kind="ExternalOutput")
        dbg["xdbl_r"] = nc.dram_tensor("dbg_xdbl_r", [96, L], f32, kind="ExternalOutput")
        dbg["dt_f"] = nc.dram_tensor("dbg_dt_f", [NCH, 128, L], f32, kind="ExternalOutput")
        dbg["y_f"] = nc.dram_tensor("dbg_y_f", [NCH, 128, L], bf16, kind="ExternalOutput")
        dbg["siluz"] = nc.dram_tensor("dbg_siluz", [NCH, 128, L], bf16, kind="ExternalOutput")

    with tile.TileContext(nc, num_cores=8) as tc:
        _build_tile_program(
            nc, tc, tile, mybir, f32, f32r, bf16, AF, OP,
            hT, winxT, winzT, woutT_d, sel_d, ident_d, wx_d, wdt_d, A_d, cw_d,
            cwdiag_d, cb_d, db_d, D_d, Ddiag_d, pout, dbg, collective,
        )

    _split_sync_waits(nc, mybir)
    return nc


def _build_tile_program(
    nc, tc, tile, mybir, f32, f32r, bf16, AF, OP,
    hT, winxT, winzT, woutT_d, sel_d, ident_d, wx_d, wdt_d, A_d, cw_d,
    cwdiag_d, cb_d, db_d, D_d, Ddiag_d, pout, dbg, collective=True,
):
    from contextlib import ExitStack

    MM = nc.tensor.matmul
    ACT = nc.scalar.activation
    TT = nc.vector.tensor_tensor
    STT = nc.vector.scalar_tensor_tensor
    TSMUL = nc.vector.tensor_scalar_mul

    def veng(code):
        return nc.vector if code == "V" else nc.gpsimd

    def bcopy(code, out, in_):
        """PSUM f32 -> SBUF copy on the chosen engine."""
        if code == "A":
            ACT(out, in_, AF.Copy)
        else:
            veng(code).tensor_copy(out, in_)

    ctx = ExitStack()
    with ctx:
        # -------- persistent pools --------
        pers = ctx.enter_context(tc.tile_pool(name="pers", bufs=1))
        dram = ctx.enter_context(tc.tile_pool(name="dram", bufs=1, space="DRAM"))

        # pass-2-only parameters: tiles declared here, loads DEFERRED into
        # the AllReduce gap so pass-1's first-chunk loads go first
        wout_sb = pers.tile([128, NCH, DM], bf16)
        sel_sb = pers.tile([48, DS * 128], bf16)
        ident_sb = pers.tile([128, 128], bf16)
        xdbl = {}      # bf16 [128, L]: rows [0:16]=B, [32:48]=C, [64:128]=dt-rank
        carry = {}
        wdt_sb = {}
        A_sb = {}
        db_sb = {}
        D_sb = {}
        Ddiag_sb = {}
        for d in ("f", "r"):
            xdbl[d] = pers.tile([128, L], bf16, name=f"xdbl_{d}")
            carry[d] = pers.tile([128, NCH, DS], bf16, name=f"carry_{d}")
            wdt_sb[d] = pers.tile([128, CH], bf16, name=f"wdt_sb_{d}")
            A_sb[d] = pers.tile([128, NCH, DS], f32, name=f"A_sb_{d}")
            db_sb[d] = pers.tile([128, NCH], f32, name=f"db_sb_{d}")
            D_sb[d] = pers.tile([128, NCH], f32, name=f"D_sb_{d}")
            Ddiag_sb[d] = pers.tile([128, NCH, 128], bf16, name=f"Ddiag_sb_{d}")
        ones = pers.tile([128, 1], f32)

        def load_pass2_params():
            nc.sync.dma_start(wout_sb[:],
                              woutT_d.ap().rearrange("(k p) n -> p k n", p=128))
            nc.sync.dma_start(sel_sb[:], sel_d[:])
            nc.sync.dma_start(ident_sb[:], ident_d[:])
            for d in ("f", "r"):
                nc.vector.memset(carry[d][:], 0.0)
                nc.sync.dma_start(wdt_sb[d][DR:128, :], wdt_d[d][:])
                nc.sync.dma_start(
                    A_sb[d][:], A_d[d].ap().rearrange("p (m s) -> p m s", m=NCH))
                nc.sync.dma_start(db_sb[d][:], db_d[d][:])
                nc.sync.dma_start(D_sb[d][:], D_d[d][:])
                nc.sync.dma_start(
                    Ddiag_sb[d][:],
                    Ddiag_d[d].ap().rearrange("m p n -> p m n"))
            nc.vector.memset(ones[:], 1.0)

        # DRAM spill buffers (per-core local HBM); all in ORIGINAL time order
        # for the forward direction; xr/sz are original-time too (pass 2r
        # flips with reversed SBUF reads).  ygr is in flipped time.
        xf_dram = dram.tile([NCH, 128, L], bf16)
        xr_dram = dram.tile([NCH, 128, L], bf16)
        sz_dram = dram.tile([NCH, 128, L], bf16)
        ygr_dram = dram.tile([NCH, 128, L], bf16)
        # AllReduce staging: [dir, 128 rows, L] f32; rows as xdbl layout.
        # dir 0 = forward (original time), dir 1 = reverse (flipped time).
        ar_in = dram.tile([2, 128, L], f32)
        ar_out = dram.tile([2, 128, L], f32)

        # ================= PASS 1: in_proj + conv + silu + partial x_dbl ====
        with tc.tile_pool(name="p1", bufs=1) as p1, \
             tc.tile_pool(name="p1psum", bufs=1, space="PSUM") as p1psum:
            winx_sb = p1.tile([128, DM // 128, CH], bf16)
            nc.sync.dma_start(winx_sb[:], winxT.ap().rearrange("(k p) n -> p k n", p=128))
            winz_sb = p1.tile([128, DM // 128, CH], bf16)
            wx_sb = {}
            cw_sb = {}
            cb_sb = {}
            for d in ("f", "r"):
                wx_sb[d] = p1.tile([128, NCH, 128], bf16, name=f"wx_sb_{d}")
                nc.sync.dma_start(wx_sb[d][:], wx_d[d].ap().rearrange("(m p) n -> p m n", p=128))
                if CFG["conv"] == "E":
                    cw_sb[d] = p1.tile([128, NCH * DC, 128], bf16,
                                       name=f"cw_sb_{d}")
                    nc.sync.dma_start(
                        cw_sb[d][:], cwdiag_d[d].ap().rearrange("k p n -> p k n"))
                else:
                    cw_sb[d] = p1.tile([128, NCH, DC], f32, name=f"cw_sb_{d}")
                    nc.sync.dma_start(
                        cw_sb[d][:], cw_d[d].ap().rearrange("p (m j) -> p m j", m=NCH))
                cb_sb[d] = p1.tile([128, NCH], f32, name=f"cb_sb_{d}")
                nc.sync.dma_start(cb_sb[d][:], cb_d[d][:])

            hT_r = hT.ap().rearrange("(k p) l -> p k l", p=128)
            prev_xe = [None] * NCH
            prev_hTt = None

            def emit_z(cc, hTt_cc):
                for m in range(NCH):
                    ps = p1psum.tile([128, T1], f32, tag="ps_ip",
                                     bufs=CFG["p1bufs"], name=f"psz_{cc}_{m}")
                    for ko in range(DM // 128):
                        MM(ps[:], winz_sb[:, ko, m * 128 : (m + 1) * 128],
                           hTt_cc[:, ko, :],
                           start=(ko == 0), stop=(ko == DM // 128 - 1))
                    zs = p1.tile([128, T1], bf16, tag=f"zs{m}", bufs=2,
                                 name=f"zs{m}_{cc}")
                    ACT(zs[:], ps[:], AF.Silu)
                    nc.sync.dma_start(sz_dram[m, :, cc * T1 : (cc + 1) * T1], zs[:])
                    if dbg:
                        nc.sync.dma_start(
                            dbg["siluz"][m, :, cc * T1 : (cc + 1) * T1], zs[:]
                        )

            def conv_dir(cc, d, xe_list):
                """Causal (d=f) / anti-causal (d=r) depthwise conv + silu on
                original-time chunk cc, using extended tiles [3|T1|3].
                Conv runs on DVE (tap0 as 4x tensor_scalar, taps 1-3 as
                STT accumulate).  Returns bf16 silu'd tiles."""
                out = []
                for m in range(NCH):
                    xe = xe_list[m]
                    if CFG["conv"] == "E":
                        acc = p1psum.tile([128, T1], f32, tag="cps", bufs=2,
                                          name=f"cps{m}_{d}_{cc}")
                        for j in range(DC):
                            off = j if d == "f" else (6 - j)
                            MM(acc[:], cw_sb[d][:, m * DC + j, :],
                               xe[:, off : off + T1],
                               start=(j == 0), stop=(j == DC - 1))
                    elif CFG["convtree"] == "1":
                        # bf16 product tree: taps 0/2/3 as 4x TSMUL on DVE,
                        # tap 1 as per-partition scaled copy on Act, then
                        # three 2x bf16 adds on DVE
                        tp = []
                        for j in range(DC):
                            off = j if d == "f" else (6 - j)
                            src = xe[:, off : off + T1]
                            wj = cw_sb[d][:, m, j : j + 1]
                            t = p1.tile([128, T1], bf16, tag=f"ct{m}_{j}",
                                        bufs=2, name=f"ct{m}_{j}_{d}_{cc}")
                            if j == 1:
                                ACT(t[:], src, AF.Copy, scale=wj)
                            else:
                                TSMUL(t[:], src, wj)
                            tp.append(t)
                        acc = p1.tile([128, T1], bf16, tag=f"cacc{m}", bufs=2,
                                      name=f"cacc{m}_{d}_{cc}")
                        TT(tp[0][:], tp[0][:], tp[1][:], OP.add)
                        TT(tp[2][:], tp[2][:], tp[3][:], OP.add)
                        TT(acc[:], tp[0][:], tp[2][:], OP.add)
                    else:
                        ce = veng(CFG["convsplit"][m])
                        acc = p1.tile([128, T1], f32, tag=f"cacc{m}", bufs=2,
                                      name=f"cacc{m}_{d}_{cc}")
                        for j in range(DC):
                            off = j if d == "f" else (6 - j)
                            src = xe[:, off : off + T1]
                            wj = cw_sb[d][:, m, j : j + 1]
                            if j == 0:
                                ce.tensor_scalar_mul(acc[:], src, wj)
                            else:
                                ce.scalar_tensor_tensor(
                                    acc[:], src, wj, acc[:], OP.mult, OP.add)
                    xcb = p1.tile([128, T1], bf16, tag=f"xcb{m}_{d}", bufs=2,
                                  name=f"xcb{m}_{d}_{cc}")
                    # reverse direction: write silu output time-flipped so the
                    # spill/x_dbl/pass-2r all see flipped time with contiguous
                    # DMAs and unreversed matmul operands
                    dst = xcb[:, ::-1] if d == "r" else xcb[:]
                    ACT(dst, acc[:], AF.Silu, bias=cb_sb[d][:, m : m + 1])
                    out.append(xcb)
                return out

            def xdbl_chunk(cc, d, xc_tiles):
                # psum rows laid out as [B 0:16 | C 32:48 | dt 64:128]
                # (W_x rows reordered+padded on host); full 128 rows go to AR.
                ps = p1psum.tile([128, T1], f32, tag="psx", bufs=2,
                                 name=f"psx_{d}_{cc}")
                for m in range(NCH):
                    MM(ps[:], wx_sb[d][:, m, :], xc_tiles[m][:],
                       start=(m == 0), stop=(m == NCH - 1))
                stage = p1.tile([128, T1], f32, tag="arstage", bufs=2,
                                name=f"arstage_{d}_{cc}")
                if d == "f":
                    ACT(stage[:], ps[:], AF.Copy)
                    nc.sync.dma_start(
                        ar_in[0, :, cc * T1 : (cc + 1) * T1], stage[:]
                    )
                else:
                    ACT(stage[:], ps[:], AF.Copy)
                    nc.sync.dma_start(
                        ar_in[1, :, L - (cc + 1) * T1 : L - cc * T1], stage[:]
                    )

            def spill_chunk(cc, d, xc_tiles):
                x_dram = xf_dram if d == "f" else xr_dram
                for m in range(NCH):
                    dsl = (slice(cc * T1, (cc + 1) * T1) if d == "f"
                           else slice(L - (cc + 1) * T1, L - cc * T1))
                    nc.sync.dma_start(x_dram[m, :, dsl], xc_tiles[m][:])
                    if dbg:
                        key = "xc_f" if d == "f" else "xc_r"
                        nc.sync.dma_start(
                            dbg[key][m, :, cc * T1 : (cc + 1) * T1], xc_tiles[m][:]
                        )

            def finish_reverse(cc, xe_list):
                xcr = conv_dir(cc, "r", xe_list)
                xdbl_chunk(cc, "r", xcr)
                spill_chunk(cc, "r", xcr)

            for c in range(NC1):
                hTt = p1.tile([128, DM // 128, T1], bf16, tag="hTt", bufs=3,
                              name=f"hTt_{c}")
                nc.sync.dma_start(hTt[:], hT_r[:, :, c * T1 : (c + 1) * T1])
                if c == 0:
                    # z weights are first needed one chunk later; keep the
                    # first hTt chunk ahead of them in the DMA queue
                    nc.sync.dma_start(
                        winz_sb[:],
                        winzT.ap().rearrange("(k p) n -> p k n", p=128))

                # x part (extended with halos) and z part (-> silu -> spill)
                cur_xe = []
                for m in range(NCH):
                    ps = p1psum.tile([128, T1], f32, tag="ps_ip", bufs=CFG["p1bufs"],
                                     name=f"psx_{c}_{m}")
                    for ko in range(DM // 128):
                        MM(ps[:], winx_sb[:, ko, m * 128 : (m + 1) * 128],
                           hTt[:, ko, :], start=(ko == 0), stop=(ko == DM // 128 - 1))
                    xe = p1.tile([128, T1 + 6], bf16, tag=f"xe{m}", bufs=CFG["xebufs"],
                                 name=f"xe{m}_{c}")
                    ACT(xe[:, 3 : 3 + T1], ps[:], AF.Copy)
                    if c == 0:
                        nc.vector.memset(xe[:, 0:3], 0.0)
                    else:
                        nc.vector.tensor_copy(xe[:, 0:3], prev_xe[m][:, T1 : T1 + 3])
                    cur_xe.append(xe)
                if c > 0:
                    # fill previous chunk's right halo, then do its reverse conv
                    for m in range(NCH):
                        nc.vector.tensor_copy(
                            prev_xe[m][:, T1 + 3 : T1 + 6], cur_xe[m][:, 3:6]
                        )
                    finish_reverse(c - 1, prev_xe)

                # forward conv on current chunk
                xcf = conv_dir(c, "f", cur_xe)
                xdbl_chunk(c, "f", xcf)
                spill_chunk(c, "f", xcf)

                # z projection deferred by one chunk: z is consumed only by
                # pass 2, so it stays off pass-1's critical path (the final
                # chunk's z runs inside the AllReduce gap)
                if c > 0:
                    emit_z(c - 1, prev_hTt)
                prev_xe = cur_xe
                prev_hTt = hTt

            for m in range(NCH):
                nc.vector.memset(prev_xe[m][:, T1 + 3 : T1 + 6], 0.0)
            finish_reverse(NC1 - 1, prev_xe)
            load_pass2_params()
            emit_z(NC1 - 1, prev_hTt)

            # -------- AllReduce of x_dbl over the 4 cores of this batch ----
            # reverse direction first: pass 2r starts as soon as its rows
            # are reduced, overlapping the forward AR
            for di, d in ((1, "r"), (0, "f")):
                if collective:
                    nc.gpsimd.collective_compute(
                        "AllReduce", OP.add,
                        replica_groups=[[0, 1, 2, 3], [4, 5, 6, 7]],
                        ins=[ar_in[di].opt()], outs=[ar_out[di].opt()],
                    )
                else:
                    nc.gpsimd.dma_start(ar_out[di], ar_in[di])
                # cast-readback f32 -> bf16 into SBUF (gpsimd DMAs may cast)
                nc.gpsimd.dma_start(xdbl[d][:], ar_out[di, :, :])
            if dbg:
                for di, d in enumerate(("f", "r")):
                    nc.sync.dma_start(dbg[f"xdbl_{d}"][0:64, :], ar_out[di, 64:128, :])
                    nc.sync.dma_start(dbg[f"xdbl_{d}"][64:80, :], ar_out[di, 0:16, :])
                    nc.sync.dma_start(dbg[f"xdbl_{d}"][80:96, :], ar_out[di, 32:48, :])

        # ================= PASS 2: dt + selective scan (+gating, out_proj) ==
        def scan_pass(d, p2, p2psum, ytot_cb, mmt_bufs=3):
            """d: 'f' or 'r'.  'r' reads x/sz spills (original time) with
            reversed SBUF access; everything else runs in flipped time.
            ytot_cb(c2, yg_tiles): consumes gated y tiles for chunk c2."""
            x_dram = xf_dram if d == "f" else xr_dram
            rev = (lambda ap: ap) if d == "f" else (lambda ap: ap[:, ::-1])
            for c2 in range(NC2):
                sl = slice(c2 * T2, (c2 + 1) * T2)
                osl = sl if d == "f" else slice(L - (c2 + 1) * T2, L - c2 * T2)
                # ---- dt projection + softplus (f32 path) ----
                dt_sb = []
                for m in range(NCH):
                    psd = p2psum.tile([128, T2], f32, tag="mmt", bufs=mmt_bufs,
                                      name=f"psd_{d}_{c2}_{m}")
                    MM(psd[:], wdt_sb[d][DR:128, m * 128 : (m + 1) * 128],
                       xdbl[d][DR:128, sl], start=True, stop=True)
                    et = p2.tile([128, T2], f32, tag="et", bufs=2,
                                 name=f"et_{d}_{c2}_{m}")
                    ACT(et[:], psd[:], AF.Exp, bias=db_sb[d][:, m : m + 1])
                    dt = p2.tile([128, T2], bf16, tag=f"dt{m}", bufs=2,
                                 name=f"dt{m}_{d}_{c2}")
                    ACT(dt[:], et[:], AF.Ln, bias=ones[:])
                    dt_sb.append(dt)
                    if dbg and d == "f":
                        nc.sync.dma_start(dbg["dt_f"][m, :, sl], dt[:])
                # ---- x load (bf16) + wd = dt*x + silu(z) load ----
                xd = []
                wd = []
                szt = []
                for m in range(NCH):
                    xt = p2.tile([128, T2], bf16, tag=f"xd{m}", bufs=2,
                                 name=f"xd{m}_{d}_{c2}")
                    # xr spill is already in flipped time: load the pass-time
                    # slice directly, no reversed reads needed
                    nc.sync.dma_start(xt[:], x_dram[m, :, sl])
                    xd.append(xt)
                    wt = p2.tile([128, T2], bf16, tag=f"wd{m}", bufs=2,
                                 name=f"wd{m}_{d}_{c2}")
                    veng(CFG["wd"]).tensor_tensor(
                        wt[:], dt_sb[m][:], xt[:], OP.mult)
                    wd.append(wt)
                    sz = p2.tile([128, T2], bf16, tag=f"sz{m}", bufs=2,
                                 name=f"sz{m}_{d}_{c2}")
                    nc.sync.dma_start(sz[:], sz_dram[m, :, osl])
                    szt.append(sz)
                # ---- selective scan over 16 states ----
                yps = [p2psum.tile([128, T2], f32, tag=f"yp{m}", bufs=1,
                                   name=f"yp{m}_{d}_{c2}") for m in range(NCH)]

                def bc_bcast(s):
                    """sel-matmul broadcast + copy for state s -> (Bb, Cb)."""
                    Bbp = p2psum.tile([128, T2], f32, tag="mmt", bufs=mmt_bufs,
                                      name=f"Bbp_{d}_{c2}_{s}")
                    MM(Bbp[:], sel_sb[0:DS, s * 128 : (s + 1) * 128],
                       xdbl[d][0:DS, sl], start=True, stop=True)
                    Bb = p2.tile([128, T2], bf16, tag="Bbs", bufs=CFG["hotbufs"],
                                 name=f"Bb_{d}_{c2}_{s}")
                    bcopy(CFG["bcopy"][s], Bb[:], Bbp[:])
                    Cbp = p2psum.tile([128, T2], f32, tag="mmt", bufs=mmt_bufs,
                                      name=f"Cbp_{d}_{c2}_{s}")
                    MM(Cbp[:], sel_sb[32 : 32 + DS, s * 128 : (s + 1) * 128],
                       xdbl[d][32 : 32 + DS, sl], start=True, stop=True)
                    Cb = p2.tile([128, T2], bf16, tag="Cbs", bufs=CFG["hotbufs"],
                                 name=f"Cb_{d}_{c2}_{s}")
                    bcopy(CFG["ccopy"][s], Cb[:], Cbp[:])
                    return Bb, Cb

                nxt_bc = bc_bcast(0)
                for s in range(DS):
                    Bb, Cb = nxt_bc
                    bt = []
                    for m in range(NCH):
                        b = p2.tile([128, T2], bf16, tag=f"bt{m}", bufs=CFG["hotbufs"],
                                    name=f"bt_{d}_{c2}_{s}_{m}")
                        veng(CFG["bt"][s * NCH + m]).tensor_tensor(
                            b[:], wd[m][:], Bb[:], OP.mult)
                        bt.append(b)
                    dAs = []
                    for m in range(NCH):
                        dA = p2.tile([128, T2], f32, tag=f"dA{m}", bufs=CFG["hotbufs"],
                                     name=f"dA_{d}_{c2}_{s}_{m}")
                        ACT(dA[:], dt_sb[m][:], AF.Exp,
                            scale=A_sb[d][:, m, s : s + 1])
                        dAs.append(dA)
                    # lookahead: issue next state's broadcasts ahead of the
                    # yacc matmuls so PE's in-order queue can't stall them
                    # behind cmul-dependent work
                    if s + 1 < DS:
                        nxt_bc = bc_bcast(s + 1)
                    # per-state hs tile holding all 4 channel groups, so the
                    # chunk-boundary carry is ONE strided copy per state
                    hs = p2.tile([128, NCH, T2], bf16, tag="hs", bufs=2,
                                 name=f"hs_{d}_{c2}_{s}")
                    for m in range(NCH):
                        nc.vector.tensor_tensor_scan(
                            hs[:, m, :], dAs[m][:], bt[m][:],
                            carry[d][:, m, s : s + 1], OP.mult, OP.add)
                    veng(CFG["carry"]).tensor_copy(
                        carry[d][:, :, s : s + 1], hs[:, :, T2 - 1 : T2])
                    for m in range(NCH):
                        cm = p2.tile([128, T2], bf16, tag=f"cm{m}", bufs=CFG["hotbufs"],
                                     name=f"cm_{d}_{c2}_{s}_{m}")
                        veng(CFG["cm"][s * NCH + m]).tensor_tensor(
                            cm[:], hs[:, m, :], Cb[:], OP.mult)
                        MM(yps[m][:], ident_sb[:], cm[:],
                           start=(s == 0), stop=False)
                # ---- gating: y = (ypsum + x*D) * silu(z) ----
                yg = []
                for m in range(NCH):
                    # x*D skip joins the PSUM accumulation as a diag matmul
                    MM(yps[m][:], Ddiag_sb[d][:, m, :], xd[m][:],
                       start=False, stop=True)
                    yt = p2.tile([128, T2], bf16, tag=f"yg{m}", bufs=2,
                                 name=f"yg_{d}_{c2}_{m}")
                    veng(CFG["gate"]).tensor_tensor(
                        yt[:], yps[m][:], rev(szt[m][:]), OP.mult)
                    yg.append(yt)
                ytot_cb(c2, yg)

        # ---- pass 2: reverse (spill gated y), then forward (combine +
        # out_proj); one shared pool so the passes overlap at the seam ----
        with tc.tile_pool(name="p2", bufs=1) as p2f, \
             tc.tile_pool(name="p2psum", bufs=1, space="PSUM") as p2fpsum:

            def spill_ygr(c2, yg):
                for m in range(NCH):
                    nc.sync.dma_start(
                        ygr_dram[m, :, c2 * T2 : (c2 + 1) * T2], yg[m][:]
                    )

            scan_pass("r", p2f, p2fpsum, spill_ygr, mmt_bufs=2)

            def combine_out(c2, yg):
                ytot = []
                for m in range(NCH):
                    ygr_t = p2f.tile([128, T2], bf16, tag=f"ygr{m}", bufs=2,
                                     name=f"ygr{m}_{c2}")
                    nc.sync.dma_start(
                        ygr_t[:], ygr_dram[m, :, L - (c2 + 1) * T2 : L - c2 * T2]
                    )
                    yt2 = p2f.tile([128, T2], bf16, tag=f"ytot{m}", bufs=2,
                                   name=f"ytot{m}_{c2}")
                    veng(CFG["comb"]).tensor_tensor(
                        yt2[:], yg[m][:], ygr_t[:, ::-1], OP.add)
                    ytot.append(yt2)
                    if dbg:
                        nc.sync.dma_start(
                            dbg["y_f"][m, :, c2 * T2 : (c2 + 1) * T2], yg[m][:]
                        )
                for mt in range(T2 // 128):
                    ob = p2f.tile([128, DM], bf16, tag="ob", bufs=2,
                                  name=f"ob_{c2}_{mt}")
                    for nh in range(DM // 512):
                        po = p2fpsum.tile([128, 512], f32, tag="po", bufs=2,
                                          name=f"po_{c2}_{mt}_{nh}")
                        for k in range(NCH):
                            MM(po[:], ytot[k][:, mt * 128 : (mt + 1) * 128],
                               wout_sb[:, k, nh * 512 : (nh + 1) * 512],
                               start=(k == 0), stop=(k == NCH - 1))
                        bcopy(CFG["obcopy"], ob[:, nh * 512 : (nh + 1) * 512],
                              po[:])
                    nc.sync.dma_start(
                        pout[c2 * T2 + mt * 128 : c2 * T2 + (mt + 1) * 128, :],
                        ob[:],
                    )

            scan_pass("f", p2f, p2fpsum, combine_out, mmt_bufs=2)


def _host_prep(inputs):
    """Slice/transpose the full inputs into the 8 per-core input maps."""
    import ml_dtypes
    bf = ml_dtypes.bfloat16

    h = np.asarray(inputs["hidden_states"], np.float32)
    W_in = np.asarray(inputs["W_in"], np.float32)
    W_out = np.asarray(inputs["W_out"], np.float32)

    sel = np.zeros((48, DS * 128), np.float32)
    for s in range(DS):
        sel[s, s * 128 : (s + 1) * 128] = 1.0
        sel[32 + s, s * 128 : (s + 1) * 128] = 1.0

    maps = []
    for core in range(8):
        b, g = divmod(core, 4)
        c0 = g * CH
        m = {
            "hT": np.ascontiguousarray(h[b].T).astype(bf),
            "winxT": np.ascontiguousarray(W_in[c0 : c0 + CH, :].T).astype(bf),
            "winzT": np.ascontiguousarray(W_in[DI + c0 : DI + c0 + CH, :].T).astype(bf),
            "woutT": np.ascontiguousarray(W_out[:, c0 : c0 + CH].T).astype(bf),
            "sel": sel.astype(bf),
            "ident": np.eye(128, dtype=np.float32).astype(bf),
        }
        for d in ("f", "r"):
            sfx = f"_{d}"
            W_x = np.asarray(inputs[f"W_x{sfx}"], np.float32)
            W_dt = np.asarray(inputs[f"W_dt{sfx}"], np.float32)
            A = -np.exp(np.asarray(inputs[f"A_log{sfx}"], np.float64)).astype(np.float32)
            cw = np.asarray(inputs[f"conv_w{sfx}"], np.float32)
            cb = np.asarray(inputs[f"conv_b{sfx}"], np.float32)
            db = np.asarray(inputs[f"b_dt{sfx}"], np.float32)
            Dp = np.asarray(inputs[f"D{sfx}"], np.float32)
            wx_re = np.zeros((CH, 128), np.float32)
            wx_re[:, 0:DS] = W_x[DR : DR + DS, c0 : c0 + CH].T        # B rows
            wx_re[:, 32 : 32 + DS] = W_x[DR + DS : 96, c0 : c0 + CH].T  # C rows
            wx_re[:, DR:128] = W_x[0:DR, c0 : c0 + CH].T              # dt-rank rows
            m[f"wx{sfx}"] = wx_re.astype(bf)
            m[f"wdt{sfx}"] = np.ascontiguousarray(W_dt[c0 : c0 + CH, :].T).astype(bf)
            # (CH, DS) -> (128, NCH, DS) -> (128, NCH*DS)
            m[f"A{sfx}"] = np.ascontiguousarray(
                A[c0 : c0 + CH].reshape(NCH, 128, DS).transpose(1, 0, 2).reshape(128, NCH * DS)
            )
            m[f"cw{sfx}"] = np.ascontiguousarray(
                cw[c0 : c0 + CH].reshape(NCH, 128, DC).transpose(1, 0, 2).reshape(128, NCH * DC)
            )
            cwd = np.zeros((NCH * DC, 128, 128), np.float32)
            cwc = cw[c0 : c0 + CH].reshape(NCH, 128, DC)
            for mm_ in range(NCH):
                for j in range(DC):
                    np.fill_diagonal(cwd[mm_ * DC + j], cwc[mm_, :, j])
            m[f"cwdiag{sfx}"] = cwd.astype(bf)
            m[f"cb{sfx}"] = np.ascontiguousarray(
                cb[c0 : c0 + CH].reshape(NCH, 128).T
            )
            m[f"db{sfx}"] = np.ascontiguousarray(
                db[c0 : c0 + CH].reshape(NCH, 128).T
            )
            m[f"D{sfx}"] = np.ascontiguousarray(
                Dp[c0 : c0 + CH].reshape(NCH, 128).T
            )
            dd = np.zeros((NCH, 128, 128), np.float32)
            for mm_ in range(NCH):
                np.fill_diagonal(dd[mm_], Dp[c0 + mm_ * 128 : c0 + (mm_ + 1) * 128])
            m[f"Ddiag{sfx}"] = dd.astype(bf)
        maps.append(m)
    return maps


def run(inputs, debug=False, trace=False):
    from concourse.bass_utils import run_bass_kernel_spmd

    if _COMPILED[0] is None or _COMPILED[0][1] != debug:
        _COMPILED[0] = (_build_program(debug=debug), debug)
    nc = _COMPILED[0][0]
    maps = _host_prep(inputs)
    res = run_bass_kernel_spmd(nc, maps, core_ids=list(range(8)), trace=trace)
    outs = [np.asarray(r["pout"], np.float32) for r in res.results]
    full = np.zeros((B, L, DM), np.float32)
    for core in range(8):
        b = core // 4
        full[b] += outs[core]
    return full, res


def kernel(**inputs):
    out, _ = run(inputs, debug=False, trace=False)
    return out
